# revision 6
# baseline (speedup 1.0000x reference)
"""Trainium2 Bass kernel for nn_BerTII (masked-mean embedding bag -> 1-dim
linear -> sigmoid), distributed over 8 NeuronCores.

reference math:
  mask[b,l] = l < lengths[b]
  pooled[b,:] = sum_l mask[b,l] * emb[tokens[b,l],:] / L
  out[b] = sigmoid(pooled[b,:] @ W.T + bias)

The 1-output linear commutes with the masked mean:
  out[b] = sigmoid( (1/L) * sum_{l<len_b} (emb[tokens[b,l]] . W) + bias )
so the kernel never materializes the [B,L,P] gather. Host-side marshaling is
integer-only index work (the "all-to-all" of the sharding hint done at
input-staging time):
  - flatten all valid (b,l) tokens, dedupe globally (np.unique) and build a
    per-(unique-row, batch) multiplicity matrix;
  - split the unique rows into 8 equal-count contiguous chunks; core c
    receives ONLY the vocab slice spanning its chunk (rebased int16 indices),
    so the 800MB table is sharded across cores, not replicated; rows are
    padded 1000->1024 floats so each row is one 4KB 256B-aligned gather
    element;
  - each core bulk-gathers its ~U/8 rows with InstDMAGatherAnt, dots each row
    with W on the Vector engine (scalar_tensor_tensor accum), and accumulates
    per-batch partial dot products with tiny PE matmuls against the
    multiplicity matrix (y stationary [128,1], counts moving [128,64]);
  - an 8-core AllReduce(add) of the [64] partials, then sigmoid(x/L + b) on
    the Scalar engine. Every core emits the full [64] output; core 0's is
    returned.

DEFAULT (BERT_SHARD=seq): the sequence-ownership variant at the bottom of this
file instead — each core owns 8 length-balanced sequences end-to-end (table
replicated in bf16, int16 gathers windowed into 32768-row vocab slabs, no
collective), which removes cross-core straggler waits: ~129 us vs ~135-142 us
for the vocab-sharded path (BERT_SHARD=vocab).
"""
import os
import sys

sys.path.insert(0, "/opt/trn_rl_repo")

import numpy as np

VOCAB = 200000
PDIM = 1000
PDIMP = 1024  # row stride padded to 256B multiple for dma_gather
B = 64
L = 2048
NCORES = 8

LAST = {}  # debug: last BassKernelResults etc.


# ---------------------------------------------------------------------------
# walrus legalization: this toolchain allows at most ONE semaphore wait per
# instruction ("Too many sync wait commands"); split extras onto NoOps.
def _legalize_sem_waits(nc, mybir, max_waits=1):
    n = 0
    for f in nc.m.functions:
        for bb in f.blocks:
            new = []
            for inst in bb.instructions:
                si = inst.sync_info
                if si is not None and si.on_wait and len(si.on_wait) > max_waits:
                    waits = list(si.on_wait)
                    extra, keep = waits[:-max_waits], waits[-max_waits:]
                    k = 0
                    while extra:
                        chunk, extra = extra[:max_waits], extra[max_waits:]
                        new.append(
                            mybir.InstNoOp(
                                name=f"{inst.name}-ws{k}",
                                sync_info=mybir.SyncInfo(on_wait=chunk, on_update=[]),
                                bass_nofuse=True,
                                engine=inst.engine,
                            )
                        )
                        k += 1
                        n += 1
                    si.on_wait = keep
                new.append(inst)
            bb.instructions[:] = new
    return n


def _build(Vmax, T, chunk, gbufs, mode="dmag", legalize=True, ybufs=16, ramp_ind=0, cc="ag", dtype="f32", compute="pe"):
    from concourse import bass, bacc, mybir
    import concourse.tile as tile
    from concourse.tile import add_dep_helper

    F32 = mybir.dt.float32
    GDT = mybir.dt.bfloat16 if dtype == "bf16" else F32
    I16 = mybir.dt.int16
    I32 = mybir.dt.int32

    nc = bacc.Bacc(None, num_devices=NCORES)
    emb = nc.declare_dram_parameter("emb", [Vmax, PDIMP], GDT, isOutput=False)
    # idx16: gather index i of this core lives at [i % 16, i // 16], rows
    # replicated x8 down the partition dim (one copy per Q7 band).
    idx16 = nc.declare_dram_parameter("idx16", [128, T * 8], I16, isOutput=False)
    idx32 = nc.declare_dram_parameter("idx32", [128, T], I32, isOutput=False)
    SELDT = GDT if compute in ("pe", "split") else F32
    sel = nc.declare_dram_parameter("sel", [128, T * B], SELDT, isOutput=False)
    WDT = F32 if compute in ("pe", "split") else GDT
    wrep = nc.declare_dram_parameter("wrep", [128, PDIM], WDT, isOutput=False)
    brep = nc.declare_dram_parameter("brep", [1, 1], F32, isOutput=False)
    outp = nc.declare_dram_parameter("out", [1, B], F32, isOutput=True)

    with tile.TileContext(nc) as tc:
        with (
            tc.tile_pool(name="meta", bufs=1) as meta,
            tc.tile_pool(name="g", bufs=gbufs) as gp,
            tc.tile_pool(name="y", bufs=ybufs) as yp,
            tc.tile_pool(name="ps", bufs=1, space="PSUM") as pp,
            tc.tile_pool(name="dram", bufs=1, space="DRAM") as dp,
        ):
            idx16_sb = meta.tile([128, T * 8], I16)
            nc.sync.dma_start(out=idx16_sb[:], in_=idx16[:])
            idx32_sb = meta.tile([128, T], I32)
            nc.sync.dma_start(out=idx32_sb[:], in_=idx32[:])
            sel_sb = meta.tile([128, T * B], SELDT)
            nc.sync.dma_start(out=sel_sb[:], in_=sel[:])
            w_sb = meta.tile([128, PDIM], WDT)
            nc.sync.dma_start(out=w_sb[:], in_=wrep[:])
            b_sb = meta.tile([1, 1], F32)
            nc.sync.dma_start(out=b_sb[:], in_=brep[:])

            # warmup collective: absorb ncfw rendezvous/setup concurrently
            # with the gather pipeline so the real AllReduce at the end is
            # cheap.
            if os.environ.get("BERT_CCWARM", "1") == "1":
                warm_sb = meta.tile([1, 4], F32)
                nc.vector.memset(warm_sb[:], 0.0)
                ccw_in = dp.tile([1, 4], F32)
                ccw_out = dp.tile([NCORES if cc == "ag" else 1, 4], F32)
                nc.sync.dma_start(out=ccw_in[:], in_=warm_sb[:])
                nc.gpsimd.collective_compute(
                    "AllGather" if cc == "ag" else "AllReduce",
                    mybir.AluOpType.bypass if cc == "ag" else mybir.AluOpType.add,
                    replica_groups=[list(range(NCORES))],
                    ins=[ccw_in[:]],
                    outs=[ccw_out[:]],
                )

            dot_ps = pp.tile([1, B], F32)
            HALF = PDIM // 2
            pool_a = pp.tile([B, HALF], F32, tag="pa")
            pool_b = pp.tile([B, HALF], F32, tag="pb")
            if compute == "pe":
                pe_set = set(range(T))
            elif compute == "split":
                pe_set = set(range(1, T, 2))
            else:
                pe_set = set()
            stt_set = set(range(T)) - pe_set
            pe_lo, pe_hi = (min(pe_set), max(pe_set)) if pe_set else (0, 0)
            st_lo, st_hi = (min(stt_set), max(stt_set)) if stt_set else (0, 0)
            YDT = GDT if compute == "split" else F32
            if compute == "split":
                w16 = meta.tile([128, PDIM], GDT)
                nc.vector.tensor_copy(out=w16[:], in_=w_sb[:])
            else:
                w16 = w_sb

            def consume(gflat, off, t):
                """gflat: [128, >=off+PDIM] gathered rows tile; tile index t."""
                if t in pe_set:
                    # pooled[b,:] += sel_t[:,b]^T @ G ; W applied once at the end
                    nc.tensor.matmul(
                        out=pool_a[:],
                        lhsT=sel_sb[:, t * B : (t + 1) * B],
                        rhs=gflat[:, off : off + HALF],
                        start=(t == pe_lo),
                        stop=(t == pe_hi),
                    )
                    nc.tensor.matmul(
                        out=pool_b[:],
                        lhsT=sel_sb[:, t * B : (t + 1) * B],
                        rhs=gflat[:, off + HALF : off + PDIM],
                        start=(t == pe_lo),
                        stop=(t == pe_hi),
                    )
                    return
                y = yp.tile([128, 1], YDT)
                gs = gflat[:, off : off + PDIM]
                nc.vector.scalar_tensor_tensor(
                    out=gs,
                    in0=gs,
                    scalar=1.0,
                    in1=w16[:],
                    op0=mybir.AluOpType.mult,
                    op1=mybir.AluOpType.mult,
                    accum_out=y[:],
                )
                nc.tensor.matmul(
                    out=dot_ps[:],
                    lhsT=y[:],
                    rhs=sel_sb[:, t * B : (t + 1) * B],
                    start=(t == st_lo),
                    stop=(t == st_hi),
                )

            if mode == "dmag":
                # ramp-in: first tiles as single-row-set indirect gathers (low
                # latency), remainder as bulk dma_gather chunks (low overhead)
                nramp = min(ramp_ind, T)
                ramp_insts = []
                for t in range(nramp):
                    gi = gp.tile([128, PDIMP], GDT, tag="gi")
                    gi_inst = nc.gpsimd.indirect_dma_start(
                        out=gi[:],
                        out_offset=None,
                        in_=emb[:],
                        in_offset=bass.IndirectOffsetOnAxis(
                            ap=idx32_sb[:, t : t + 1], axis=0
                        ),
                    )
                    # keep the low-latency ramp singles in issue order
                    if ramp_insts:
                        add_dep_helper(gi_inst.ins, ramp_insts[-1].ins, reason="ramp order")
                    ramp_insts.append(gi_inst)
                    consume(gi[:], 0, t)
                sched = []
                rem = T - nramp
                while rem > 0:
                    c = min(chunk, rem)
                    sched.append(c)
                    rem -= c
                s = nramp
                first_dmag = True
                for c in sched:
                    g = gp.tile([128, c, PDIMP], GDT, tag="g")
                    dg_inst = nc.gpsimd.dma_gather(
                        out_ap=g[:],
                        in_ap=emb[:],
                        idxs_ap=idx16_sb[:, s * 8 : (s + c) * 8],
                        num_idxs=c * 128,
                        num_idxs_reg=c * 128,
                        elem_size=PDIMP,
                    )
                    if first_dmag and ramp_insts:
                        add_dep_helper(dg_inst.ins, ramp_insts[-1].ins, reason="ramp first")
                        first_dmag = False
                    gflat = g[:].rearrange("p c e -> p (c e)")
                    for j in range(c):
                        consume(gflat, j * PDIMP, s + j)
                    s += c
            else:  # indirect: one [128, PDIMP] row-gather per tile
                for t in range(T):
                    g = gp.tile([128, PDIMP], F32, tag="g")
                    nc.gpsimd.indirect_dma_start(
                        out=g[:],
                        out_offset=None,
                        in_=emb[:],
                        in_offset=bass.IndirectOffsetOnAxis(
                            ap=idx32_sb[:, t : t + 1], axis=0
                        ),
                    )
                    consume(g[:], 0, t)

            if compute in ("pe", "split"):
                pooled_sb = meta.tile([B, PDIM], F32)
                nc.vector.tensor_copy(out=pooled_sb[:, :HALF], in_=pool_a[:])
                nc.vector.tensor_copy(out=pooled_sb[:, HALF:], in_=pool_b[:])
                scr = meta.tile([B, PDIM], F32)
                y64 = meta.tile([B, 1], F32)
                nc.vector.scalar_tensor_tensor(
                    out=scr[:],
                    in0=pooled_sb[:],
                    scalar=1.0,
                    in1=w_sb[:B, :],
                    op0=mybir.AluOpType.mult,
                    op1=mybir.AluOpType.mult,
                    accum_out=y64[:],
                )
                if compute == "split":
                    # fold the stt-half partial [1,B] into partition-major form
                    part1_sb = meta.tile([1, B], F32)
                    nc.vector.tensor_copy(out=part1_sb[:], in_=dot_ps[:])
                    ident1 = meta.tile([1, 1], F32)
                    nc.vector.memset(ident1[:], 1.0)
                    dot_t = pp.tile([B, 1], F32, tag="dt")
                    nc.tensor.transpose(out=dot_t[:], in_=part1_sb[:], identity=ident1[:])
                    both = meta.tile([B, 1], F32)
                    nc.vector.tensor_tensor(
                        out=both[:], in0=y64[:], in1=dot_t[:], op=mybir.AluOpType.add
                    )
                    part_sb = both
                else:
                    part_sb = y64
                cc_in = dp.tile([B, 1], F32)
            else:
                part_sb = meta.tile([1, B], F32)
                nc.vector.tensor_copy(out=part_sb[:], in_=dot_ps[:])
                cc_in = dp.tile([1, B], F32)
            nc.sync.dma_start(out=cc_in[:], in_=part_sb[:])
            pmajor = compute in ("pe", "split")
            if cc == "ag":
                cc_out = dp.tile([NCORES * B, 1] if pmajor else [NCORES, B], F32)
                nc.gpsimd.collective_compute(
                    "AllGather",
                    mybir.AluOpType.bypass,
                    replica_groups=[list(range(NCORES))],
                    ins=[cc_in[:]],
                    outs=[cc_out[:]],
                )
                allg_sb = meta.tile([NCORES, B], F32)
                nc.sync.dma_start(out=allg_sb[:], in_=cc_out[:].rearrange("a b -> (a b)").rearrange("(c n) -> c n", c=NCORES) if pmajor else cc_out[:])
                ones_sb = meta.tile([NCORES, 1], F32)
                nc.vector.memset(ones_sb[:], 1.0)
                sum_ps = pp.tile([1, B], F32, tag="sum")
                nc.tensor.matmul(
                    out=sum_ps[:],
                    lhsT=ones_sb[:],
                    rhs=allg_sb[:],
                    start=True,
                    stop=True,
                )
                red_ap = sum_ps[:]
            else:
                cc_out = dp.tile([1, B], F32)
                nc.gpsimd.collective_compute(
                    "AllReduce",
                    mybir.AluOpType.add,
                    replica_groups=[list(range(NCORES))],
                    ins=[cc_in[:]],
                    outs=[cc_out[:]],
                )
                red_sb = meta.tile([1, B], F32)
                nc.sync.dma_start(out=red_sb[:], in_=cc_out[:])
                red_ap = red_sb[:]
            o_sb = meta.tile([1, B], F32)
            nc.scalar.activation(
                out=o_sb[:],
                in_=red_ap,
                func=mybir.ActivationFunctionType.Sigmoid,
                bias=b_sb[:],
                scale=1.0 / float(L),
            )
            nc.sync.dma_start(out=outp[:], in_=o_sb[:])

    nc.compile()
    if legalize:
        _legalize_sem_waits(nc, mybir)
    return nc


def _marshal(tokens, lengths, emb_table, W, b, dtype="f32"):
    if dtype == "bf16":
        import ml_dtypes

        sdt = ml_dtypes.bfloat16
    else:
        sdt = np.float32
    tokens = np.asarray(tokens)
    lengths = np.asarray(lengths).astype(np.int64)
    emb_table = np.ascontiguousarray(emb_table, dtype=np.float32)

    mask = np.arange(L)[None, :] < lengths[:, None]
    flat_tok = tokens[mask].astype(np.int64)
    flat_b = np.broadcast_to(np.arange(B)[:, None], (B, L))[mask]
    uniq, inv = np.unique(flat_tok, return_inverse=True)
    U = len(uniq)
    cnt = np.zeros((U, B), dtype=np.float32)
    np.add.at(cnt, (inv, flat_b), 1.0)

    bounds = [U * c // NCORES for c in range(NCORES + 1)]
    rows_max = max(bounds[c + 1] - bounds[c] for c in range(NCORES))
    T = -(-rows_max // 128)

    spans = []
    for c in range(NCORES):
        s, e = bounds[c], bounds[c + 1]
        lo = int(uniq[s]) if e > s else 0
        hi = int(uniq[e - 1]) + 1 if e > s else 1
        spans.append((s, e, lo, hi))
    Vmax = max(hi - lo for _, _, lo, hi in spans)

    wdt = np.float32 if os.environ.get("BERT_COMPUTE", "stt") in ("pe", "split") else sdt
    wrep = np.broadcast_to(
        np.asarray(W, dtype=np.float32).astype(wdt).reshape(1, PDIM), (128, PDIM)
    ).copy()
    brep = np.full((1, 1), np.float32(np.asarray(b).reshape(-1)[0]), dtype=np.float32)

    in_maps = []
    for c in range(NCORES):
        s, e, lo, hi = spans[c]
        span = hi - lo
        emb_c = np.zeros((Vmax, PDIMP), dtype=sdt)
        emb_c[:span, :PDIM] = emb_table[lo:hi].astype(sdt)
        rows = np.zeros(T * 128, dtype=np.int32)
        rows[: e - s] = (uniq[s:e] - lo).astype(np.int32)
        # int16 wrapped layout: index i -> [i % 16, i // 16], replicated x8
        wrapped = rows.astype(np.int16).reshape(T * 8, 16).T  # [16, T*8]
        idx16 = np.tile(wrapped, (8, 1)).copy()  # [128, T*8]
        seldt = sdt if os.environ.get("BERT_COMPUTE", "stt") in ("pe", "split") else np.float32
        selm = np.zeros((T * 128, B), dtype=seldt)
        selm[: e - s] = cnt[s:e].astype(seldt)
        in_maps.append(
            {
                "emb": emb_c,
                "idx16": idx16,
                "idx32": rows.reshape(T, 128).T.copy(),
                "sel": selm.reshape(T, 128, B).transpose(1, 0, 2).reshape(128, T * B).copy(),
                "wrep": wrep,
                "brep": brep,
            }
        )
    return T, Vmax, in_maps


def kernel(tokens, lengths, emb_table, W, b):
    from concourse.bass_utils import run_bass_kernel_spmd

    mode = os.environ.get("BERT_MODE", "dmag")
    chunk = int(os.environ.get("BERT_CHUNK", "8"))
    gbufs = int(os.environ.get("BERT_GBUFS", "4"))
    ybufs = int(os.environ.get("BERT_YBUFS", "16"))
    ramp_ind = int(os.environ.get("BERT_RAMPIND", "0"))
    cc = os.environ.get("BERT_CC", "ag")
    compute = os.environ.get("BERT_COMPUTE", "stt")
    trace = os.environ.get("BERT_TRACE", "0") == "1"

    dtype = os.environ.get("BERT_DTYPE", "bf16")
    T, Vmax, in_maps = _marshal(tokens, lengths, emb_table, W, b, dtype=dtype)
    nc = _build(Vmax, T, chunk, gbufs, mode=mode, ybufs=ybufs, ramp_ind=ramp_ind, cc=cc, dtype=dtype, compute=compute)
    res = run_bass_kernel_spmd(nc, in_maps, core_ids=list(range(NCORES)), trace=trace)
    LAST["results"] = res
    LAST["T"] = T
    LAST["Vmax"] = Vmax
    return res.results[0]["out"].reshape(B).astype(np.float32)

# ---------------------------------------------------------------------------
# Sequence-ownership variant: each core owns 8 sequences end-to-end (no
# collective, no cross-core skew sensitivity). Table replicated in bf16;
# gathers windowed into 32768-row vocab windows so rebased indices fit int16.
WIN = 32768
NW = -(-VOCAB // WIN)
NSEQ = B // NCORES


def _marshal_seq(tokens, lengths, emb_table, W, b, dtype="bf16"):
    import ml_dtypes

    sdt = ml_dtypes.bfloat16 if dtype == "bf16" else np.float32
    tokens = np.asarray(tokens)
    lengths = np.asarray(lengths).astype(np.int64)

    # per-sequence unique-token histograms over vocab windows; greedy
    # vector-balancing assignment minimizes sum_w max_c rows (the padded
    # tile count is driven by per-window maxima, not total length)
    order = np.argsort(-lengths, kind="stable")
    hists = np.zeros((B, NW), dtype=np.int64)
    for bidx in range(B):
        u = np.unique(tokens[bidx, : lengths[bidx]].astype(np.int64))
        hists[bidx] = np.bincount(u // WIN, minlength=NW)
    Wc = np.zeros((NCORES, NW), dtype=np.int64)
    counts = np.zeros(NCORES, dtype=np.int64)
    assign = np.full((NCORES, NSEQ), -1, dtype=np.int64)
    for bidx in order:
        cands = np.where(counts < NSEQ)[0]
        best, bobj = None, None
        for c in cands:
            trial = Wc.copy()
            trial[c] += hists[bidx]
            obj = trial.max(axis=0).sum()
            if bobj is None or obj < bobj:
                best, bobj = c, obj
        assign[best, counts[best]] = bidx
        counts[best] += 1
        Wc[best] += hists[bidx]

    def _obj(Wm):
        return (-(-Wm.max(axis=0) // 128)).sum() * 1000000 + Wm.max(axis=0).sum()

    # swap refinement: directly minimize padded tile count sum_w ceil(max/128)
    for _ in range(40):
        improved = False
        cur = _obj(Wc)
        for c1 in range(NCORES):
            for j1 in range(NSEQ):
                for c2 in range(c1 + 1, NCORES):
                    for j2 in range(NSEQ):
                        b1, b2 = assign[c1, j1], assign[c2, j2]
                        trial = Wc.copy()
                        trial[c1] += hists[b2] - hists[b1]
                        trial[c2] += hists[b1] - hists[b2]
                        if _obj(trial) < cur:
                            assign[c1, j1], assign[c2, j2] = b2, b1
                            Wc = trial
                            cur = _obj(Wc)
                            improved = True
        if not improved:
            break

    per_core_rows = []  # (uniq, cnt8) per core
    for c in range(NCORES):
        toks = np.concatenate(
            [tokens[assign[c, j], : lengths[assign[c, j]]] for j in range(NSEQ)]
        ).astype(np.int64)
        locb = np.concatenate(
            [np.full(int(lengths[assign[c, j]]), j, dtype=np.int64) for j in range(NSEQ)]
        )
        uniq, inv = np.unique(toks, return_inverse=True)
        cnt8 = np.zeros((len(uniq), NSEQ), dtype=np.float32)
        np.add.at(cnt8, (inv, locb), 1.0)
        per_core_rows.append((uniq, cnt8))

    # per-window tile counts, common across cores (SPMD: same program)
    Tw = []
    bnds = []
    for w in range(NW):
        lo, hi = w * WIN, min((w + 1) * WIN, VOCAB)
        per_core_bnd = [
            (np.searchsorted(u, lo), np.searchsorted(u, hi)) for u, _ in per_core_rows
        ]
        bnds.append(per_core_bnd)
        Tw.append(max(-(-int(e - s) // 128) for s, e in per_core_bnd))
    T = sum(Tw)

    emb16 = np.zeros((VOCAB, PDIMP), dtype=sdt)
    emb16[:, :PDIM] = np.ascontiguousarray(emb_table, dtype=np.float32).astype(sdt)
    wdt = np.float32 if os.environ.get("BERT_SEQSPLIT", "1") == "1" else sdt
    wrep = np.broadcast_to(
        np.asarray(W, dtype=np.float32).astype(wdt).reshape(1, PDIM), (128, PDIM)
    ).copy()
    brep = np.full((NSEQ, 1), np.float32(np.asarray(b).reshape(-1)[0]), dtype=np.float32)

    in_maps = []
    for c in range(NCORES):
        uniq, cnt8 = per_core_rows[c]
        rows = np.zeros(T * 128, dtype=np.int16)
        selm = np.zeros((T * 128, NSEQ), dtype=np.float32)
        t0 = 0
        for w in range(NW):
            s0, e0 = bnds[w][c]
            n = int(e0 - s0)
            rows[t0 * 128 : t0 * 128 + n] = (uniq[s0:e0] - w * WIN).astype(np.int16)
            selm[t0 * 128 : t0 * 128 + n] = cnt8[s0:e0]
            t0 += Tw[w]
        if os.environ.get("BERT_SEQSPLIT", "1") == "1":
            selm = selm.astype(sdt)
        wrapped = rows.reshape(T * 8, 16).T  # [16, T*8]
        in_maps.append(
            {
                "emb": emb16,
                "idx16": np.tile(wrapped, (8, 1)).copy(),
                "sel": selm.reshape(T, 128, NSEQ)
                .transpose(1, 0, 2)
                .reshape(128, T * NSEQ)
                .copy(),
                "wrep": wrep,
                "brep": brep,
            }
        )
    return Tw, in_maps, assign


def _build_seq(Tw, chunk, gbufs, ybufs, dtype="bf16", legalize=True, split=True):
    from concourse import bacc, mybir
    import concourse.tile as tile

    F32 = mybir.dt.float32
    GDT = mybir.dt.bfloat16 if dtype == "bf16" else F32
    I16 = mybir.dt.int16
    T = sum(Tw)

    scratch = int(os.environ.get("BERT_DMASCRATCH", "131072"))
    nc = bacc.Bacc(None, num_devices=NCORES, dynamic_dma_scratch_size=scratch)
    emb = nc.declare_dram_parameter("emb", [VOCAB, PDIMP], GDT, isOutput=False)
    idx16 = nc.declare_dram_parameter("idx16", [128, T * 8], I16, isOutput=False)
    SELDT = GDT if split else F32
    sel = nc.declare_dram_parameter("sel", [128, T * NSEQ], SELDT, isOutput=False)
    WDT = F32 if split else GDT
    wrep = nc.declare_dram_parameter("wrep", [128, PDIM], WDT, isOutput=False)
    brep = nc.declare_dram_parameter("brep", [NSEQ, 1], F32, isOutput=False)
    outp = nc.declare_dram_parameter("out", [1, NSEQ], F32, isOutput=True)

    with tile.TileContext(nc) as tc:
        with (
            tc.tile_pool(name="meta", bufs=1) as meta,
            tc.tile_pool(name="g", bufs=gbufs) as gp,
            tc.tile_pool(name="y", bufs=ybufs) as yp,
            tc.tile_pool(name="ps", bufs=1, space="PSUM") as pp,
        ):
            idx16_sb = meta.tile([128, T * 8], I16)
            nc.sync.dma_start(out=idx16_sb[:], in_=idx16[:])
            sel_sb = meta.tile([128, T * NSEQ], SELDT)
            nc.sync.dma_start(out=sel_sb[:], in_=sel[:])
            w_sb = meta.tile([128, PDIM], WDT)
            nc.sync.dma_start(out=w_sb[:], in_=wrep[:])
            b_sb = meta.tile([NSEQ, 1], F32)
            nc.sync.dma_start(out=b_sb[:], in_=brep[:])

            dot_ps = pp.tile([1, NSEQ], F32)
            first_chunk = True
            HALF = PDIM // 2
            if split:
                # DVE handles even tiles (row.W dot), PE handles odd tiles
                # (pooled accumulation); W applied to the pooled half once.
                pe_set = set(range(1, T, 2))
                dot8 = pp.tile([NSEQ, 1], F32, tag="d8")
                pool_a = pp.tile([NSEQ, HALF], F32, tag="pa")
                pool_b = pp.tile([NSEQ, HALF], F32, tag="pb")
                w16 = meta.tile([128, PDIM], GDT)
                nc.vector.tensor_copy(out=w16[:], in_=w_sb[:])
            else:
                pe_set = set()
                w16 = w_sb
            stt_set = set(range(T)) - pe_set
            pe_lo, pe_hi = (min(pe_set), max(pe_set)) if pe_set else (0, 0)
            st_lo, st_hi = (min(stt_set), max(stt_set)) if stt_set else (0, 0)
            t = 0
            for w in range(NW):
                wlo = w * WIN
                whi = min(wlo + WIN, VOCAB)
                left = Tw[w]
                while left > 0:
                    # small first chunk: first gathered data lands sooner,
                    # cutting pipeline ramp-in before the consumers start
                    c = min(4 if first_chunk else chunk, left)
                    first_chunk = False
                    g = gp.tile([128, c, PDIMP], GDT, tag="g")
                    nc.gpsimd.dma_gather(
                        out_ap=g[:],
                        in_ap=emb[wlo:whi],
                        idxs_ap=idx16_sb[:, t * 8 : (t + c) * 8],
                        num_idxs=c * 128,
                        num_idxs_reg=c * 128,
                        elem_size=PDIMP,
                    )
                    gflat = g[:].rearrange("p c e -> p (c e)")
                    for j in range(c):
                        tt = t + j
                        off = j * PDIMP
                        if tt in pe_set:
                            nc.tensor.matmul(
                                out=pool_a[:],
                                lhsT=sel_sb[:, tt * NSEQ : (tt + 1) * NSEQ],
                                rhs=gflat[:, off : off + HALF],
                                start=(tt == pe_lo),
                                stop=(tt == pe_hi),
                            )
                            nc.tensor.matmul(
                                out=pool_b[:],
                                lhsT=sel_sb[:, tt * NSEQ : (tt + 1) * NSEQ],
                                rhs=gflat[:, off + HALF : off + PDIM],
                                start=(tt == pe_lo),
                                stop=(tt == pe_hi),
                            )
                            continue
                        y = yp.tile([128, 1], GDT if split else F32)
                        gs = gflat[:, off : off + PDIM]
                        nc.vector.scalar_tensor_tensor(
                            out=gs,
                            in0=gs,
                            scalar=1.0,
                            in1=w16[:],
                            op0=mybir.AluOpType.mult,
                            op1=mybir.AluOpType.mult,
                            accum_out=y[:],
                        )
                        if split:
                            nc.tensor.matmul(
                                out=dot8[:],
                                lhsT=sel_sb[:, tt * NSEQ : (tt + 1) * NSEQ],
                                rhs=y[:],
                                start=(tt == st_lo),
                                stop=(tt == st_hi),
                            )
                        else:
                            nc.tensor.matmul(
                                out=dot_ps[:],
                                lhsT=y[:],
                                rhs=sel_sb[:, tt * NSEQ : (tt + 1) * NSEQ],
                                start=(tt == st_lo),
                                stop=(tt == st_hi),
                            )
                    t += c
                    left -= c

            if split:
                pooled_sb = meta.tile([NSEQ, PDIM], F32)
                nc.vector.tensor_copy(out=pooled_sb[:, :HALF], in_=pool_a[:])
                nc.vector.tensor_copy(out=pooled_sb[:, HALF:], in_=pool_b[:])
                scr = meta.tile([NSEQ, PDIM], F32)
                y8 = meta.tile([NSEQ, 1], F32)
                nc.vector.scalar_tensor_tensor(
                    out=scr[:],
                    in0=pooled_sb[:],
                    scalar=1.0,
                    in1=w_sb[:NSEQ, :],
                    op0=mybir.AluOpType.mult,
                    op1=mybir.AluOpType.mult,
                    accum_out=y8[:],
                )
                part = meta.tile([NSEQ, 1], F32)
                nc.vector.tensor_tensor(
                    out=part[:], in0=dot8[:], in1=y8[:], op=mybir.AluOpType.add
                )
                o_sb = meta.tile([NSEQ, 1], F32)
                nc.scalar.activation(
                    out=o_sb[:],
                    in_=part[:],
                    func=mybir.ActivationFunctionType.Sigmoid,
                    bias=b_sb[:],
                    scale=1.0 / float(L),
                )
                nc.sync.dma_start(out=outp[0, :, None], in_=o_sb[:])
            else:
                o_sb = meta.tile([1, NSEQ], F32)
                nc.scalar.activation(
                    out=o_sb[:],
                    in_=dot_ps[:],
                    func=mybir.ActivationFunctionType.Sigmoid,
                    bias=b_sb[:1, :],
                    scale=1.0 / float(L),
                )
                nc.sync.dma_start(out=outp[:], in_=o_sb[:])

    nc.compile()
    if legalize:
        _legalize_sem_waits(nc, __import__("concourse.mybir", fromlist=["x"]))
    return nc


def _kernel_seq(tokens, lengths, emb_table, W, b):
    from concourse.bass_utils import run_bass_kernel_spmd

    dtype = os.environ.get("BERT_DTYPE", "bf16")
    chunk = int(os.environ.get("BERT_CHUNK", "8"))
    gbufs = int(os.environ.get("BERT_GBUFS", "4"))
    ybufs = int(os.environ.get("BERT_YBUFS", "16"))
    trace = os.environ.get("BERT_TRACE", "0") == "1"

    split = os.environ.get("BERT_SEQSPLIT", "1") == "1"
    Tw, in_maps, assign = _marshal_seq(tokens, lengths, emb_table, W, b, dtype=dtype)
    nc = _build_seq(Tw, chunk, gbufs, ybufs, dtype=dtype, split=split)
    res = run_bass_kernel_spmd(nc, in_maps, core_ids=list(range(NCORES)), trace=trace)
    LAST["results"] = res
    LAST["T"] = sum(Tw)
    LAST["Vmax"] = VOCAB
    out = np.zeros(B, dtype=np.float32)
    for c in range(NCORES):
        vals = res.results[c]["out"].reshape(-1)
        for j in range(NSEQ):
            out[assign[c, j]] = vals[j]
    return out


_kernel_vocab = kernel


# ---------------------------------------------------------------------------
# fp8 all-PE variant (BERT_SHARD=pe8, default): sequence-ownership sharding as
# above, but the table is cast to fp8e4 (halves gather DMA traffic; final
# sigmoid output error ~1e-4 << 2e-2 budget) and ALL per-tile compute runs on
# the PE: pooled[seq,:] += sel_t^T @ g_t accumulated in two PSUM banks across
# every tile. This removes the DVE<->PE zigzag (STT -> dot8 -> pool-MM) that
# paced the old pipeline at ~10.3us per 8-tile chunk with no engine saturated.
# The W dot + sigmoid happen once on [8,1000] at the end.
def _marshal_pe8(tokens, lengths, emb_table, W, b):
    import ml_dtypes

    f8 = ml_dtypes.float8_e4m3
    tokens = np.asarray(tokens)
    lengths = np.asarray(lengths).astype(np.int64)

    order = np.argsort(-lengths, kind="stable")
    hists = np.zeros((B, NW), dtype=np.int64)
    for bidx in range(B):
        u = np.unique(tokens[bidx, : lengths[bidx]].astype(np.int64))
        hists[bidx] = np.bincount(u // WIN, minlength=NW)
    Wc = np.zeros((NCORES, NW), dtype=np.int64)
    counts = np.zeros(NCORES, dtype=np.int64)
    assign = np.full((NCORES, NSEQ), -1, dtype=np.int64)
    for bidx in order:
        cands = np.where(counts < NSEQ)[0]
        best, bobj = None, None
        for c in cands:
            trial = Wc.copy()
            trial[c] += hists[bidx]
            obj = trial.max(axis=0).sum()
            if bobj is None or obj < bobj:
                best, bobj = c, obj
        assign[best, counts[best]] = bidx
        counts[best] += 1
        Wc[best] += hists[bidx]

    def _obj(Wm):
        return (-(-Wm.max(axis=0) // 128)).sum() * 1000000 + Wm.max(axis=0).sum()

    for _ in range(40):
        improved = False
        cur = _obj(Wc)
        for c1 in range(NCORES):
            for j1 in range(NSEQ):
                for c2 in range(c1 + 1, NCORES):
                    for j2 in range(NSEQ):
                        b1, b2 = assign[c1, j1], assign[c2, j2]
                        trial = Wc.copy()
                        trial[c1] += hists[b2] - hists[b1]
                        trial[c2] += hists[b1] - hists[b2]
                        if _obj(trial) < cur:
                            assign[c1, j1], assign[c2, j2] = b2, b1
                            Wc = trial
                            cur = _obj(Wc)
                            improved = True
        if not improved:
            break

    per_core_rows = []
    for c in range(NCORES):
        toks = np.concatenate(
            [tokens[assign[c, j], : lengths[assign[c, j]]] for j in range(NSEQ)]
        ).astype(np.int64)
        locb = np.concatenate(
            [np.full(int(lengths[assign[c, j]]), j, dtype=np.int64) for j in range(NSEQ)]
        )
        uniq, inv = np.unique(toks, return_inverse=True)
        cnt8 = np.zeros((len(uniq), NSEQ), dtype=np.float32)
        np.add.at(cnt8, (inv, locb), 1.0)
        per_core_rows.append((uniq, cnt8))

    Tw = []
    bnds = []
    for w in range(NW):
        lo, hi = w * WIN, min((w + 1) * WIN, VOCAB)
        per_core_bnd = [
            (np.searchsorted(u, lo), np.searchsorted(u, hi)) for u, _ in per_core_rows
        ]
        bnds.append(per_core_bnd)
        Tw.append(max(-(-int(e - s) // 128) for s, e in per_core_bnd))
    T = sum(Tw)

    emb8 = np.zeros((VOCAB, PDIMP), dtype=f8)
    emb8[:, :PDIM] = np.ascontiguousarray(emb_table, dtype=np.float32).astype(f8)
    w8 = np.ascontiguousarray(
        np.broadcast_to(np.asarray(W, dtype=np.float32).reshape(1, PDIM), (NSEQ, PDIM))
    )
    brep = np.full((NSEQ, 1), np.float32(np.asarray(b).reshape(-1)[0]), dtype=np.float32)

    gmode = os.environ.get("BERT_GMODE", "dmag")
    if gmode == "ind":
        # int32 full-vocab row indices -> no 32768-row windows, no window
        # padding; T is just the cross-core max tile count.
        T = max(-(-len(u) // 128) for u, _ in per_core_rows)
        in_maps = []
        for c in range(NCORES):
            uniq, cnt8 = per_core_rows[c]
            n = len(uniq)
            rows = np.zeros(T * 128, dtype=np.int32)
            rows[:n] = uniq.astype(np.int32)
            selm = np.zeros((T * 128, NSEQ), dtype=np.float32)
            selm[:n] = cnt8
            in_maps.append(
                {
                    "emb": emb8,
                    "idx32": rows.reshape(T, 128).T.copy(),
                    "sel": selm.astype(f8)
                    .reshape(T, 128, NSEQ)
                    .transpose(1, 0, 2)
                    .reshape(128, T * NSEQ)
                    .copy(),
                    "wrep": w8,
                    "brep": brep,
                }
            )
        return [T], in_maps, assign

    in_maps = []
    for c in range(NCORES):
        uniq, cnt8 = per_core_rows[c]
        rows = np.zeros(T * 128, dtype=np.int16)
        selm = np.zeros((T * 128, NSEQ), dtype=np.float32)
        t0 = 0
        for w in range(NW):
            s0, e0 = bnds[w][c]
            n = int(e0 - s0)
            rows[t0 * 128 : t0 * 128 + n] = (uniq[s0:e0] - w * WIN).astype(np.int16)
            selm[t0 * 128 : t0 * 128 + n] = cnt8[s0:e0]
            t0 += Tw[w]
        wrapped = rows.reshape(T * 8, 16).T  # [16, T*8]
        in_maps.append(
            {
                "emb": emb8,
                "idx16": np.tile(wrapped, (8, 1)).copy(),
                "sel": selm.astype(f8)
                .reshape(T, 128, NSEQ)
                .transpose(1, 0, 2)
                .reshape(128, T * NSEQ)
                .copy(),
                "wrep": w8,
                "brep": brep,
            }
        )
    return Tw, in_maps, assign


def _build_pe8(Tw, chunk, gbufs, legalize=True, gmode="dmag"):
    from concourse import bass, bacc, mybir
    import concourse.tile as tile

    F32 = mybir.dt.float32
    F8 = mybir.dt.float8e4
    I16 = mybir.dt.int16
    I32 = mybir.dt.int32
    T = sum(Tw)

    nc = bacc.Bacc(None, num_devices=NCORES)
    emb = nc.declare_dram_parameter("emb", [VOCAB, PDIMP], F8, isOutput=False)
    if gmode == "ind":
        idx32 = nc.declare_dram_parameter("idx32", [128, T], I32, isOutput=False)
    else:
        idx16 = nc.declare_dram_parameter("idx16", [128, T * 8], I16, isOutput=False)
    sel = nc.declare_dram_parameter("sel", [128, T * NSEQ], F8, isOutput=False)
    wrep = nc.declare_dram_parameter("wrep", [NSEQ, PDIM], F32, isOutput=False)
    brep = nc.declare_dram_parameter("brep", [NSEQ, 1], F32, isOutput=False)
    outp = nc.declare_dram_parameter("out", [1, NSEQ], F32, isOutput=True)

    HALF = PDIM // 2
    with tile.TileContext(nc) as tc:
        with (
            tc.tile_pool(name="meta", bufs=1) as meta,
            tc.tile_pool(name="g", bufs=gbufs) as gp,
            tc.tile_pool(name="ps", bufs=1, space="PSUM") as pp,
        ):
            if gmode == "ind":
                idx32_sb = meta.tile([128, T], I32)
                nc.sync.dma_start(out=idx32_sb[:], in_=idx32[:])
            else:
                idx16_sb = meta.tile([128, T * 8], I16)
                nc.sync.dma_start(out=idx16_sb[:], in_=idx16[:])
            sel_sb = meta.tile([128, T * NSEQ], F8)
            nc.sync.dma_start(out=sel_sb[:], in_=sel[:])
            w_sb = meta.tile([NSEQ, PDIM], F32)
            nc.sync.dma_start(out=w_sb[:], in_=wrep[:])
            b_sb = meta.tile([NSEQ, 1], F32)
            nc.sync.dma_start(out=b_sb[:], in_=brep[:])

            pool_a = pp.tile([NSEQ, HALF], F32, tag="pa")
            pool_b = pp.tile([NSEQ, HALF], F32, tag="pb")

            def consume(gflat, off, tt):
                nc.tensor.matmul(
                    out=pool_a[:],
                    lhsT=sel_sb[:, tt * NSEQ : (tt + 1) * NSEQ],
                    rhs=gflat[:, off : off + HALF],
                    start=(tt == 0),
                    stop=(tt == T - 1),
                )
                nc.tensor.matmul(
                    out=pool_b[:],
                    lhsT=sel_sb[:, tt * NSEQ : (tt + 1) * NSEQ],
                    rhs=gflat[:, off + HALF : off + PDIM],
                    start=(tt == 0),
                    stop=(tt == T - 1),
                )

            if gmode == "ind":
                for t in range(T):
                    g = gp.tile([128, PDIMP], F8, tag="g")
                    nc.gpsimd.indirect_dma_start(
                        out=g[:],
                        out_offset=None,
                        in_=emb[:],
                        in_offset=bass.IndirectOffsetOnAxis(
                            ap=idx32_sb[:, t : t + 1], axis=0
                        ),
                    )
                    consume(g[:], 0, t)
            else:
                t = 0
                first_chunk = True
                for w in range(NW):
                    wlo = w * WIN
                    whi = min(wlo + WIN, VOCAB)
                    left = Tw[w]
                    while left > 0:
                        c = min(4 if first_chunk else chunk, left)
                        first_chunk = False
                        g = gp.tile([128, c, PDIMP], F8, tag="g")
                        nc.gpsimd.dma_gather(
                            out_ap=g[:],
                            in_ap=emb[wlo:whi],
                            idxs_ap=idx16_sb[:, t * 8 : (t + c) * 8],
                            num_idxs=c * 128,
                            num_idxs_reg=c * 128,
                            elem_size=PDIMP,
                        )
                        gflat = g[:].rearrange("p c e -> p (c e)")
                        for j in range(c):
                            consume(gflat, j * PDIMP, t + j)
                        t += c
                        left -= c

            pooled_sb = meta.tile([NSEQ, PDIM], F32)
            nc.vector.tensor_copy(out=pooled_sb[:, :HALF], in_=pool_a[:])
            nc.vector.tensor_copy(out=pooled_sb[:, HALF:], in_=pool_b[:])
            scr = meta.tile([NSEQ, PDIM], F32)
            y8 = meta.tile([NSEQ, 1], F32)
            nc.vector.scalar_tensor_tensor(
                out=scr[:],
                in0=pooled_sb[:],
                scalar=1.0,
                in1=w_sb[:],
                op0=mybir.AluOpType.mult,
                op1=mybir.AluOpType.mult,
                accum_out=y8[:],
            )
            o_sb = meta.tile([NSEQ, 1], F32)
            nc.scalar.activation(
                out=o_sb[:],
                in_=y8[:],
                func=mybir.ActivationFunctionType.Sigmoid,
                bias=b_sb[:],
                scale=1.0 / float(L),
            )
            nc.sync.dma_start(out=outp[0, :, None], in_=o_sb[:])

    nc.compile()
    if legalize:
        _legalize_sem_waits(nc, __import__("concourse.mybir", fromlist=["x"]))
    return nc


def _kernel_pe8(tokens, lengths, emb_table, W, b):
    from concourse.bass_utils import run_bass_kernel_spmd

    chunk = int(os.environ.get("BERT_CHUNK", "8"))
    gbufs = int(os.environ.get("BERT_GBUFS", "4"))
    gmode = os.environ.get("BERT_GMODE", "dmag")
    trace = os.environ.get("BERT_TRACE", "0") == "1"

    Tw, in_maps, assign = _marshal_pe8(tokens, lengths, emb_table, W, b)
    nc = _build_pe8(Tw, chunk, gbufs, gmode=gmode)
    res = run_bass_kernel_spmd(nc, in_maps, core_ids=list(range(NCORES)), trace=trace)
    LAST["results"] = res
    LAST["T"] = sum(Tw)
    LAST["Vmax"] = VOCAB
    out = np.zeros(B, dtype=np.float32)
    for c in range(NCORES):
        vals = res.results[c]["out"].reshape(-1)
        for j in range(NSEQ):
            out[assign[c, j]] = vals[j]
    return out


# ---------------------------------------------------------------------------
# Vocab-sharded fp8 all-PE variant (BERT_SHARD=vp8): global dedup across all
# 64 sequences, unique rows split into 8 equal contiguous vocab chunks (each
# span < 32768 so int16 indices need no windows). Each core gathers ~U/8 rows
# (~7.1k vs ~9.5k for the seq split -- the Pool engine's descriptor-gen ucode
# at ~8.5ns/row is the wall, so fewer rows is the lever), accumulates
# pooled[64,1000] on the PE, dots with W, and an AllGather (warmed up early)
# combines the per-core [64] partials.
def _marshal_vp8(tokens, lengths, emb_table, W, b):
    import ml_dtypes

    f8 = ml_dtypes.float8_e4m3
    tokens = np.asarray(tokens)
    lengths = np.asarray(lengths).astype(np.int64)

    mask = np.arange(L)[None, :] < lengths[:, None]
    flat_tok = tokens[mask].astype(np.int64)
    flat_b = np.broadcast_to(np.arange(B)[:, None], (B, L))[mask]
    uniq, inv = np.unique(flat_tok, return_inverse=True)
    U = len(uniq)
    cnt = np.zeros((U, B), dtype=np.float32)
    np.add.at(cnt, (inv, flat_b), 1.0)

    bounds = [U * c // NCORES for c in range(NCORES + 1)]
    T = max(-(-(bounds[c + 1] - bounds[c]) // 128) for c in range(NCORES))
    spans = []
    for c in range(NCORES):
        s, e = bounds[c], bounds[c + 1]
        lo = int(uniq[s]) if e > s else 0
        hi = int(uniq[e - 1]) + 1 if e > s else 1
        assert hi - lo < 32768, f"core {c} vocab span {hi-lo} exceeds int16"
        spans.append((s, e, lo, hi))
    Vmax = max(hi - lo for _, _, lo, hi in spans)

    emb8 = np.zeros((VOCAB, PDIMP), dtype=f8)
    emb8[:, :PDIM] = np.ascontiguousarray(emb_table, dtype=np.float32).astype(f8)
    w64 = np.ascontiguousarray(
        np.broadcast_to(np.asarray(W, dtype=np.float32).reshape(1, PDIM), (B, PDIM))
    )
    brep = np.full((1, 1), np.float32(np.asarray(b).reshape(-1)[0]), dtype=np.float32)

    in_maps = []
    for c in range(NCORES):
        s, e, lo, hi = spans[c]
        emb_c = np.zeros((Vmax, PDIMP), dtype=f8)
        emb_c[: hi - lo] = emb8[lo:hi]
        rows = np.zeros(T * 128, dtype=np.int16)
        rows[: e - s] = (uniq[s:e] - lo).astype(np.int16)
        selm = np.zeros((T * 128, B), dtype=np.float32)
        selm[: e - s] = cnt[s:e]
        wrapped = rows.reshape(T * 8, 16).T  # [16, T*8]
        in_maps.append(
            {
                "emb": emb_c,
                "idx16": np.tile(wrapped, (8, 1)).copy(),
                "sel": selm.astype(f8)
                .reshape(T, 128, B)
                .transpose(1, 0, 2)
                .reshape(128, T * B)
                .copy(),
                "wrep": w64,
                "brep": brep,
            }
        )
    return T, Vmax, in_maps


def _build_vp8(T, Vmax, chunk, gbufs, legalize=True, ccwarm=True):
    from concourse import bacc, mybir
    import concourse.tile as tile

    F32 = mybir.dt.float32
    F8 = mybir.dt.float8e4
    I16 = mybir.dt.int16

    nc = bacc.Bacc(None, num_devices=NCORES)
    emb = nc.declare_dram_parameter("emb", [Vmax, PDIMP], F8, isOutput=False)
    idx16 = nc.declare_dram_parameter("idx16", [128, T * 8], I16, isOutput=False)
    sel = nc.declare_dram_parameter("sel", [128, T * B], F8, isOutput=False)
    wrep = nc.declare_dram_parameter("wrep", [B, PDIM], F32, isOutput=False)
    brep = nc.declare_dram_parameter("brep", [1, 1], F32, isOutput=False)
    outp = nc.declare_dram_parameter("out", [1, B], F32, isOutput=True)

    HALF = PDIM // 2
    with tile.TileContext(nc) as tc:
        with (
            tc.tile_pool(name="meta", bufs=1) as meta,
            tc.tile_pool(name="g", bufs=gbufs) as gp,
            tc.tile_pool(name="ps", bufs=1, space="PSUM") as pp,
            tc.tile_pool(name="dram", bufs=1, space="DRAM") as dp,
        ):
            idx16_sb = meta.tile([128, T * 8], I16)
            nc.sync.dma_start(out=idx16_sb[:], in_=idx16[:])
            sel_sb = meta.tile([128, T * B], F8)
            nc.sync.dma_start(out=sel_sb[:], in_=sel[:])
            w_sb = meta.tile([B, PDIM], F32)
            nc.sync.dma_start(out=w_sb[:], in_=wrep[:])
            b_sb = meta.tile([1, 1], F32)
            nc.sync.dma_start(out=b_sb[:], in_=brep[:])

            if ccwarm:
                warm_sb = meta.tile([1, 4], F32)
                nc.vector.memset(warm_sb[:], 0.0)
                ccw_in = dp.tile([1, 4], F32)
                ccw_out = dp.tile([NCORES, 4], F32)
                nc.sync.dma_start(out=ccw_in[:], in_=warm_sb[:])
                nc.gpsimd.collective_compute(
                    "AllGather",
                    mybir.AluOpType.bypass,
                    replica_groups=[list(range(NCORES))],
                    ins=[ccw_in[:]],
                    outs=[ccw_out[:]],
                )

            pool_a = pp.tile([B, HALF], F32, tag="pa")
            pool_b = pp.tile([B, HALF], F32, tag="pb")

            t = 0
            first_chunk = True
            while t < T:
                c = min(4 if first_chunk else chunk, T - t)
                first_chunk = False
                g = gp.tile([128, c, PDIMP], F8, tag="g")
                nc.gpsimd.dma_gather(
                    out_ap=g[:],
                    in_ap=emb[:],
                    idxs_ap=idx16_sb[:, t * 8 : (t + c) * 8],
                    num_idxs=c * 128,
                    num_idxs_reg=c * 128,
                    elem_size=PDIMP,
                )
                gflat = g[:].rearrange("p c e -> p (c e)")
                for j in range(c):
                    tt = t + j
                    off = j * PDIMP
                    nc.tensor.matmul(
                        out=pool_a[:],
                        lhsT=sel_sb[:, tt * B : (tt + 1) * B],
                        rhs=gflat[:, off : off + HALF],
                        start=(tt == 0),
                        stop=(tt == T - 1),
                    )
                    nc.tensor.matmul(
                        out=pool_b[:],
                        lhsT=sel_sb[:, tt * B : (tt + 1) * B],
                        rhs=gflat[:, off + HALF : off + PDIM],
                        start=(tt == 0),
                        stop=(tt == T - 1),
                    )
                t += c

            pooled_sb = meta.tile([B, PDIM], F32)
            nc.vector.tensor_copy(out=pooled_sb[:, :HALF], in_=pool_a[:])
            nc.vector.tensor_copy(out=pooled_sb[:, HALF:], in_=pool_b[:])
            scr = meta.tile([B, PDIM], F32)
            y64 = meta.tile([B, 1], F32)
            nc.vector.scalar_tensor_tensor(
                out=scr[:],
                in0=pooled_sb[:],
                scalar=1.0,
                in1=w_sb[:],
                op0=mybir.AluOpType.mult,
                op1=mybir.AluOpType.mult,
                accum_out=y64[:],
            )
            cc_in = dp.tile([B, 1], F32)
            nc.sync.dma_start(out=cc_in[:], in_=y64[:])
            cc_out = dp.tile([NCORES * B, 1], F32)
            nc.gpsimd.collective_compute(
                "AllGather",
                mybir.AluOpType.bypass,
                replica_groups=[list(range(NCORES))],
                ins=[cc_in[:]],
                outs=[cc_out[:]],
            )
            allg_sb = meta.tile([NCORES, B], F32)
            nc.sync.dma_start(
                out=allg_sb[:],
                in_=cc_out[:].rearrange("a b -> (a b)").rearrange("(c n) -> c n", c=NCORES),
            )
            ones_sb = meta.tile([NCORES, 1], F32)
            nc.vector.memset(ones_sb[:], 1.0)
            sum_ps = pp.tile([1, B], F32, tag="sum")
            nc.tensor.matmul(
                out=sum_ps[:],
                lhsT=ones_sb[:],
                rhs=allg_sb[:],
                start=True,
                stop=True,
            )
            o_sb = meta.tile([1, B], F32)
            nc.scalar.activation(
                out=o_sb[:],
                in_=sum_ps[:],
                func=mybir.ActivationFunctionType.Sigmoid,
                bias=b_sb[:],
                scale=1.0 / float(L),
            )
            nc.sync.dma_start(out=outp[:], in_=o_sb[:])

    nc.compile()
    if legalize:
        _legalize_sem_waits(nc, __import__("concourse.mybir", fromlist=["x"]))
    return nc


def _kernel_vp8(tokens, lengths, emb_table, W, b):
    from concourse.bass_utils import run_bass_kernel_spmd

    chunk = int(os.environ.get("BERT_CHUNK", "8"))
    gbufs = int(os.environ.get("BERT_GBUFS", "4"))
    ccwarm = os.environ.get("BERT_CCWARM", "1") == "1"
    trace = os.environ.get("BERT_TRACE", "0") == "1"

    T, Vmax, in_maps = _marshal_vp8(tokens, lengths, emb_table, W, b)
    nc = _build_vp8(T, Vmax, chunk, gbufs, ccwarm=ccwarm)
    res = run_bass_kernel_spmd(nc, in_maps, core_ids=list(range(NCORES)), trace=trace)
    LAST["results"] = res
    LAST["T"] = T
    LAST["Vmax"] = Vmax
    return res.results[0]["out"].reshape(B).astype(np.float32)


def kernel(tokens, lengths, emb_table, W, b):
    shard = os.environ.get("BERT_SHARD", "vp8")
    if shard == "vp8":
        return _kernel_vp8(tokens, lengths, emb_table, W, b)
    if shard == "pe8":
        return _kernel_pe8(tokens, lengths, emb_table, W, b)
    if shard == "seq":
        return _kernel_seq(tokens, lengths, emb_table, W, b)
    return _kernel_vocab(tokens, lengths, emb_table, W, b)



# revision 9
# speedup vs baseline: 2.0438x; 2.0438x over previous
"""Trainium2 Bass kernel for nn_BerTII (masked-mean embedding bag -> 1-dim
linear -> sigmoid), distributed over 8 NeuronCores.

reference math:
  mask[b,l] = l < lengths[b]
  pooled[b,:] = sum_l mask[b,l] * emb[tokens[b,l],:] / L
  out[b] = sigmoid(pooled[b,:] @ W.T + bias)

The 1-output linear commutes with the masked mean:
  out[b] = sigmoid( (1/L) * sum_{l<len_b} (emb[tokens[b,l]] . W) + bias )
so the kernel never materializes the [B,L,P] gather. Host-side marshaling is
integer-only index work (the "all-to-all" of the sharding hint done at
input-staging time):
  - flatten all valid (b,l) tokens, dedupe globally (np.unique) and build a
    per-(unique-row, batch) multiplicity matrix;
  - split the unique rows into 8 equal-count contiguous chunks; core c
    receives ONLY the vocab slice spanning its chunk (rebased int16 indices),
    so the 800MB table is sharded across cores, not replicated; rows are
    padded 1000->1024 floats so each row is one 4KB 256B-aligned gather
    element;
  - each core bulk-gathers its ~U/8 rows with InstDMAGatherAnt, dots each row
    with W on the Vector engine (scalar_tensor_tensor accum), and accumulates
    per-batch partial dot products with tiny PE matmuls against the
    multiplicity matrix (y stationary [128,1], counts moving [128,64]);
  - an 8-core AllReduce(add) of the [64] partials, then sigmoid(x/L + b) on
    the Scalar engine. Every core emits the full [64] output; core 0's is
    returned.

DEFAULT (BERT_SHARD=seq): the sequence-ownership variant at the bottom of this
file instead — each core owns 8 length-balanced sequences end-to-end (table
replicated in bf16, int16 gathers windowed into 32768-row vocab slabs, no
collective), which removes cross-core straggler waits: ~129 us vs ~135-142 us
for the vocab-sharded path (BERT_SHARD=vocab).
"""
import os
import sys

sys.path.insert(0, "/opt/trn_rl_repo")

import numpy as np

VOCAB = 200000
PDIM = 1000
PDIMP = 1024  # row stride padded to 256B multiple for dma_gather
B = 64
L = 2048
NCORES = 8

LAST = {}  # debug: last BassKernelResults etc.


# ---------------------------------------------------------------------------
# walrus legalization: this toolchain allows at most ONE semaphore wait per
# instruction ("Too many sync wait commands"); split extras onto NoOps.
def _legalize_sem_waits(nc, mybir, max_waits=1):
    n = 0
    for f in nc.m.functions:
        for bb in f.blocks:
            new = []
            for inst in bb.instructions:
                si = inst.sync_info
                if si is not None and si.on_wait and len(si.on_wait) > max_waits:
                    waits = list(si.on_wait)
                    extra, keep = waits[:-max_waits], waits[-max_waits:]
                    k = 0
                    while extra:
                        chunk, extra = extra[:max_waits], extra[max_waits:]
                        new.append(
                            mybir.InstNoOp(
                                name=f"{inst.name}-ws{k}",
                                sync_info=mybir.SyncInfo(on_wait=chunk, on_update=[]),
                                bass_nofuse=True,
                                engine=inst.engine,
                            )
                        )
                        k += 1
                        n += 1
                    si.on_wait = keep
                new.append(inst)
            bb.instructions[:] = new
    return n


def _build(Vmax, T, chunk, gbufs, mode="dmag", legalize=True, ybufs=16, ramp_ind=0, cc="ag", dtype="f32", compute="pe"):
    from concourse import bass, bacc, mybir
    import concourse.tile as tile
    from concourse.tile import add_dep_helper

    F32 = mybir.dt.float32
    GDT = mybir.dt.bfloat16 if dtype == "bf16" else F32
    I16 = mybir.dt.int16
    I32 = mybir.dt.int32

    nc = bacc.Bacc(None, num_devices=NCORES)
    emb = nc.declare_dram_parameter("emb", [Vmax, PDIMP], GDT, isOutput=False)
    # idx16: gather index i of this core lives at [i % 16, i // 16], rows
    # replicated x8 down the partition dim (one copy per Q7 band).
    idx16 = nc.declare_dram_parameter("idx16", [128, T * 8], I16, isOutput=False)
    idx32 = nc.declare_dram_parameter("idx32", [128, T], I32, isOutput=False)
    SELDT = GDT if compute in ("pe", "split") else F32
    sel = nc.declare_dram_parameter("sel", [128, T * B], SELDT, isOutput=False)
    WDT = F32 if compute in ("pe", "split") else GDT
    wrep = nc.declare_dram_parameter("wrep", [128, PDIM], WDT, isOutput=False)
    brep = nc.declare_dram_parameter("brep", [1, 1], F32, isOutput=False)
    outp = nc.declare_dram_parameter("out", [1, B], F32, isOutput=True)

    with tile.TileContext(nc) as tc:
        with (
            tc.tile_pool(name="meta", bufs=1) as meta,
            tc.tile_pool(name="g", bufs=gbufs) as gp,
            tc.tile_pool(name="y", bufs=ybufs) as yp,
            tc.tile_pool(name="ps", bufs=1, space="PSUM") as pp,
            tc.tile_pool(name="dram", bufs=1, space="DRAM") as dp,
        ):
            idx16_sb = meta.tile([128, T * 8], I16)
            nc.sync.dma_start(out=idx16_sb[:], in_=idx16[:])
            idx32_sb = meta.tile([128, T], I32)
            nc.sync.dma_start(out=idx32_sb[:], in_=idx32[:])
            sel_sb = meta.tile([128, T * B], SELDT)
            nc.sync.dma_start(out=sel_sb[:], in_=sel[:])
            w_sb = meta.tile([128, PDIM], WDT)
            nc.sync.dma_start(out=w_sb[:], in_=wrep[:])
            b_sb = meta.tile([1, 1], F32)
            nc.sync.dma_start(out=b_sb[:], in_=brep[:])

            # warmup collective: absorb ncfw rendezvous/setup concurrently
            # with the gather pipeline so the real AllReduce at the end is
            # cheap.
            if os.environ.get("BERT_CCWARM", "1") == "1":
                warm_sb = meta.tile([1, 4], F32)
                nc.vector.memset(warm_sb[:], 0.0)
                ccw_in = dp.tile([1, 4], F32)
                ccw_out = dp.tile([NCORES if cc == "ag" else 1, 4], F32)
                nc.sync.dma_start(out=ccw_in[:], in_=warm_sb[:])
                nc.gpsimd.collective_compute(
                    "AllGather" if cc == "ag" else "AllReduce",
                    mybir.AluOpType.bypass if cc == "ag" else mybir.AluOpType.add,
                    replica_groups=[list(range(NCORES))],
                    ins=[ccw_in[:]],
                    outs=[ccw_out[:]],
                )

            dot_ps = pp.tile([1, B], F32)
            HALF = PDIM // 2
            pool_a = pp.tile([B, HALF], F32, tag="pa")
            pool_b = pp.tile([B, HALF], F32, tag="pb")
            if compute == "pe":
                pe_set = set(range(T))
            elif compute == "split":
                pe_set = set(range(1, T, 2))
            else:
                pe_set = set()
            stt_set = set(range(T)) - pe_set
            pe_lo, pe_hi = (min(pe_set), max(pe_set)) if pe_set else (0, 0)
            st_lo, st_hi = (min(stt_set), max(stt_set)) if stt_set else (0, 0)
            YDT = GDT if compute == "split" else F32
            if compute == "split":
                w16 = meta.tile([128, PDIM], GDT)
                nc.vector.tensor_copy(out=w16[:], in_=w_sb[:])
            else:
                w16 = w_sb

            def consume(gflat, off, t):
                """gflat: [128, >=off+PDIM] gathered rows tile; tile index t."""
                if t in pe_set:
                    # pooled[b,:] += sel_t[:,b]^T @ G ; W applied once at the end
                    nc.tensor.matmul(
                        out=pool_a[:],
                        lhsT=sel_sb[:, t * B : (t + 1) * B],
                        rhs=gflat[:, off : off + HALF],
                        start=(t == pe_lo),
                        stop=(t == pe_hi),
                    )
                    nc.tensor.matmul(
                        out=pool_b[:],
                        lhsT=sel_sb[:, t * B : (t + 1) * B],
                        rhs=gflat[:, off + HALF : off + PDIM],
                        start=(t == pe_lo),
                        stop=(t == pe_hi),
                    )
                    return
                y = yp.tile([128, 1], YDT)
                gs = gflat[:, off : off + PDIM]
                nc.vector.scalar_tensor_tensor(
                    out=gs,
                    in0=gs,
                    scalar=1.0,
                    in1=w16[:],
                    op0=mybir.AluOpType.mult,
                    op1=mybir.AluOpType.mult,
                    accum_out=y[:],
                )
                nc.tensor.matmul(
                    out=dot_ps[:],
                    lhsT=y[:],
                    rhs=sel_sb[:, t * B : (t + 1) * B],
                    start=(t == st_lo),
                    stop=(t == st_hi),
                )

            if mode == "dmag":
                # ramp-in: first tiles as single-row-set indirect gathers (low
                # latency), remainder as bulk dma_gather chunks (low overhead)
                nramp = min(ramp_ind, T)
                ramp_insts = []
                for t in range(nramp):
                    gi = gp.tile([128, PDIMP], GDT, tag="gi")
                    gi_inst = nc.gpsimd.indirect_dma_start(
                        out=gi[:],
                        out_offset=None,
                        in_=emb[:],
                        in_offset=bass.IndirectOffsetOnAxis(
                            ap=idx32_sb[:, t : t + 1], axis=0
                        ),
                    )
                    # keep the low-latency ramp singles in issue order
                    if ramp_insts:
                        add_dep_helper(gi_inst.ins, ramp_insts[-1].ins, reason="ramp order")
                    ramp_insts.append(gi_inst)
                    consume(gi[:], 0, t)
                sched = []
                rem = T - nramp
                while rem > 0:
                    c = min(chunk, rem)
                    sched.append(c)
                    rem -= c
                s = nramp
                first_dmag = True
                for c in sched:
                    g = gp.tile([128, c, PDIMP], GDT, tag="g")
                    dg_inst = nc.gpsimd.dma_gather(
                        out_ap=g[:],
                        in_ap=emb[:],
                        idxs_ap=idx16_sb[:, s * 8 : (s + c) * 8],
                        num_idxs=c * 128,
                        num_idxs_reg=c * 128,
                        elem_size=PDIMP,
                    )
                    if first_dmag and ramp_insts:
                        add_dep_helper(dg_inst.ins, ramp_insts[-1].ins, reason="ramp first")
                        first_dmag = False
                    gflat = g[:].rearrange("p c e -> p (c e)")
                    for j in range(c):
                        consume(gflat, j * PDIMP, s + j)
                    s += c
            else:  # indirect: one [128, PDIMP] row-gather per tile
                for t in range(T):
                    g = gp.tile([128, PDIMP], F32, tag="g")
                    nc.gpsimd.indirect_dma_start(
                        out=g[:],
                        out_offset=None,
                        in_=emb[:],
                        in_offset=bass.IndirectOffsetOnAxis(
                            ap=idx32_sb[:, t : t + 1], axis=0
                        ),
                    )
                    consume(g[:], 0, t)

            if compute in ("pe", "split"):
                pooled_sb = meta.tile([B, PDIM], F32)
                nc.vector.tensor_copy(out=pooled_sb[:, :HALF], in_=pool_a[:])
                nc.vector.tensor_copy(out=pooled_sb[:, HALF:], in_=pool_b[:])
                scr = meta.tile([B, PDIM], F32)
                y64 = meta.tile([B, 1], F32)
                nc.vector.scalar_tensor_tensor(
                    out=scr[:],
                    in0=pooled_sb[:],
                    scalar=1.0,
                    in1=w_sb[:B, :],
                    op0=mybir.AluOpType.mult,
                    op1=mybir.AluOpType.mult,
                    accum_out=y64[:],
                )
                if compute == "split":
                    # fold the stt-half partial [1,B] into partition-major form
                    part1_sb = meta.tile([1, B], F32)
                    nc.vector.tensor_copy(out=part1_sb[:], in_=dot_ps[:])
                    ident1 = meta.tile([1, 1], F32)
                    nc.vector.memset(ident1[:], 1.0)
                    dot_t = pp.tile([B, 1], F32, tag="dt")
                    nc.tensor.transpose(out=dot_t[:], in_=part1_sb[:], identity=ident1[:])
                    both = meta.tile([B, 1], F32)
                    nc.vector.tensor_tensor(
                        out=both[:], in0=y64[:], in1=dot_t[:], op=mybir.AluOpType.add
                    )
                    part_sb = both
                else:
                    part_sb = y64
                cc_in = dp.tile([B, 1], F32)
            else:
                part_sb = meta.tile([1, B], F32)
                nc.vector.tensor_copy(out=part_sb[:], in_=dot_ps[:])
                cc_in = dp.tile([1, B], F32)
            nc.sync.dma_start(out=cc_in[:], in_=part_sb[:])
            pmajor = compute in ("pe", "split")
            if cc == "ag":
                cc_out = dp.tile([NCORES * B, 1] if pmajor else [NCORES, B], F32)
                nc.gpsimd.collective_compute(
                    "AllGather",
                    mybir.AluOpType.bypass,
                    replica_groups=[list(range(NCORES))],
                    ins=[cc_in[:]],
                    outs=[cc_out[:]],
                )
                allg_sb = meta.tile([NCORES, B], F32)
                nc.sync.dma_start(out=allg_sb[:], in_=cc_out[:].rearrange("a b -> (a b)").rearrange("(c n) -> c n", c=NCORES) if pmajor else cc_out[:])
                ones_sb = meta.tile([NCORES, 1], F32)
                nc.vector.memset(ones_sb[:], 1.0)
                sum_ps = pp.tile([1, B], F32, tag="sum")
                nc.tensor.matmul(
                    out=sum_ps[:],
                    lhsT=ones_sb[:],
                    rhs=allg_sb[:],
                    start=True,
                    stop=True,
                )
                red_ap = sum_ps[:]
            else:
                cc_out = dp.tile([1, B], F32)
                nc.gpsimd.collective_compute(
                    "AllReduce",
                    mybir.AluOpType.add,
                    replica_groups=[list(range(NCORES))],
                    ins=[cc_in[:]],
                    outs=[cc_out[:]],
                )
                red_sb = meta.tile([1, B], F32)
                nc.sync.dma_start(out=red_sb[:], in_=cc_out[:])
                red_ap = red_sb[:]
            o_sb = meta.tile([1, B], F32)
            nc.scalar.activation(
                out=o_sb[:],
                in_=red_ap,
                func=mybir.ActivationFunctionType.Sigmoid,
                bias=b_sb[:],
                scale=1.0 / float(L),
            )
            nc.sync.dma_start(out=outp[:], in_=o_sb[:])

    nc.compile()
    if legalize:
        _legalize_sem_waits(nc, mybir)
    return nc


def _marshal(tokens, lengths, emb_table, W, b, dtype="f32"):
    if dtype == "bf16":
        import ml_dtypes

        sdt = ml_dtypes.bfloat16
    else:
        sdt = np.float32
    tokens = np.asarray(tokens)
    lengths = np.asarray(lengths).astype(np.int64)
    emb_table = np.ascontiguousarray(emb_table, dtype=np.float32)

    mask = np.arange(L)[None, :] < lengths[:, None]
    flat_tok = tokens[mask].astype(np.int64)
    flat_b = np.broadcast_to(np.arange(B)[:, None], (B, L))[mask]
    uniq, inv = np.unique(flat_tok, return_inverse=True)
    U = len(uniq)
    cnt = np.zeros((U, B), dtype=np.float32)
    np.add.at(cnt, (inv, flat_b), 1.0)

    bounds = [U * c // NCORES for c in range(NCORES + 1)]
    rows_max = max(bounds[c + 1] - bounds[c] for c in range(NCORES))
    T = -(-rows_max // 128)

    spans = []
    for c in range(NCORES):
        s, e = bounds[c], bounds[c + 1]
        lo = int(uniq[s]) if e > s else 0
        hi = int(uniq[e - 1]) + 1 if e > s else 1
        spans.append((s, e, lo, hi))
    Vmax = max(hi - lo for _, _, lo, hi in spans)

    wdt = np.float32 if os.environ.get("BERT_COMPUTE", "stt") in ("pe", "split") else sdt
    wrep = np.broadcast_to(
        np.asarray(W, dtype=np.float32).astype(wdt).reshape(1, PDIM), (128, PDIM)
    ).copy()
    brep = np.full((1, 1), np.float32(np.asarray(b).reshape(-1)[0]), dtype=np.float32)

    in_maps = []
    for c in range(NCORES):
        s, e, lo, hi = spans[c]
        span = hi - lo
        emb_c = np.zeros((Vmax, PDIMP), dtype=sdt)
        emb_c[:span, :PDIM] = emb_table[lo:hi].astype(sdt)
        rows = np.zeros(T * 128, dtype=np.int32)
        rows[: e - s] = (uniq[s:e] - lo).astype(np.int32)
        # int16 wrapped layout: index i -> [i % 16, i // 16], replicated x8
        wrapped = rows.astype(np.int16).reshape(T * 8, 16).T  # [16, T*8]
        idx16 = np.tile(wrapped, (8, 1)).copy()  # [128, T*8]
        seldt = sdt if os.environ.get("BERT_COMPUTE", "stt") in ("pe", "split") else np.float32
        selm = np.zeros((T * 128, B), dtype=seldt)
        selm[: e - s] = cnt[s:e].astype(seldt)
        in_maps.append(
            {
                "emb": emb_c,
                "idx16": idx16,
                "idx32": rows.reshape(T, 128).T.copy(),
                "sel": selm.reshape(T, 128, B).transpose(1, 0, 2).reshape(128, T * B).copy(),
                "wrep": wrep,
                "brep": brep,
            }
        )
    return T, Vmax, in_maps


def kernel(tokens, lengths, emb_table, W, b):
    from concourse.bass_utils import run_bass_kernel_spmd

    mode = os.environ.get("BERT_MODE", "dmag")
    chunk = int(os.environ.get("BERT_CHUNK", "8"))
    gbufs = int(os.environ.get("BERT_GBUFS", "4"))
    ybufs = int(os.environ.get("BERT_YBUFS", "16"))
    ramp_ind = int(os.environ.get("BERT_RAMPIND", "0"))
    cc = os.environ.get("BERT_CC", "ag")
    compute = os.environ.get("BERT_COMPUTE", "stt")
    trace = os.environ.get("BERT_TRACE", "0") == "1"

    dtype = os.environ.get("BERT_DTYPE", "bf16")
    T, Vmax, in_maps = _marshal(tokens, lengths, emb_table, W, b, dtype=dtype)
    nc = _build(Vmax, T, chunk, gbufs, mode=mode, ybufs=ybufs, ramp_ind=ramp_ind, cc=cc, dtype=dtype, compute=compute)
    res = run_bass_kernel_spmd(nc, in_maps, core_ids=list(range(NCORES)), trace=trace)
    LAST["results"] = res
    LAST["T"] = T
    LAST["Vmax"] = Vmax
    return res.results[0]["out"].reshape(B).astype(np.float32)

# ---------------------------------------------------------------------------
# Sequence-ownership variant: each core owns 8 sequences end-to-end (no
# collective, no cross-core skew sensitivity). Table replicated in bf16;
# gathers windowed into 32768-row vocab windows so rebased indices fit int16.
WIN = 32768
NW = -(-VOCAB // WIN)
NSEQ = B // NCORES


def _marshal_seq(tokens, lengths, emb_table, W, b, dtype="bf16"):
    import ml_dtypes

    sdt = ml_dtypes.bfloat16 if dtype == "bf16" else np.float32
    tokens = np.asarray(tokens)
    lengths = np.asarray(lengths).astype(np.int64)

    # per-sequence unique-token histograms over vocab windows; greedy
    # vector-balancing assignment minimizes sum_w max_c rows (the padded
    # tile count is driven by per-window maxima, not total length)
    order = np.argsort(-lengths, kind="stable")
    hists = np.zeros((B, NW), dtype=np.int64)
    for bidx in range(B):
        u = np.unique(tokens[bidx, : lengths[bidx]].astype(np.int64))
        hists[bidx] = np.bincount(u // WIN, minlength=NW)
    Wc = np.zeros((NCORES, NW), dtype=np.int64)
    counts = np.zeros(NCORES, dtype=np.int64)
    assign = np.full((NCORES, NSEQ), -1, dtype=np.int64)
    for bidx in order:
        cands = np.where(counts < NSEQ)[0]
        best, bobj = None, None
        for c in cands:
            trial = Wc.copy()
            trial[c] += hists[bidx]
            obj = trial.max(axis=0).sum()
            if bobj is None or obj < bobj:
                best, bobj = c, obj
        assign[best, counts[best]] = bidx
        counts[best] += 1
        Wc[best] += hists[bidx]

    def _obj(Wm):
        return (-(-Wm.max(axis=0) // 128)).sum() * 1000000 + Wm.max(axis=0).sum()

    # swap refinement: directly minimize padded tile count sum_w ceil(max/128)
    for _ in range(40):
        improved = False
        cur = _obj(Wc)
        for c1 in range(NCORES):
            for j1 in range(NSEQ):
                for c2 in range(c1 + 1, NCORES):
                    for j2 in range(NSEQ):
                        b1, b2 = assign[c1, j1], assign[c2, j2]
                        trial = Wc.copy()
                        trial[c1] += hists[b2] - hists[b1]
                        trial[c2] += hists[b1] - hists[b2]
                        if _obj(trial) < cur:
                            assign[c1, j1], assign[c2, j2] = b2, b1
                            Wc = trial
                            cur = _obj(Wc)
                            improved = True
        if not improved:
            break

    per_core_rows = []  # (uniq, cnt8) per core
    for c in range(NCORES):
        toks = np.concatenate(
            [tokens[assign[c, j], : lengths[assign[c, j]]] for j in range(NSEQ)]
        ).astype(np.int64)
        locb = np.concatenate(
            [np.full(int(lengths[assign[c, j]]), j, dtype=np.int64) for j in range(NSEQ)]
        )
        uniq, inv = np.unique(toks, return_inverse=True)
        cnt8 = np.zeros((len(uniq), NSEQ), dtype=np.float32)
        np.add.at(cnt8, (inv, locb), 1.0)
        per_core_rows.append((uniq, cnt8))

    # per-window tile counts, common across cores (SPMD: same program)
    Tw = []
    bnds = []
    for w in range(NW):
        lo, hi = w * WIN, min((w + 1) * WIN, VOCAB)
        per_core_bnd = [
            (np.searchsorted(u, lo), np.searchsorted(u, hi)) for u, _ in per_core_rows
        ]
        bnds.append(per_core_bnd)
        Tw.append(max(-(-int(e - s) // 128) for s, e in per_core_bnd))
    T = sum(Tw)

    emb16 = np.zeros((VOCAB, PDIMP), dtype=sdt)
    emb16[:, :PDIM] = np.ascontiguousarray(emb_table, dtype=np.float32).astype(sdt)
    wdt = np.float32 if os.environ.get("BERT_SEQSPLIT", "1") == "1" else sdt
    wrep = np.broadcast_to(
        np.asarray(W, dtype=np.float32).astype(wdt).reshape(1, PDIM), (128, PDIM)
    ).copy()
    brep = np.full((NSEQ, 1), np.float32(np.asarray(b).reshape(-1)[0]), dtype=np.float32)

    in_maps = []
    for c in range(NCORES):
        uniq, cnt8 = per_core_rows[c]
        rows = np.zeros(T * 128, dtype=np.int16)
        selm = np.zeros((T * 128, NSEQ), dtype=np.float32)
        t0 = 0
        for w in range(NW):
            s0, e0 = bnds[w][c]
            n = int(e0 - s0)
            rows[t0 * 128 : t0 * 128 + n] = (uniq[s0:e0] - w * WIN).astype(np.int16)
            selm[t0 * 128 : t0 * 128 + n] = cnt8[s0:e0]
            t0 += Tw[w]
        if os.environ.get("BERT_SEQSPLIT", "1") == "1":
            selm = selm.astype(sdt)
        wrapped = rows.reshape(T * 8, 16).T  # [16, T*8]
        in_maps.append(
            {
                "emb": emb16,
                "idx16": np.tile(wrapped, (8, 1)).copy(),
                "sel": selm.reshape(T, 128, NSEQ)
                .transpose(1, 0, 2)
                .reshape(128, T * NSEQ)
                .copy(),
                "wrep": wrep,
                "brep": brep,
            }
        )
    return Tw, in_maps, assign


def _build_seq(Tw, chunk, gbufs, ybufs, dtype="bf16", legalize=True, split=True):
    from concourse import bacc, mybir
    import concourse.tile as tile

    F32 = mybir.dt.float32
    GDT = mybir.dt.bfloat16 if dtype == "bf16" else F32
    I16 = mybir.dt.int16
    T = sum(Tw)

    scratch = int(os.environ.get("BERT_DMASCRATCH", "131072"))
    nc = bacc.Bacc(None, num_devices=NCORES, dynamic_dma_scratch_size=scratch)
    emb = nc.declare_dram_parameter("emb", [VOCAB, PDIMP], GDT, isOutput=False)
    idx16 = nc.declare_dram_parameter("idx16", [128, T * 8], I16, isOutput=False)
    SELDT = GDT if split else F32
    sel = nc.declare_dram_parameter("sel", [128, T * NSEQ], SELDT, isOutput=False)
    WDT = F32 if split else GDT
    wrep = nc.declare_dram_parameter("wrep", [128, PDIM], WDT, isOutput=False)
    brep = nc.declare_dram_parameter("brep", [NSEQ, 1], F32, isOutput=False)
    outp = nc.declare_dram_parameter("out", [1, NSEQ], F32, isOutput=True)

    with tile.TileContext(nc) as tc:
        with (
            tc.tile_pool(name="meta", bufs=1) as meta,
            tc.tile_pool(name="g", bufs=gbufs) as gp,
            tc.tile_pool(name="y", bufs=ybufs) as yp,
            tc.tile_pool(name="ps", bufs=1, space="PSUM") as pp,
        ):
            idx16_sb = meta.tile([128, T * 8], I16)
            nc.sync.dma_start(out=idx16_sb[:], in_=idx16[:])
            sel_sb = meta.tile([128, T * NSEQ], SELDT)
            nc.sync.dma_start(out=sel_sb[:], in_=sel[:])
            w_sb = meta.tile([128, PDIM], WDT)
            nc.sync.dma_start(out=w_sb[:], in_=wrep[:])
            b_sb = meta.tile([NSEQ, 1], F32)
            nc.sync.dma_start(out=b_sb[:], in_=brep[:])

            dot_ps = pp.tile([1, NSEQ], F32)
            first_chunk = True
            HALF = PDIM // 2
            if split:
                # DVE handles even tiles (row.W dot), PE handles odd tiles
                # (pooled accumulation); W applied to the pooled half once.
                pe_set = set(range(1, T, 2))
                dot8 = pp.tile([NSEQ, 1], F32, tag="d8")
                pool_a = pp.tile([NSEQ, HALF], F32, tag="pa")
                pool_b = pp.tile([NSEQ, HALF], F32, tag="pb")
                w16 = meta.tile([128, PDIM], GDT)
                nc.vector.tensor_copy(out=w16[:], in_=w_sb[:])
            else:
                pe_set = set()
                w16 = w_sb
            stt_set = set(range(T)) - pe_set
            pe_lo, pe_hi = (min(pe_set), max(pe_set)) if pe_set else (0, 0)
            st_lo, st_hi = (min(stt_set), max(stt_set)) if stt_set else (0, 0)
            t = 0
            for w in range(NW):
                wlo = w * WIN
                whi = min(wlo + WIN, VOCAB)
                left = Tw[w]
                while left > 0:
                    # small first chunk: first gathered data lands sooner,
                    # cutting pipeline ramp-in before the consumers start
                    c = min(4 if first_chunk else chunk, left)
                    first_chunk = False
                    g = gp.tile([128, c, PDIMP], GDT, tag="g")
                    nc.gpsimd.dma_gather(
                        out_ap=g[:],
                        in_ap=emb[wlo:whi],
                        idxs_ap=idx16_sb[:, t * 8 : (t + c) * 8],
                        num_idxs=c * 128,
                        num_idxs_reg=c * 128,
                        elem_size=PDIMP,
                    )
                    gflat = g[:].rearrange("p c e -> p (c e)")
                    for j in range(c):
                        tt = t + j
                        off = j * PDIMP
                        if tt in pe_set:
                            nc.tensor.matmul(
                                out=pool_a[:],
                                lhsT=sel_sb[:, tt * NSEQ : (tt + 1) * NSEQ],
                                rhs=gflat[:, off : off + HALF],
                                start=(tt == pe_lo),
                                stop=(tt == pe_hi),
                            )
                            nc.tensor.matmul(
                                out=pool_b[:],
                                lhsT=sel_sb[:, tt * NSEQ : (tt + 1) * NSEQ],
                                rhs=gflat[:, off + HALF : off + PDIM],
                                start=(tt == pe_lo),
                                stop=(tt == pe_hi),
                            )
                            continue
                        y = yp.tile([128, 1], GDT if split else F32)
                        gs = gflat[:, off : off + PDIM]
                        nc.vector.scalar_tensor_tensor(
                            out=gs,
                            in0=gs,
                            scalar=1.0,
                            in1=w16[:],
                            op0=mybir.AluOpType.mult,
                            op1=mybir.AluOpType.mult,
                            accum_out=y[:],
                        )
                        if split:
                            nc.tensor.matmul(
                                out=dot8[:],
                                lhsT=sel_sb[:, tt * NSEQ : (tt + 1) * NSEQ],
                                rhs=y[:],
                                start=(tt == st_lo),
                                stop=(tt == st_hi),
                            )
                        else:
                            nc.tensor.matmul(
                                out=dot_ps[:],
                                lhsT=y[:],
                                rhs=sel_sb[:, tt * NSEQ : (tt + 1) * NSEQ],
                                start=(tt == st_lo),
                                stop=(tt == st_hi),
                            )
                    t += c
                    left -= c

            if split:
                pooled_sb = meta.tile([NSEQ, PDIM], F32)
                nc.vector.tensor_copy(out=pooled_sb[:, :HALF], in_=pool_a[:])
                nc.vector.tensor_copy(out=pooled_sb[:, HALF:], in_=pool_b[:])
                scr = meta.tile([NSEQ, PDIM], F32)
                y8 = meta.tile([NSEQ, 1], F32)
                nc.vector.scalar_tensor_tensor(
                    out=scr[:],
                    in0=pooled_sb[:],
                    scalar=1.0,
                    in1=w_sb[:NSEQ, :],
                    op0=mybir.AluOpType.mult,
                    op1=mybir.AluOpType.mult,
                    accum_out=y8[:],
                )
                part = meta.tile([NSEQ, 1], F32)
                nc.vector.tensor_tensor(
                    out=part[:], in0=dot8[:], in1=y8[:], op=mybir.AluOpType.add
                )
                o_sb = meta.tile([NSEQ, 1], F32)
                nc.scalar.activation(
                    out=o_sb[:],
                    in_=part[:],
                    func=mybir.ActivationFunctionType.Sigmoid,
                    bias=b_sb[:],
                    scale=1.0 / float(L),
                )
                nc.sync.dma_start(out=outp[0, :, None], in_=o_sb[:])
            else:
                o_sb = meta.tile([1, NSEQ], F32)
                nc.scalar.activation(
                    out=o_sb[:],
                    in_=dot_ps[:],
                    func=mybir.ActivationFunctionType.Sigmoid,
                    bias=b_sb[:1, :],
                    scale=1.0 / float(L),
                )
                nc.sync.dma_start(out=outp[:], in_=o_sb[:])

    nc.compile()
    if legalize:
        _legalize_sem_waits(nc, __import__("concourse.mybir", fromlist=["x"]))
    return nc


def _kernel_seq(tokens, lengths, emb_table, W, b):
    from concourse.bass_utils import run_bass_kernel_spmd

    dtype = os.environ.get("BERT_DTYPE", "bf16")
    chunk = int(os.environ.get("BERT_CHUNK", "8"))
    gbufs = int(os.environ.get("BERT_GBUFS", "4"))
    ybufs = int(os.environ.get("BERT_YBUFS", "16"))
    trace = os.environ.get("BERT_TRACE", "0") == "1"

    split = os.environ.get("BERT_SEQSPLIT", "1") == "1"
    Tw, in_maps, assign = _marshal_seq(tokens, lengths, emb_table, W, b, dtype=dtype)
    nc = _build_seq(Tw, chunk, gbufs, ybufs, dtype=dtype, split=split)
    res = run_bass_kernel_spmd(nc, in_maps, core_ids=list(range(NCORES)), trace=trace)
    LAST["results"] = res
    LAST["T"] = sum(Tw)
    LAST["Vmax"] = VOCAB
    out = np.zeros(B, dtype=np.float32)
    for c in range(NCORES):
        vals = res.results[c]["out"].reshape(-1)
        for j in range(NSEQ):
            out[assign[c, j]] = vals[j]
    return out


_kernel_vocab = kernel


# ---------------------------------------------------------------------------
# fp8 all-PE variant (BERT_SHARD=pe8, default): sequence-ownership sharding as
# above, but the table is cast to fp8e4 (halves gather DMA traffic; final
# sigmoid output error ~1e-4 << 2e-2 budget) and ALL per-tile compute runs on
# the PE: pooled[seq,:] += sel_t^T @ g_t accumulated in two PSUM banks across
# every tile. This removes the DVE<->PE zigzag (STT -> dot8 -> pool-MM) that
# paced the old pipeline at ~10.3us per 8-tile chunk with no engine saturated.
# The W dot + sigmoid happen once on [8,1000] at the end.
def _marshal_pe8(tokens, lengths, emb_table, W, b):
    import ml_dtypes

    f8 = ml_dtypes.float8_e4m3
    tokens = np.asarray(tokens)
    lengths = np.asarray(lengths).astype(np.int64)

    order = np.argsort(-lengths, kind="stable")
    hists = np.zeros((B, NW), dtype=np.int64)
    for bidx in range(B):
        u = np.unique(tokens[bidx, : lengths[bidx]].astype(np.int64))
        hists[bidx] = np.bincount(u // WIN, minlength=NW)
    Wc = np.zeros((NCORES, NW), dtype=np.int64)
    counts = np.zeros(NCORES, dtype=np.int64)
    assign = np.full((NCORES, NSEQ), -1, dtype=np.int64)
    for bidx in order:
        cands = np.where(counts < NSEQ)[0]
        best, bobj = None, None
        for c in cands:
            trial = Wc.copy()
            trial[c] += hists[bidx]
            obj = trial.max(axis=0).sum()
            if bobj is None or obj < bobj:
                best, bobj = c, obj
        assign[best, counts[best]] = bidx
        counts[best] += 1
        Wc[best] += hists[bidx]

    def _obj(Wm):
        return (-(-Wm.max(axis=0) // 128)).sum() * 1000000 + Wm.max(axis=0).sum()

    for _ in range(40):
        improved = False
        cur = _obj(Wc)
        for c1 in range(NCORES):
            for j1 in range(NSEQ):
                for c2 in range(c1 + 1, NCORES):
                    for j2 in range(NSEQ):
                        b1, b2 = assign[c1, j1], assign[c2, j2]
                        trial = Wc.copy()
                        trial[c1] += hists[b2] - hists[b1]
                        trial[c2] += hists[b1] - hists[b2]
                        if _obj(trial) < cur:
                            assign[c1, j1], assign[c2, j2] = b2, b1
                            Wc = trial
                            cur = _obj(Wc)
                            improved = True
        if not improved:
            break

    per_core_rows = []
    for c in range(NCORES):
        toks = np.concatenate(
            [tokens[assign[c, j], : lengths[assign[c, j]]] for j in range(NSEQ)]
        ).astype(np.int64)
        locb = np.concatenate(
            [np.full(int(lengths[assign[c, j]]), j, dtype=np.int64) for j in range(NSEQ)]
        )
        uniq, inv = np.unique(toks, return_inverse=True)
        cnt8 = np.zeros((len(uniq), NSEQ), dtype=np.float32)
        np.add.at(cnt8, (inv, locb), 1.0)
        per_core_rows.append((uniq, cnt8))

    Tw = []
    bnds = []
    for w in range(NW):
        lo, hi = w * WIN, min((w + 1) * WIN, VOCAB)
        per_core_bnd = [
            (np.searchsorted(u, lo), np.searchsorted(u, hi)) for u, _ in per_core_rows
        ]
        bnds.append(per_core_bnd)
        Tw.append(max(-(-int(e - s) // 128) for s, e in per_core_bnd))
    T = sum(Tw)

    emb8 = np.zeros((VOCAB, PDIMP), dtype=f8)
    emb8[:, :PDIM] = np.ascontiguousarray(emb_table, dtype=np.float32).astype(f8)
    w8 = np.ascontiguousarray(
        np.broadcast_to(np.asarray(W, dtype=np.float32).reshape(1, PDIM), (NSEQ, PDIM))
    )
    brep = np.full((NSEQ, 1), np.float32(np.asarray(b).reshape(-1)[0]), dtype=np.float32)

    gmode = os.environ.get("BERT_GMODE", "dmag")
    if gmode == "ind":
        # int32 full-vocab row indices -> no 32768-row windows, no window
        # padding; T is just the cross-core max tile count.
        T = max(-(-len(u) // 128) for u, _ in per_core_rows)
        in_maps = []
        for c in range(NCORES):
            uniq, cnt8 = per_core_rows[c]
            n = len(uniq)
            rows = np.zeros(T * 128, dtype=np.int32)
            rows[:n] = uniq.astype(np.int32)
            selm = np.zeros((T * 128, NSEQ), dtype=np.float32)
            selm[:n] = cnt8
            in_maps.append(
                {
                    "emb": emb8,
                    "idx32": rows.reshape(T, 128).T.copy(),
                    "sel": selm.astype(f8)
                    .reshape(T, 128, NSEQ)
                    .transpose(1, 0, 2)
                    .reshape(128, T * NSEQ)
                    .copy(),
                    "wrep": w8,
                    "brep": brep,
                }
            )
        return [T], in_maps, assign

    in_maps = []
    for c in range(NCORES):
        uniq, cnt8 = per_core_rows[c]
        rows = np.zeros(T * 128, dtype=np.int16)
        selm = np.zeros((T * 128, NSEQ), dtype=np.float32)
        t0 = 0
        for w in range(NW):
            s0, e0 = bnds[w][c]
            n = int(e0 - s0)
            rows[t0 * 128 : t0 * 128 + n] = (uniq[s0:e0] - w * WIN).astype(np.int16)
            selm[t0 * 128 : t0 * 128 + n] = cnt8[s0:e0]
            t0 += Tw[w]
        wrapped = rows.reshape(T * 8, 16).T  # [16, T*8]
        in_maps.append(
            {
                "emb": emb8,
                "idx16": np.tile(wrapped, (8, 1)).copy(),
                "sel": selm.astype(f8)
                .reshape(T, 128, NSEQ)
                .transpose(1, 0, 2)
                .reshape(128, T * NSEQ)
                .copy(),
                "wrep": w8,
                "brep": brep,
            }
        )
    return Tw, in_maps, assign


def _build_pe8(Tw, chunk, gbufs, legalize=True, gmode="dmag", nq=1):
    from concourse import bass, bacc, mybir
    import concourse.tile as tile

    F32 = mybir.dt.float32
    F8 = mybir.dt.float8e4
    I16 = mybir.dt.int16
    I32 = mybir.dt.int32
    T = sum(Tw)

    nc = bacc.Bacc(None, num_devices=NCORES, num_swdge_queues=nq)
    emb = nc.declare_dram_parameter("emb", [VOCAB, PDIMP], F8, isOutput=False)
    if gmode == "ind":
        idx32 = nc.declare_dram_parameter("idx32", [128, T], I32, isOutput=False)
    else:
        idx16 = nc.declare_dram_parameter("idx16", [128, T * 8], I16, isOutput=False)
    sel = nc.declare_dram_parameter("sel", [128, T * NSEQ], F8, isOutput=False)
    wrep = nc.declare_dram_parameter("wrep", [NSEQ, PDIM], F32, isOutput=False)
    brep = nc.declare_dram_parameter("brep", [NSEQ, 1], F32, isOutput=False)
    outp = nc.declare_dram_parameter("out", [1, NSEQ], F32, isOutput=True)

    HALF = PDIM // 2
    with tile.TileContext(nc) as tc:
        with (
            tc.tile_pool(name="meta", bufs=1) as meta,
            tc.tile_pool(name="g", bufs=gbufs) as gp,
            tc.tile_pool(name="ps", bufs=1, space="PSUM") as pp,
        ):
            if gmode == "ind":
                idx32_sb = meta.tile([128, T], I32)
                nc.sync.dma_start(out=idx32_sb[:], in_=idx32[:])
            else:
                idx16_sb = meta.tile([128, T * 8], I16)
                nc.sync.dma_start(out=idx16_sb[:], in_=idx16[:])
            sel_sb = meta.tile([128, T * NSEQ], F8)
            nc.sync.dma_start(out=sel_sb[:], in_=sel[:])
            w_sb = meta.tile([NSEQ, PDIM], F32)
            nc.sync.dma_start(out=w_sb[:], in_=wrep[:])
            b_sb = meta.tile([NSEQ, 1], F32)
            nc.sync.dma_start(out=b_sb[:], in_=brep[:])

            pool_a = pp.tile([NSEQ, HALF], F32, tag="pa")
            pool_b = pp.tile([NSEQ, HALF], F32, tag="pb")

            def consume(gflat, off, tt):
                nc.tensor.matmul(
                    out=pool_a[:],
                    lhsT=sel_sb[:, tt * NSEQ : (tt + 1) * NSEQ],
                    rhs=gflat[:, off : off + HALF],
                    start=(tt == 0),
                    stop=(tt == T - 1),
                )
                nc.tensor.matmul(
                    out=pool_b[:],
                    lhsT=sel_sb[:, tt * NSEQ : (tt + 1) * NSEQ],
                    rhs=gflat[:, off + HALF : off + PDIM],
                    start=(tt == 0),
                    stop=(tt == T - 1),
                )

            if gmode == "ind":
                for t in range(T):
                    g = gp.tile([128, PDIMP], F8, tag="g")
                    nc.gpsimd.indirect_dma_start(
                        out=g[:],
                        out_offset=None,
                        in_=emb[:],
                        in_offset=bass.IndirectOffsetOnAxis(
                            ap=idx32_sb[:, t : t + 1], axis=0
                        ),
                    )
                    consume(g[:], 0, t)
            else:
                t = 0
                gi = 0
                first_chunk = True
                for w in range(NW):
                    wlo = w * WIN
                    whi = min(wlo + WIN, VOCAB)
                    left = Tw[w]
                    while left > 0:
                        c = min(4 if first_chunk else chunk, left)
                        first_chunk = False
                        g = gp.tile([128, c, PDIMP], F8, tag="g")
                        nc.gpsimd.dma_gather(
                            out_ap=g[:],
                            in_ap=emb[wlo:whi],
                            idxs_ap=idx16_sb[:, t * 8 : (t + c) * 8],
                            num_idxs=c * 128,
                            num_idxs_reg=c * 128,
                            elem_size=PDIMP,
                            queue_num=gi % nq,
                        )
                        gi += 1
                        gflat = g[:].rearrange("p c e -> p (c e)")
                        for j in range(c):
                            consume(gflat, j * PDIMP, t + j)
                        t += c
                        left -= c

            pooled_sb = meta.tile([NSEQ, PDIM], F32)
            nc.vector.tensor_copy(out=pooled_sb[:, :HALF], in_=pool_a[:])
            nc.vector.tensor_copy(out=pooled_sb[:, HALF:], in_=pool_b[:])
            scr = meta.tile([NSEQ, PDIM], F32)
            y8 = meta.tile([NSEQ, 1], F32)
            nc.vector.scalar_tensor_tensor(
                out=scr[:],
                in0=pooled_sb[:],
                scalar=1.0,
                in1=w_sb[:],
                op0=mybir.AluOpType.mult,
                op1=mybir.AluOpType.mult,
                accum_out=y8[:],
            )
            o_sb = meta.tile([NSEQ, 1], F32)
            nc.scalar.activation(
                out=o_sb[:],
                in_=y8[:],
                func=mybir.ActivationFunctionType.Sigmoid,
                bias=b_sb[:],
                scale=1.0 / float(L),
            )
            nc.sync.dma_start(out=outp[0, :, None], in_=o_sb[:])

    nc.compile()
    if legalize:
        _legalize_sem_waits(nc, __import__("concourse.mybir", fromlist=["x"]))
    return nc


def _kernel_pe8(tokens, lengths, emb_table, W, b):
    from concourse.bass_utils import run_bass_kernel_spmd

    chunk = int(os.environ.get("BERT_CHUNK", "8"))
    gbufs = int(os.environ.get("BERT_GBUFS", "4"))
    gmode = os.environ.get("BERT_GMODE", "dmag")
    nq = int(os.environ.get("BERT_NQ", "4"))
    trace = os.environ.get("BERT_TRACE", "0") == "1"

    Tw, in_maps, assign = _marshal_pe8(tokens, lengths, emb_table, W, b)
    nc = _build_pe8(Tw, chunk, gbufs, gmode=gmode, nq=nq)
    res = run_bass_kernel_spmd(nc, in_maps, core_ids=list(range(NCORES)), trace=trace)
    LAST["results"] = res
    LAST["T"] = sum(Tw)
    LAST["Vmax"] = VOCAB
    out = np.zeros(B, dtype=np.float32)
    for c in range(NCORES):
        vals = res.results[c]["out"].reshape(-1)
        for j in range(NSEQ):
            out[assign[c, j]] = vals[j]
    return out


# ---------------------------------------------------------------------------
# Vocab-sharded fp8 all-PE variant (BERT_SHARD=vp8): global dedup across all
# 64 sequences, unique rows split into 8 equal contiguous vocab chunks (each
# span < 32768 so int16 indices need no windows). Each core gathers ~U/8 rows
# (~7.1k vs ~9.5k for the seq split -- the Pool engine's descriptor-gen ucode
# at ~8.5ns/row is the wall, so fewer rows is the lever), accumulates
# pooled[64,1000] on the PE, dots with W, and an AllGather (warmed up early)
# combines the per-core [64] partials.
def _marshal_vp8(tokens, lengths, emb_table, W, b):
    import ml_dtypes

    f8 = ml_dtypes.float8_e4m3
    tokens = np.asarray(tokens)
    lengths = np.asarray(lengths).astype(np.int64)

    mask = np.arange(L)[None, :] < lengths[:, None]
    flat_tok = tokens[mask].astype(np.int64)
    flat_b = np.broadcast_to(np.arange(B)[:, None], (B, L))[mask]
    uniq, inv = np.unique(flat_tok, return_inverse=True)
    U = len(uniq)
    cnt = np.zeros((U, B), dtype=np.float32)
    np.add.at(cnt, (inv, flat_b), 1.0)

    bounds = [U * c // NCORES for c in range(NCORES + 1)]
    T = max(-(-(bounds[c + 1] - bounds[c]) // 128) for c in range(NCORES))
    spans = []
    for c in range(NCORES):
        s, e = bounds[c], bounds[c + 1]
        lo = int(uniq[s]) if e > s else 0
        hi = int(uniq[e - 1]) + 1 if e > s else 1
        assert hi - lo < 32768, f"core {c} vocab span {hi-lo} exceeds int16"
        spans.append((s, e, lo, hi))
    Vmax = max(hi - lo for _, _, lo, hi in spans)

    emb8 = np.zeros((VOCAB, PDIMP), dtype=f8)
    emb8[:, :PDIM] = np.ascontiguousarray(emb_table, dtype=np.float32).astype(f8)
    w64 = np.ascontiguousarray(
        np.broadcast_to(np.asarray(W, dtype=np.float32).reshape(1, PDIM), (B, PDIM))
    )
    brep = np.full((1, 1), np.float32(np.asarray(b).reshape(-1)[0]), dtype=np.float32)

    in_maps = []
    for c in range(NCORES):
        s, e, lo, hi = spans[c]
        emb_c = np.zeros((Vmax, PDIMP), dtype=f8)
        emb_c[: hi - lo] = emb8[lo:hi]
        rows = np.zeros(T * 128, dtype=np.int16)
        rows[: e - s] = (uniq[s:e] - lo).astype(np.int16)
        selm = np.zeros((T * 128, B), dtype=np.float32)
        selm[: e - s] = cnt[s:e]
        wrapped = rows.reshape(T * 8, 16).T  # [16, T*8]
        in_maps.append(
            {
                "emb": emb_c,
                "idx16": np.tile(wrapped, (8, 1)).copy(),
                "sel": selm.astype(f8)
                .reshape(T, 128, B)
                .transpose(1, 0, 2)
                .reshape(128, T * B)
                .copy(),
                "wrep": w64,
                "brep": brep,
            }
        )
    return T, Vmax, in_maps


def _build_vp8(T, Vmax, chunk, gbufs, legalize=True, ccwarm=True):
    from concourse import bacc, mybir
    import concourse.tile as tile

    F32 = mybir.dt.float32
    F8 = mybir.dt.float8e4
    I16 = mybir.dt.int16

    nc = bacc.Bacc(None, num_devices=NCORES)
    emb = nc.declare_dram_parameter("emb", [Vmax, PDIMP], F8, isOutput=False)
    idx16 = nc.declare_dram_parameter("idx16", [128, T * 8], I16, isOutput=False)
    sel = nc.declare_dram_parameter("sel", [128, T * B], F8, isOutput=False)
    wrep = nc.declare_dram_parameter("wrep", [B, PDIM], F32, isOutput=False)
    brep = nc.declare_dram_parameter("brep", [1, 1], F32, isOutput=False)
    outp = nc.declare_dram_parameter("out", [1, B], F32, isOutput=True)

    HALF = PDIM // 2
    with tile.TileContext(nc) as tc:
        with (
            tc.tile_pool(name="meta", bufs=1) as meta,
            tc.tile_pool(name="g", bufs=gbufs) as gp,
            tc.tile_pool(name="ps", bufs=1, space="PSUM") as pp,
            tc.tile_pool(name="dram", bufs=1, space="DRAM") as dp,
        ):
            idx16_sb = meta.tile([128, T * 8], I16)
            nc.sync.dma_start(out=idx16_sb[:], in_=idx16[:])
            sel_sb = meta.tile([128, T * B], F8)
            nc.sync.dma_start(out=sel_sb[:], in_=sel[:])
            w_sb = meta.tile([B, PDIM], F32)
            nc.sync.dma_start(out=w_sb[:], in_=wrep[:])
            b_sb = meta.tile([1, 1], F32)
            nc.sync.dma_start(out=b_sb[:], in_=brep[:])

            if ccwarm:
                warm_sb = meta.tile([1, 4], F32)
                nc.vector.memset(warm_sb[:], 0.0)
                ccw_in = dp.tile([1, 4], F32)
                ccw_out = dp.tile([NCORES, 4], F32)
                nc.sync.dma_start(out=ccw_in[:], in_=warm_sb[:])
                nc.gpsimd.collective_compute(
                    "AllGather",
                    mybir.AluOpType.bypass,
                    replica_groups=[list(range(NCORES))],
                    ins=[ccw_in[:]],
                    outs=[ccw_out[:]],
                )

            pool_a = pp.tile([B, HALF], F32, tag="pa")
            pool_b = pp.tile([B, HALF], F32, tag="pb")

            t = 0
            first_chunk = True
            while t < T:
                c = min(4 if first_chunk else chunk, T - t)
                first_chunk = False
                g = gp.tile([128, c, PDIMP], F8, tag="g")
                nc.gpsimd.dma_gather(
                    out_ap=g[:],
                    in_ap=emb[:],
                    idxs_ap=idx16_sb[:, t * 8 : (t + c) * 8],
                    num_idxs=c * 128,
                    num_idxs_reg=c * 128,
                    elem_size=PDIMP,
                )
                gflat = g[:].rearrange("p c e -> p (c e)")
                for j in range(c):
                    tt = t + j
                    off = j * PDIMP
                    nc.tensor.matmul(
                        out=pool_a[:],
                        lhsT=sel_sb[:, tt * B : (tt + 1) * B],
                        rhs=gflat[:, off : off + HALF],
                        start=(tt == 0),
                        stop=(tt == T - 1),
                    )
                    nc.tensor.matmul(
                        out=pool_b[:],
                        lhsT=sel_sb[:, tt * B : (tt + 1) * B],
                        rhs=gflat[:, off + HALF : off + PDIM],
                        start=(tt == 0),
                        stop=(tt == T - 1),
                    )
                t += c

            pooled_sb = meta.tile([B, PDIM], F32)
            nc.vector.tensor_copy(out=pooled_sb[:, :HALF], in_=pool_a[:])
            nc.vector.tensor_copy(out=pooled_sb[:, HALF:], in_=pool_b[:])
            scr = meta.tile([B, PDIM], F32)
            y64 = meta.tile([B, 1], F32)
            nc.vector.scalar_tensor_tensor(
                out=scr[:],
                in0=pooled_sb[:],
                scalar=1.0,
                in1=w_sb[:],
                op0=mybir.AluOpType.mult,
                op1=mybir.AluOpType.mult,
                accum_out=y64[:],
            )
            cc_in = dp.tile([B, 1], F32)
            nc.sync.dma_start(out=cc_in[:], in_=y64[:])
            cc_out = dp.tile([NCORES * B, 1], F32)
            nc.gpsimd.collective_compute(
                "AllGather",
                mybir.AluOpType.bypass,
                replica_groups=[list(range(NCORES))],
                ins=[cc_in[:]],
                outs=[cc_out[:]],
            )
            allg_sb = meta.tile([NCORES, B], F32)
            nc.sync.dma_start(
                out=allg_sb[:],
                in_=cc_out[:].rearrange("a b -> (a b)").rearrange("(c n) -> c n", c=NCORES),
            )
            ones_sb = meta.tile([NCORES, 1], F32)
            nc.vector.memset(ones_sb[:], 1.0)
            sum_ps = pp.tile([1, B], F32, tag="sum")
            nc.tensor.matmul(
                out=sum_ps[:],
                lhsT=ones_sb[:],
                rhs=allg_sb[:],
                start=True,
                stop=True,
            )
            o_sb = meta.tile([1, B], F32)
            nc.scalar.activation(
                out=o_sb[:],
                in_=sum_ps[:],
                func=mybir.ActivationFunctionType.Sigmoid,
                bias=b_sb[:],
                scale=1.0 / float(L),
            )
            nc.sync.dma_start(out=outp[:], in_=o_sb[:])

    nc.compile()
    if legalize:
        _legalize_sem_waits(nc, __import__("concourse.mybir", fromlist=["x"]))
    return nc


def _kernel_vp8(tokens, lengths, emb_table, W, b):
    from concourse.bass_utils import run_bass_kernel_spmd

    chunk = int(os.environ.get("BERT_CHUNK", "8"))
    gbufs = int(os.environ.get("BERT_GBUFS", "4"))
    ccwarm = os.environ.get("BERT_CCWARM", "1") == "1"
    trace = os.environ.get("BERT_TRACE", "0") == "1"

    T, Vmax, in_maps = _marshal_vp8(tokens, lengths, emb_table, W, b)
    nc = _build_vp8(T, Vmax, chunk, gbufs, ccwarm=ccwarm)
    res = run_bass_kernel_spmd(nc, in_maps, core_ids=list(range(NCORES)), trace=trace)
    LAST["results"] = res
    LAST["T"] = T
    LAST["Vmax"] = Vmax
    return res.results[0]["out"].reshape(B).astype(np.float32)


def kernel(tokens, lengths, emb_table, W, b):
    shard = os.environ.get("BERT_SHARD", "vp8")
    if shard == "vp8":
        return _kernel_vp8(tokens, lengths, emb_table, W, b)
    if shard == "pe8":
        return _kernel_pe8(tokens, lengths, emb_table, W, b)
    if shard == "seq":
        return _kernel_seq(tokens, lengths, emb_table, W, b)
    return _kernel_vocab(tokens, lengths, emb_table, W, b)



# revision 17
# speedup vs baseline: 2.1860x; 1.0696x over previous
"""Trainium2 Bass kernel for nn_BerTII (masked-mean embedding bag -> 1-dim
linear -> sigmoid), distributed over 8 NeuronCores.

reference math:
  mask[b,l] = l < lengths[b]
  pooled[b,:] = sum_l mask[b,l] * emb[tokens[b,l],:] / L
  out[b] = sigmoid(pooled[b,:] @ W.T + bias)

The 1-output linear commutes with the masked mean:
  out[b] = sigmoid( (1/L) * sum_{l<len_b} (emb[tokens[b,l]] . W) + bias )
so the kernel never materializes the [B,L,P] gather. Host-side marshaling is
integer-only index work (the "all-to-all" of the sharding hint done at
input-staging time):
  - flatten all valid (b,l) tokens, dedupe globally (np.unique) and build a
    per-(unique-row, batch) multiplicity matrix;
  - split the unique rows into 8 equal-count contiguous chunks; core c
    receives ONLY the vocab slice spanning its chunk (rebased int16 indices),
    so the 800MB table is sharded across cores, not replicated; rows are
    padded 1000->1024 floats so each row is one 4KB 256B-aligned gather
    element;
  - each core bulk-gathers its ~U/8 rows with InstDMAGatherAnt, dots each row
    with W on the Vector engine (scalar_tensor_tensor accum), and accumulates
    per-batch partial dot products with tiny PE matmuls against the
    multiplicity matrix (y stationary [128,1], counts moving [128,64]);
  - an 8-core AllReduce(add) of the [64] partials, then sigmoid(x/L + b) on
    the Scalar engine. Every core emits the full [64] output; core 0's is
    returned.

DEFAULT (BERT_SHARD=seq): the sequence-ownership variant at the bottom of this
file instead — each core owns 8 length-balanced sequences end-to-end (table
replicated in bf16, int16 gathers windowed into 32768-row vocab slabs, no
collective), which removes cross-core straggler waits: ~129 us vs ~135-142 us
for the vocab-sharded path (BERT_SHARD=vocab).
"""
import os
import sys

sys.path.insert(0, "/opt/trn_rl_repo")

import numpy as np

VOCAB = 200000
PDIM = 1000
PDIMP = 1024  # row stride padded to 256B multiple for dma_gather
B = 64
L = 2048
NCORES = 8

LAST = {}  # debug: last BassKernelResults etc.


# ---------------------------------------------------------------------------
# walrus legalization: this toolchain allows at most ONE semaphore wait per
# instruction ("Too many sync wait commands"); split extras onto NoOps.
def _legalize_sem_waits(nc, mybir, max_waits=1):
    n = 0
    for f in nc.m.functions:
        for bb in f.blocks:
            new = []
            for inst in bb.instructions:
                si = inst.sync_info
                if si is not None and si.on_wait and len(si.on_wait) > max_waits:
                    waits = list(si.on_wait)
                    extra, keep = waits[:-max_waits], waits[-max_waits:]
                    k = 0
                    while extra:
                        chunk, extra = extra[:max_waits], extra[max_waits:]
                        new.append(
                            mybir.InstNoOp(
                                name=f"{inst.name}-ws{k}",
                                sync_info=mybir.SyncInfo(on_wait=chunk, on_update=[]),
                                bass_nofuse=True,
                                engine=inst.engine,
                            )
                        )
                        k += 1
                        n += 1
                    si.on_wait = keep
                new.append(inst)
            bb.instructions[:] = new
    return n


def _build(Vmax, T, chunk, gbufs, mode="dmag", legalize=True, ybufs=16, ramp_ind=0, cc="ag", dtype="f32", compute="pe"):
    from concourse import bass, bacc, mybir
    import concourse.tile as tile
    from concourse.tile import add_dep_helper

    F32 = mybir.dt.float32
    GDT = mybir.dt.bfloat16 if dtype == "bf16" else F32
    I16 = mybir.dt.int16
    I32 = mybir.dt.int32

    nc = bacc.Bacc(None, num_devices=NCORES)
    emb = nc.declare_dram_parameter("emb", [Vmax, PDIMP], GDT, isOutput=False)
    # idx16: gather index i of this core lives at [i % 16, i // 16], rows
    # replicated x8 down the partition dim (one copy per Q7 band).
    idx16 = nc.declare_dram_parameter("idx16", [128, T * 8], I16, isOutput=False)
    idx32 = nc.declare_dram_parameter("idx32", [128, T], I32, isOutput=False)
    SELDT = GDT if compute in ("pe", "split") else F32
    sel = nc.declare_dram_parameter("sel", [128, T * B], SELDT, isOutput=False)
    WDT = F32 if compute in ("pe", "split") else GDT
    wrep = nc.declare_dram_parameter("wrep", [128, PDIM], WDT, isOutput=False)
    brep = nc.declare_dram_parameter("brep", [1, 1], F32, isOutput=False)
    outp = nc.declare_dram_parameter("out", [1, B], F32, isOutput=True)

    with tile.TileContext(nc) as tc:
        with (
            tc.tile_pool(name="meta", bufs=1) as meta,
            tc.tile_pool(name="g", bufs=gbufs) as gp,
            tc.tile_pool(name="y", bufs=ybufs) as yp,
            tc.tile_pool(name="ps", bufs=1, space="PSUM") as pp,
            tc.tile_pool(name="dram", bufs=1, space="DRAM") as dp,
        ):
            idx16_sb = meta.tile([128, T * 8], I16)
            nc.sync.dma_start(out=idx16_sb[:], in_=idx16[:])
            idx32_sb = meta.tile([128, T], I32)
            nc.sync.dma_start(out=idx32_sb[:], in_=idx32[:])
            sel_sb = meta.tile([128, T * B], SELDT)
            nc.sync.dma_start(out=sel_sb[:], in_=sel[:])
            w_sb = meta.tile([128, PDIM], WDT)
            nc.sync.dma_start(out=w_sb[:], in_=wrep[:])
            b_sb = meta.tile([1, 1], F32)
            nc.sync.dma_start(out=b_sb[:], in_=brep[:])

            # warmup collective: absorb ncfw rendezvous/setup concurrently
            # with the gather pipeline so the real AllReduce at the end is
            # cheap.
            if os.environ.get("BERT_CCWARM", "1") == "1":
                warm_sb = meta.tile([1, 4], F32)
                nc.vector.memset(warm_sb[:], 0.0)
                ccw_in = dp.tile([1, 4], F32)
                ccw_out = dp.tile([NCORES if cc == "ag" else 1, 4], F32)
                nc.sync.dma_start(out=ccw_in[:], in_=warm_sb[:])
                nc.gpsimd.collective_compute(
                    "AllGather" if cc == "ag" else "AllReduce",
                    mybir.AluOpType.bypass if cc == "ag" else mybir.AluOpType.add,
                    replica_groups=[list(range(NCORES))],
                    ins=[ccw_in[:]],
                    outs=[ccw_out[:]],
                )

            dot_ps = pp.tile([1, B], F32)
            HALF = PDIM // 2
            pool_a = pp.tile([B, HALF], F32, tag="pa")
            pool_b = pp.tile([B, HALF], F32, tag="pb")
            if compute == "pe":
                pe_set = set(range(T))
            elif compute == "split":
                pe_set = set(range(1, T, 2))
            else:
                pe_set = set()
            stt_set = set(range(T)) - pe_set
            pe_lo, pe_hi = (min(pe_set), max(pe_set)) if pe_set else (0, 0)
            st_lo, st_hi = (min(stt_set), max(stt_set)) if stt_set else (0, 0)
            YDT = GDT if compute == "split" else F32
            if compute == "split":
                w16 = meta.tile([128, PDIM], GDT)
                nc.vector.tensor_copy(out=w16[:], in_=w_sb[:])
            else:
                w16 = w_sb

            def consume(gflat, off, t):
                """gflat: [128, >=off+PDIM] gathered rows tile; tile index t."""
                if t in pe_set:
                    # pooled[b,:] += sel_t[:,b]^T @ G ; W applied once at the end
                    nc.tensor.matmul(
                        out=pool_a[:],
                        lhsT=sel_sb[:, t * B : (t + 1) * B],
                        rhs=gflat[:, off : off + HALF],
                        start=(t == pe_lo),
                        stop=(t == pe_hi),
                    )
                    nc.tensor.matmul(
                        out=pool_b[:],
                        lhsT=sel_sb[:, t * B : (t + 1) * B],
                        rhs=gflat[:, off + HALF : off + PDIM],
                        start=(t == pe_lo),
                        stop=(t == pe_hi),
                    )
                    return
                y = yp.tile([128, 1], YDT)
                gs = gflat[:, off : off + PDIM]
                nc.vector.scalar_tensor_tensor(
                    out=gs,
                    in0=gs,
                    scalar=1.0,
                    in1=w16[:],
                    op0=mybir.AluOpType.mult,
                    op1=mybir.AluOpType.mult,
                    accum_out=y[:],
                )
                nc.tensor.matmul(
                    out=dot_ps[:],
                    lhsT=y[:],
                    rhs=sel_sb[:, t * B : (t + 1) * B],
                    start=(t == st_lo),
                    stop=(t == st_hi),
                )

            if mode == "dmag":
                # ramp-in: first tiles as single-row-set indirect gathers (low
                # latency), remainder as bulk dma_gather chunks (low overhead)
                nramp = min(ramp_ind, T)
                ramp_insts = []
                for t in range(nramp):
                    gi = gp.tile([128, PDIMP], GDT, tag="gi")
                    gi_inst = nc.gpsimd.indirect_dma_start(
                        out=gi[:],
                        out_offset=None,
                        in_=emb[:],
                        in_offset=bass.IndirectOffsetOnAxis(
                            ap=idx32_sb[:, t : t + 1], axis=0
                        ),
                    )
                    # keep the low-latency ramp singles in issue order
                    if ramp_insts:
                        add_dep_helper(gi_inst.ins, ramp_insts[-1].ins, reason="ramp order")
                    ramp_insts.append(gi_inst)
                    consume(gi[:], 0, t)
                sched = []
                rem = T - nramp
                while rem > 0:
                    c = min(chunk, rem)
                    sched.append(c)
                    rem -= c
                s = nramp
                first_dmag = True
                for c in sched:
                    g = gp.tile([128, c, PDIMP], GDT, tag="g")
                    dg_inst = nc.gpsimd.dma_gather(
                        out_ap=g[:],
                        in_ap=emb[:],
                        idxs_ap=idx16_sb[:, s * 8 : (s + c) * 8],
                        num_idxs=c * 128,
                        num_idxs_reg=c * 128,
                        elem_size=PDIMP,
                    )
                    if first_dmag and ramp_insts:
                        add_dep_helper(dg_inst.ins, ramp_insts[-1].ins, reason="ramp first")
                        first_dmag = False
                    gflat = g[:].rearrange("p c e -> p (c e)")
                    for j in range(c):
                        consume(gflat, j * PDIMP, s + j)
                    s += c
            else:  # indirect: one [128, PDIMP] row-gather per tile
                for t in range(T):
                    g = gp.tile([128, PDIMP], F32, tag="g")
                    nc.gpsimd.indirect_dma_start(
                        out=g[:],
                        out_offset=None,
                        in_=emb[:],
                        in_offset=bass.IndirectOffsetOnAxis(
                            ap=idx32_sb[:, t : t + 1], axis=0
                        ),
                    )
                    consume(g[:], 0, t)

            if compute in ("pe", "split"):
                pooled_sb = meta.tile([B, PDIM], F32)
                nc.vector.tensor_copy(out=pooled_sb[:, :HALF], in_=pool_a[:])
                nc.vector.tensor_copy(out=pooled_sb[:, HALF:], in_=pool_b[:])
                scr = meta.tile([B, PDIM], F32)
                y64 = meta.tile([B, 1], F32)
                nc.vector.scalar_tensor_tensor(
                    out=scr[:],
                    in0=pooled_sb[:],
                    scalar=1.0,
                    in1=w_sb[:B, :],
                    op0=mybir.AluOpType.mult,
                    op1=mybir.AluOpType.mult,
                    accum_out=y64[:],
                )
                if compute == "split":
                    # fold the stt-half partial [1,B] into partition-major form
                    part1_sb = meta.tile([1, B], F32)
                    nc.vector.tensor_copy(out=part1_sb[:], in_=dot_ps[:])
                    ident1 = meta.tile([1, 1], F32)
                    nc.vector.memset(ident1[:], 1.0)
                    dot_t = pp.tile([B, 1], F32, tag="dt")
                    nc.tensor.transpose(out=dot_t[:], in_=part1_sb[:], identity=ident1[:])
                    both = meta.tile([B, 1], F32)
                    nc.vector.tensor_tensor(
                        out=both[:], in0=y64[:], in1=dot_t[:], op=mybir.AluOpType.add
                    )
                    part_sb = both
                else:
                    part_sb = y64
                cc_in = dp.tile([B, 1], F32)
            else:
                part_sb = meta.tile([1, B], F32)
                nc.vector.tensor_copy(out=part_sb[:], in_=dot_ps[:])
                cc_in = dp.tile([1, B], F32)
            nc.sync.dma_start(out=cc_in[:], in_=part_sb[:])
            pmajor = compute in ("pe", "split")
            if cc == "ag":
                cc_out = dp.tile([NCORES * B, 1] if pmajor else [NCORES, B], F32)
                nc.gpsimd.collective_compute(
                    "AllGather",
                    mybir.AluOpType.bypass,
                    replica_groups=[list(range(NCORES))],
                    ins=[cc_in[:]],
                    outs=[cc_out[:]],
                )
                allg_sb = meta.tile([NCORES, B], F32)
                nc.sync.dma_start(out=allg_sb[:], in_=cc_out[:].rearrange("a b -> (a b)").rearrange("(c n) -> c n", c=NCORES) if pmajor else cc_out[:])
                ones_sb = meta.tile([NCORES, 1], F32)
                nc.vector.memset(ones_sb[:], 1.0)
                sum_ps = pp.tile([1, B], F32, tag="sum")
                nc.tensor.matmul(
                    out=sum_ps[:],
                    lhsT=ones_sb[:],
                    rhs=allg_sb[:],
                    start=True,
                    stop=True,
                )
                red_ap = sum_ps[:]
            else:
                cc_out = dp.tile([1, B], F32)
                nc.gpsimd.collective_compute(
                    "AllReduce",
                    mybir.AluOpType.add,
                    replica_groups=[list(range(NCORES))],
                    ins=[cc_in[:]],
                    outs=[cc_out[:]],
                )
                red_sb = meta.tile([1, B], F32)
                nc.sync.dma_start(out=red_sb[:], in_=cc_out[:])
                red_ap = red_sb[:]
            o_sb = meta.tile([1, B], F32)
            nc.scalar.activation(
                out=o_sb[:],
                in_=red_ap,
                func=mybir.ActivationFunctionType.Sigmoid,
                bias=b_sb[:],
                scale=1.0 / float(L),
            )
            nc.sync.dma_start(out=outp[:], in_=o_sb[:])

    nc.compile()
    if legalize:
        _legalize_sem_waits(nc, mybir)
    return nc


def _marshal(tokens, lengths, emb_table, W, b, dtype="f32"):
    if dtype == "bf16":
        import ml_dtypes

        sdt = ml_dtypes.bfloat16
    else:
        sdt = np.float32
    tokens = np.asarray(tokens)
    lengths = np.asarray(lengths).astype(np.int64)
    emb_table = np.ascontiguousarray(emb_table, dtype=np.float32)

    mask = np.arange(L)[None, :] < lengths[:, None]
    flat_tok = tokens[mask].astype(np.int64)
    flat_b = np.broadcast_to(np.arange(B)[:, None], (B, L))[mask]
    uniq, inv = np.unique(flat_tok, return_inverse=True)
    U = len(uniq)
    cnt = np.zeros((U, B), dtype=np.float32)
    np.add.at(cnt, (inv, flat_b), 1.0)

    bounds = [U * c // NCORES for c in range(NCORES + 1)]
    rows_max = max(bounds[c + 1] - bounds[c] for c in range(NCORES))
    T = -(-rows_max // 128)

    spans = []
    for c in range(NCORES):
        s, e = bounds[c], bounds[c + 1]
        lo = int(uniq[s]) if e > s else 0
        hi = int(uniq[e - 1]) + 1 if e > s else 1
        spans.append((s, e, lo, hi))
    Vmax = max(hi - lo for _, _, lo, hi in spans)

    wdt = np.float32 if os.environ.get("BERT_COMPUTE", "stt") in ("pe", "split") else sdt
    wrep = np.broadcast_to(
        np.asarray(W, dtype=np.float32).astype(wdt).reshape(1, PDIM), (128, PDIM)
    ).copy()
    brep = np.full((1, 1), np.float32(np.asarray(b).reshape(-1)[0]), dtype=np.float32)

    in_maps = []
    for c in range(NCORES):
        s, e, lo, hi = spans[c]
        span = hi - lo
        emb_c = np.zeros((Vmax, PDIMP), dtype=sdt)
        emb_c[:span, :PDIM] = emb_table[lo:hi].astype(sdt)
        rows = np.zeros(T * 128, dtype=np.int32)
        rows[: e - s] = (uniq[s:e] - lo).astype(np.int32)
        # int16 wrapped layout: index i -> [i % 16, i // 16], replicated x8
        wrapped = rows.astype(np.int16).reshape(T * 8, 16).T  # [16, T*8]
        idx16 = np.tile(wrapped, (8, 1)).copy()  # [128, T*8]
        seldt = sdt if os.environ.get("BERT_COMPUTE", "stt") in ("pe", "split") else np.float32
        selm = np.zeros((T * 128, B), dtype=seldt)
        selm[: e - s] = cnt[s:e].astype(seldt)
        in_maps.append(
            {
                "emb": emb_c,
                "idx16": idx16,
                "idx32": rows.reshape(T, 128).T.copy(),
                "sel": selm.reshape(T, 128, B).transpose(1, 0, 2).reshape(128, T * B).copy(),
                "wrep": wrep,
                "brep": brep,
            }
        )
    return T, Vmax, in_maps


def kernel(tokens, lengths, emb_table, W, b):
    from concourse.bass_utils import run_bass_kernel_spmd

    mode = os.environ.get("BERT_MODE", "dmag")
    chunk = int(os.environ.get("BERT_CHUNK", "8"))
    gbufs = int(os.environ.get("BERT_GBUFS", "4"))
    ybufs = int(os.environ.get("BERT_YBUFS", "16"))
    ramp_ind = int(os.environ.get("BERT_RAMPIND", "0"))
    cc = os.environ.get("BERT_CC", "ag")
    compute = os.environ.get("BERT_COMPUTE", "stt")
    trace = os.environ.get("BERT_TRACE", "0") == "1"

    dtype = os.environ.get("BERT_DTYPE", "bf16")
    T, Vmax, in_maps = _marshal(tokens, lengths, emb_table, W, b, dtype=dtype)
    nc = _build(Vmax, T, chunk, gbufs, mode=mode, ybufs=ybufs, ramp_ind=ramp_ind, cc=cc, dtype=dtype, compute=compute)
    res = run_bass_kernel_spmd(nc, in_maps, core_ids=list(range(NCORES)), trace=trace)
    LAST["results"] = res
    LAST["T"] = T
    LAST["Vmax"] = Vmax
    return res.results[0]["out"].reshape(B).astype(np.float32)

# ---------------------------------------------------------------------------
# Sequence-ownership variant: each core owns 8 sequences end-to-end (no
# collective, no cross-core skew sensitivity). Table replicated in bf16;
# gathers windowed into 32768-row vocab windows so rebased indices fit int16.
WIN = 32768
NW = -(-VOCAB // WIN)
NSEQ = B // NCORES
NSEL = 16  # sel columns per tile (NSEQ real + zero pad; DoubleRow wants %16)


def _marshal_seq(tokens, lengths, emb_table, W, b, dtype="bf16"):
    import ml_dtypes

    sdt = ml_dtypes.bfloat16 if dtype == "bf16" else np.float32
    tokens = np.asarray(tokens)
    lengths = np.asarray(lengths).astype(np.int64)

    # per-sequence unique-token histograms over vocab windows; greedy
    # vector-balancing assignment minimizes sum_w max_c rows (the padded
    # tile count is driven by per-window maxima, not total length)
    order = np.argsort(-lengths, kind="stable")
    hists = np.zeros((B, NW), dtype=np.int64)
    for bidx in range(B):
        u = np.unique(tokens[bidx, : lengths[bidx]].astype(np.int64))
        hists[bidx] = np.bincount(u // WIN, minlength=NW)
    Wc = np.zeros((NCORES, NW), dtype=np.int64)
    counts = np.zeros(NCORES, dtype=np.int64)
    assign = np.full((NCORES, NSEQ), -1, dtype=np.int64)
    for bidx in order:
        cands = np.where(counts < NSEQ)[0]
        best, bobj = None, None
        for c in cands:
            trial = Wc.copy()
            trial[c] += hists[bidx]
            obj = trial.max(axis=0).sum()
            if bobj is None or obj < bobj:
                best, bobj = c, obj
        assign[best, counts[best]] = bidx
        counts[best] += 1
        Wc[best] += hists[bidx]

    def _obj(Wm):
        return (-(-Wm.max(axis=0) // 128)).sum() * 1000000 + Wm.max(axis=0).sum()

    # swap refinement: directly minimize padded tile count sum_w ceil(max/128)
    for _ in range(40):
        improved = False
        cur = _obj(Wc)
        for c1 in range(NCORES):
            for j1 in range(NSEQ):
                for c2 in range(c1 + 1, NCORES):
                    for j2 in range(NSEQ):
                        b1, b2 = assign[c1, j1], assign[c2, j2]
                        trial = Wc.copy()
                        trial[c1] += hists[b2] - hists[b1]
                        trial[c2] += hists[b1] - hists[b2]
                        if _obj(trial) < cur:
                            assign[c1, j1], assign[c2, j2] = b2, b1
                            Wc = trial
                            cur = _obj(Wc)
                            improved = True
        if not improved:
            break

    per_core_rows = []  # (uniq, cnt8) per core
    for c in range(NCORES):
        toks = np.concatenate(
            [tokens[assign[c, j], : lengths[assign[c, j]]] for j in range(NSEQ)]
        ).astype(np.int64)
        locb = np.concatenate(
            [np.full(int(lengths[assign[c, j]]), j, dtype=np.int64) for j in range(NSEQ)]
        )
        uniq, inv = np.unique(toks, return_inverse=True)
        cnt8 = np.zeros((len(uniq), NSEQ), dtype=np.float32)
        np.add.at(cnt8, (inv, locb), 1.0)
        per_core_rows.append((uniq, cnt8))

    # per-window tile counts, common across cores (SPMD: same program)
    Tw = []
    bnds = []
    for w in range(NW):
        lo, hi = w * WIN, min((w + 1) * WIN, VOCAB)
        per_core_bnd = [
            (np.searchsorted(u, lo), np.searchsorted(u, hi)) for u, _ in per_core_rows
        ]
        bnds.append(per_core_bnd)
        Tw.append(max(-(-int(e - s) // 128) for s, e in per_core_bnd))
    T = sum(Tw)

    emb16 = np.zeros((VOCAB, PDIMP), dtype=sdt)
    emb16[:, :PDIM] = np.ascontiguousarray(emb_table, dtype=np.float32).astype(sdt)
    wdt = np.float32 if os.environ.get("BERT_SEQSPLIT", "1") == "1" else sdt
    wrep = np.broadcast_to(
        np.asarray(W, dtype=np.float32).astype(wdt).reshape(1, PDIM), (128, PDIM)
    ).copy()
    brep = np.full((NSEQ, 1), np.float32(np.asarray(b).reshape(-1)[0]), dtype=np.float32)

    in_maps = []
    for c in range(NCORES):
        uniq, cnt8 = per_core_rows[c]
        rows = np.zeros(T * 128, dtype=np.int16)
        selm = np.zeros((T * 128, NSEQ), dtype=np.float32)
        t0 = 0
        for w in range(NW):
            s0, e0 = bnds[w][c]
            n = int(e0 - s0)
            rows[t0 * 128 : t0 * 128 + n] = (uniq[s0:e0] - w * WIN).astype(np.int16)
            selm[t0 * 128 : t0 * 128 + n] = cnt8[s0:e0]
            t0 += Tw[w]
        if os.environ.get("BERT_SEQSPLIT", "1") == "1":
            selm = selm.astype(sdt)
        wrapped = rows.reshape(T * 8, 16).T  # [16, T*8]
        in_maps.append(
            {
                "emb": emb16,
                "idx16": np.tile(wrapped, (8, 1)).copy(),
                "sel": selm.reshape(T, 128, NSEQ)
                .transpose(1, 0, 2)
                .reshape(128, T * NSEQ)
                .copy(),
                "wrep": wrep,
                "brep": brep,
            }
        )
    return Tw, in_maps, assign


def _build_seq(Tw, chunk, gbufs, ybufs, dtype="bf16", legalize=True, split=True):
    from concourse import bacc, mybir
    import concourse.tile as tile

    F32 = mybir.dt.float32
    GDT = mybir.dt.bfloat16 if dtype == "bf16" else F32
    I16 = mybir.dt.int16
    T = sum(Tw)

    scratch = int(os.environ.get("BERT_DMASCRATCH", "131072"))
    nc = bacc.Bacc(None, num_devices=NCORES, dynamic_dma_scratch_size=scratch)
    emb = nc.declare_dram_parameter("emb", [VOCAB, PDIMP], GDT, isOutput=False)
    idx16 = nc.declare_dram_parameter("idx16", [128, T * 8], I16, isOutput=False)
    SELDT = GDT if split else F32
    sel = nc.declare_dram_parameter("sel", [128, T * NSEQ], SELDT, isOutput=False)
    WDT = F32 if split else GDT
    wrep = nc.declare_dram_parameter("wrep", [128, PDIM], WDT, isOutput=False)
    brep = nc.declare_dram_parameter("brep", [NSEQ, 1], F32, isOutput=False)
    outp = nc.declare_dram_parameter("out", [1, NSEQ], F32, isOutput=True)

    with tile.TileContext(nc) as tc:
        with (
            tc.tile_pool(name="meta", bufs=1) as meta,
            tc.tile_pool(name="g", bufs=gbufs) as gp,
            tc.tile_pool(name="y", bufs=ybufs) as yp,
            tc.tile_pool(name="ps", bufs=1, space="PSUM") as pp,
        ):
            idx16_sb = meta.tile([128, T * 8], I16)
            nc.sync.dma_start(out=idx16_sb[:], in_=idx16[:])
            sel_sb = meta.tile([128, T * NSEQ], SELDT)
            nc.sync.dma_start(out=sel_sb[:], in_=sel[:])
            w_sb = meta.tile([128, PDIM], WDT)
            nc.sync.dma_start(out=w_sb[:], in_=wrep[:])
            b_sb = meta.tile([NSEQ, 1], F32)
            nc.sync.dma_start(out=b_sb[:], in_=brep[:])

            dot_ps = pp.tile([1, NSEQ], F32)
            first_chunk = True
            HALF = PDIM // 2
            if split:
                # DVE handles even tiles (row.W dot), PE handles odd tiles
                # (pooled accumulation); W applied to the pooled half once.
                pe_set = set(range(1, T, 2))
                dot8 = pp.tile([NSEQ, 1], F32, tag="d8")
                pool_a = pp.tile([NSEQ, HALF], F32, tag="pa")
                pool_b = pp.tile([NSEQ, HALF], F32, tag="pb")
                w16 = meta.tile([128, PDIM], GDT)
                nc.vector.tensor_copy(out=w16[:], in_=w_sb[:])
            else:
                pe_set = set()
                w16 = w_sb
            stt_set = set(range(T)) - pe_set
            pe_lo, pe_hi = (min(pe_set), max(pe_set)) if pe_set else (0, 0)
            st_lo, st_hi = (min(stt_set), max(stt_set)) if stt_set else (0, 0)
            t = 0
            for w in range(NW):
                wlo = w * WIN
                whi = min(wlo + WIN, VOCAB)
                left = Tw[w]
                while left > 0:
                    # small first chunk: first gathered data lands sooner,
                    # cutting pipeline ramp-in before the consumers start
                    c = min(4 if first_chunk else chunk, left)
                    first_chunk = False
                    g = gp.tile([128, c, PDIMP], GDT, tag="g")
                    nc.gpsimd.dma_gather(
                        out_ap=g[:],
                        in_ap=emb[wlo:whi],
                        idxs_ap=idx16_sb[:, t * 8 : (t + c) * 8],
                        num_idxs=c * 128,
                        num_idxs_reg=c * 128,
                        elem_size=PDIMP,
                    )
                    gflat = g[:].rearrange("p c e -> p (c e)")
                    for j in range(c):
                        tt = t + j
                        off = j * PDIMP
                        if tt in pe_set:
                            nc.tensor.matmul(
                                out=pool_a[:],
                                lhsT=sel_sb[:, tt * NSEQ : (tt + 1) * NSEQ],
                                rhs=gflat[:, off : off + HALF],
                                start=(tt == pe_lo),
                                stop=(tt == pe_hi),
                            )
                            nc.tensor.matmul(
                                out=pool_b[:],
                                lhsT=sel_sb[:, tt * NSEQ : (tt + 1) * NSEQ],
                                rhs=gflat[:, off + HALF : off + PDIM],
                                start=(tt == pe_lo),
                                stop=(tt == pe_hi),
                            )
                            continue
                        y = yp.tile([128, 1], GDT if split else F32)
                        gs = gflat[:, off : off + PDIM]
                        nc.vector.scalar_tensor_tensor(
                            out=gs,
                            in0=gs,
                            scalar=1.0,
                            in1=w16[:],
                            op0=mybir.AluOpType.mult,
                            op1=mybir.AluOpType.mult,
                            accum_out=y[:],
                        )
                        if split:
                            nc.tensor.matmul(
                                out=dot8[:],
                                lhsT=sel_sb[:, tt * NSEQ : (tt + 1) * NSEQ],
                                rhs=y[:],
                                start=(tt == st_lo),
                                stop=(tt == st_hi),
                            )
                        else:
                            nc.tensor.matmul(
                                out=dot_ps[:],
                                lhsT=y[:],
                                rhs=sel_sb[:, tt * NSEQ : (tt + 1) * NSEQ],
                                start=(tt == st_lo),
                                stop=(tt == st_hi),
                            )
                    t += c
                    left -= c

            if split:
                pooled_sb = meta.tile([NSEQ, PDIM], F32)
                nc.vector.tensor_copy(out=pooled_sb[:, :HALF], in_=pool_a[:])
                nc.vector.tensor_copy(out=pooled_sb[:, HALF:], in_=pool_b[:])
                scr = meta.tile([NSEQ, PDIM], F32)
                y8 = meta.tile([NSEQ, 1], F32)
                nc.vector.scalar_tensor_tensor(
                    out=scr[:],
                    in0=pooled_sb[:],
                    scalar=1.0,
                    in1=w_sb[:NSEQ, :],
                    op0=mybir.AluOpType.mult,
                    op1=mybir.AluOpType.mult,
                    accum_out=y8[:],
                )
                part = meta.tile([NSEQ, 1], F32)
                nc.vector.tensor_tensor(
                    out=part[:], in0=dot8[:], in1=y8[:], op=mybir.AluOpType.add
                )
                o_sb = meta.tile([NSEQ, 1], F32)
                nc.scalar.activation(
                    out=o_sb[:],
                    in_=part[:],
                    func=mybir.ActivationFunctionType.Sigmoid,
                    bias=b_sb[:],
                    scale=1.0 / float(L),
                )
                nc.sync.dma_start(out=outp[0, :, None], in_=o_sb[:])
            else:
                o_sb = meta.tile([1, NSEQ], F32)
                nc.scalar.activation(
                    out=o_sb[:],
                    in_=dot_ps[:],
                    func=mybir.ActivationFunctionType.Sigmoid,
                    bias=b_sb[:1, :],
                    scale=1.0 / float(L),
                )
                nc.sync.dma_start(out=outp[:], in_=o_sb[:])

    nc.compile()
    if legalize:
        _legalize_sem_waits(nc, __import__("concourse.mybir", fromlist=["x"]))
    return nc


def _kernel_seq(tokens, lengths, emb_table, W, b):
    from concourse.bass_utils import run_bass_kernel_spmd

    dtype = os.environ.get("BERT_DTYPE", "bf16")
    chunk = int(os.environ.get("BERT_CHUNK", "8"))
    gbufs = int(os.environ.get("BERT_GBUFS", "4"))
    ybufs = int(os.environ.get("BERT_YBUFS", "16"))
    trace = os.environ.get("BERT_TRACE", "0") == "1"

    split = os.environ.get("BERT_SEQSPLIT", "1") == "1"
    Tw, in_maps, assign = _marshal_seq(tokens, lengths, emb_table, W, b, dtype=dtype)
    nc = _build_seq(Tw, chunk, gbufs, ybufs, dtype=dtype, split=split)
    res = run_bass_kernel_spmd(nc, in_maps, core_ids=list(range(NCORES)), trace=trace)
    LAST["results"] = res
    LAST["T"] = sum(Tw)
    LAST["Vmax"] = VOCAB
    out = np.zeros(B, dtype=np.float32)
    for c in range(NCORES):
        vals = res.results[c]["out"].reshape(-1)
        for j in range(NSEQ):
            out[assign[c, j]] = vals[j]
    return out


_kernel_vocab = kernel


# ---------------------------------------------------------------------------
# fp8 all-PE variant (BERT_SHARD=pe8, default): sequence-ownership sharding as
# above, but the table is cast to fp8e4 (halves gather DMA traffic; final
# sigmoid output error ~1e-4 << 2e-2 budget) and ALL per-tile compute runs on
# the PE: pooled[seq,:] += sel_t^T @ g_t accumulated in two PSUM banks across
# every tile. This removes the DVE<->PE zigzag (STT -> dot8 -> pool-MM) that
# paced the old pipeline at ~10.3us per 8-tile chunk with no engine saturated.
# The W dot + sigmoid happen once on [8,1000] at the end.
def _marshal_pe8(tokens, lengths, emb_table, W, b):
    import ml_dtypes

    f8 = ml_dtypes.float8_e4m3
    tokens = np.asarray(tokens)
    lengths = np.asarray(lengths).astype(np.int64)

    order = np.argsort(-lengths, kind="stable")
    hists = np.zeros((B, NW), dtype=np.int64)
    for bidx in range(B):
        u = np.unique(tokens[bidx, : lengths[bidx]].astype(np.int64))
        hists[bidx] = np.bincount(u // WIN, minlength=NW)
    Wc = np.zeros((NCORES, NW), dtype=np.int64)
    counts = np.zeros(NCORES, dtype=np.int64)
    assign = np.full((NCORES, NSEQ), -1, dtype=np.int64)
    for bidx in order:
        cands = np.where(counts < NSEQ)[0]
        best, bobj = None, None
        for c in cands:
            trial = Wc.copy()
            trial[c] += hists[bidx]
            obj = trial.max(axis=0).sum()
            if bobj is None or obj < bobj:
                best, bobj = c, obj
        assign[best, counts[best]] = bidx
        counts[best] += 1
        Wc[best] += hists[bidx]

    def _obj(Wm):
        return (-(-Wm.max(axis=0) // 128)).sum() * 1000000 + Wm.max(axis=0).sum()

    for _ in range(40):
        improved = False
        cur = _obj(Wc)
        for c1 in range(NCORES):
            for j1 in range(NSEQ):
                for c2 in range(c1 + 1, NCORES):
                    for j2 in range(NSEQ):
                        b1, b2 = assign[c1, j1], assign[c2, j2]
                        trial = Wc.copy()
                        trial[c1] += hists[b2] - hists[b1]
                        trial[c2] += hists[b1] - hists[b2]
                        if _obj(trial) < cur:
                            assign[c1, j1], assign[c2, j2] = b2, b1
                            Wc = trial
                            cur = _obj(Wc)
                            improved = True
        if not improved:
            break

    per_core_rows = []
    for c in range(NCORES):
        toks = np.concatenate(
            [tokens[assign[c, j], : lengths[assign[c, j]]] for j in range(NSEQ)]
        ).astype(np.int64)
        locb = np.concatenate(
            [np.full(int(lengths[assign[c, j]]), j, dtype=np.int64) for j in range(NSEQ)]
        )
        uniq, inv = np.unique(toks, return_inverse=True)
        cnt8 = np.zeros((len(uniq), NSEQ), dtype=np.float32)
        np.add.at(cnt8, (inv, locb), 1.0)
        per_core_rows.append((uniq, cnt8))

    Tw = []
    bnds = []
    for w in range(NW):
        lo, hi = w * WIN, min((w + 1) * WIN, VOCAB)
        per_core_bnd = [
            (np.searchsorted(u, lo), np.searchsorted(u, hi)) for u, _ in per_core_rows
        ]
        bnds.append(per_core_bnd)
        Tw.append(max(-(-int(e - s) // 128) for s, e in per_core_bnd))
    T = sum(Tw)

    emb8 = np.zeros((VOCAB, PDIMP), dtype=f8)
    emb8[:, :PDIM] = np.ascontiguousarray(emb_table, dtype=np.float32).astype(f8)
    w8 = np.ascontiguousarray(
        np.broadcast_to(np.asarray(W, dtype=np.float32).reshape(1, PDIM), (NSEQ, PDIM))
    )
    brep = np.full((NSEQ, 1), np.float32(np.asarray(b).reshape(-1)[0]), dtype=np.float32)

    gmode = os.environ.get("BERT_GMODE", "dmag")
    if gmode == "ind":
        # int32 full-vocab row indices -> no 32768-row windows, no window
        # padding; T is just the cross-core max tile count.
        T = max(-(-len(u) // 128) for u, _ in per_core_rows)
        in_maps = []
        for c in range(NCORES):
            uniq, cnt8 = per_core_rows[c]
            n = len(uniq)
            rows = np.zeros(T * 128, dtype=np.int32)
            rows[:n] = uniq.astype(np.int32)
            selm = np.zeros((T * 128, NSEL), dtype=np.float32)
            selm[:n, :NSEQ] = cnt8
            in_maps.append(
                {
                    "emb": emb8,
                    "idx32": rows.reshape(T, 128).T.copy(),
                    "sel": selm.astype(f8)
                    .reshape(T, 128, NSEL)
                    .transpose(1, 0, 2)
                    .reshape(128, T * NSEL)
                    .copy(),
                    "wrep": w8,
                    "brep": brep,
                }
            )
        return [T], in_maps, assign

    in_maps = []
    for c in range(NCORES):
        uniq, cnt8 = per_core_rows[c]
        rows = np.zeros(T * 128, dtype=np.int16)
        selm = np.zeros((T * 128, NSEQ), dtype=np.float32)
        t0 = 0
        for w in range(NW):
            s0, e0 = bnds[w][c]
            n = int(e0 - s0)
            rows[t0 * 128 : t0 * 128 + n] = (uniq[s0:e0] - w * WIN).astype(np.int16)
            selm[t0 * 128 : t0 * 128 + n] = cnt8[s0:e0]
            t0 += Tw[w]
        selp = np.zeros((T * 128, NSEL), dtype=np.float32)
        selp[:, :NSEQ] = selm
        wrapped = rows.reshape(T * 8, 16).T  # [16, T*8]
        in_maps.append(
            {
                "emb": emb8,
                "idx16": np.tile(wrapped, (8, 1)).copy(),
                "sel": selp.astype(f8)
                .reshape(T, 128, NSEL)
                .transpose(1, 0, 2)
                .reshape(128, T * NSEL)
                .copy(),
                "wrep": w8,
                "brep": brep,
            }
        )
    return Tw, in_maps, assign


def _build_pe8(Tw, chunk, gbufs, legalize=True, gmode="dmag", nq=1):
    from concourse import bass, bacc, mybir
    import concourse.tile as tile

    F32 = mybir.dt.float32
    F8 = mybir.dt.float8e4
    I16 = mybir.dt.int16
    I32 = mybir.dt.int32
    T = sum(Tw)

    nc = bacc.Bacc(None, num_devices=NCORES, num_swdge_queues=nq)
    emb = nc.declare_dram_parameter("emb", [VOCAB, PDIMP], F8, isOutput=False)
    if gmode == "ind":
        idx32 = nc.declare_dram_parameter("idx32", [128, T], I32, isOutput=False)
    else:
        idx16 = nc.declare_dram_parameter("idx16", [128, T * 8], I16, isOutput=False)
    sel = nc.declare_dram_parameter("sel", [128, T * NSEL], F8, isOutput=False)
    wrep = nc.declare_dram_parameter("wrep", [NSEQ, PDIM], F32, isOutput=False)
    brep = nc.declare_dram_parameter("brep", [NSEQ, 1], F32, isOutput=False)
    outp = nc.declare_dram_parameter("out", [1, NSEQ], F32, isOutput=True)

    HALF = PDIM // 2
    with tile.TileContext(nc) as tc:
        with (
            tc.tile_pool(name="meta", bufs=1) as meta,
            tc.tile_pool(name="g", bufs=gbufs) as gp,
            tc.tile_pool(name="ps", bufs=1, space="PSUM") as pp,
        ):
            if gmode == "ind":
                idx32_sb = meta.tile([128, T], I32)
                nc.sync.dma_start(out=idx32_sb[:], in_=idx32[:])
            else:
                idx16_sb = meta.tile([128, T * 8], I16)
                nc.sync.dma_start(out=idx16_sb[:], in_=idx16[:])
            sel_sb = meta.tile([128, T * NSEL], F8)
            nc.sync.dma_start(out=sel_sb[:], in_=sel[:])
            w_sb = meta.tile([NSEQ, PDIM], F32)
            nc.sync.dma_start(out=w_sb[:], in_=wrep[:])
            b_sb = meta.tile([NSEQ, 1], F32)
            nc.sync.dma_start(out=b_sb[:], in_=brep[:])

            DRH = PDIMP // 2  # 512-col halves: DoubleRow needs inner %16==0
            pool_a = pp.tile([NSEL, DRH], F32, tag="pa")
            pool_b = pp.tile([NSEL, DRH], F32, tag="pb")
            use_dr = os.environ.get("BERT_DR", "1") == "1"

            def consume(gflat, off, tt):
                nc.tensor.matmul(
                    out=pool_a[:],
                    lhsT=sel_sb[:, tt * NSEL : tt * NSEL + NSEL],
                    rhs=gflat[:, off : off + DRH],
                    start=(tt == 0),
                    stop=(tt == T - 1),
                )
                nc.tensor.matmul(
                    out=pool_b[:],
                    lhsT=sel_sb[:, tt * NSEL : tt * NSEL + NSEL],
                    rhs=gflat[:, off + DRH : off + PDIMP],
                    start=(tt == 0),
                    stop=(tt == T - 1),
                )

            def consume_pair(g3, j, tt):
                """DoubleRow: tiles tt, tt+1 (chunk slots j, j+1) in one pass.

                g3 is the [128, c, PDIMP] chunk tile; rhs [128, 2, 512] pairs
                the two tiles' half-columns as the two k-tiles (512 not 500:
                DoubleRow needs inner size % 16 == 0; cols 1000-1023 are zero
                pad in the table so they add nothing), lhsT [128, 2, NSEQ] is
                their adjacent sel slices.
                """
                sel2 = sel_sb[:, tt * NSEL : (tt + 2) * NSEL].rearrange(
                    "p (two f) -> p two f", two=2
                )
                nc.tensor.matmul(
                    out=pool_a[:],
                    lhsT=sel2,
                    rhs=g3[:, j : j + 2, 0:DRH],
                    start=(tt == 0),
                    stop=(tt + 1 == T - 1),
                    perf_mode=mybir.MatmulPerfMode.DoubleRow,
                )
                nc.tensor.matmul(
                    out=pool_b[:],
                    lhsT=sel2,
                    rhs=g3[:, j : j + 2, DRH:PDIMP],
                    start=(tt == 0),
                    stop=(tt + 1 == T - 1),
                    perf_mode=mybir.MatmulPerfMode.DoubleRow,
                )

            if gmode == "ind":
                for t in range(T):
                    g = gp.tile([128, PDIMP], F8, tag="g")
                    nc.gpsimd.indirect_dma_start(
                        out=g[:],
                        out_offset=None,
                        in_=emb[:],
                        in_offset=bass.IndirectOffsetOnAxis(
                            ap=idx32_sb[:, t : t + 1], axis=0
                        ),
                    )
                    consume(g[:], 0, t)
            else:
                t = 0
                gi = 0
                first_chunk = True
                for w in range(NW):
                    wlo = w * WIN
                    whi = min(wlo + WIN, VOCAB)
                    left = Tw[w]
                    while left > 0:
                        c = min(4 if first_chunk else chunk, left)
                        first_chunk = False
                        g = gp.tile([128, c, PDIMP], F8, tag="g")
                        nc.gpsimd.dma_gather(
                            out_ap=g[:],
                            in_ap=emb[wlo:whi],
                            idxs_ap=idx16_sb[:, t * 8 : (t + c) * 8],
                            num_idxs=c * 128,
                            num_idxs_reg=c * 128,
                            elem_size=PDIMP,
                            queue_num=gi % nq,
                        )
                        gi += 1
                        gflat = g[:].rearrange("p c e -> p (c e)")
                        j = 0
                        while j < c:
                            if use_dr and j + 1 < c:
                                consume_pair(g[:], j, t + j)
                                j += 2
                            else:
                                consume(gflat, j * PDIMP, t + j)
                                j += 1
                        t += c
                        left -= c

            pooled_sb = meta.tile([NSEQ, PDIMP], F32)
            nc.vector.tensor_copy(out=pooled_sb[:, :DRH], in_=pool_a[:NSEQ, :])
            nc.vector.tensor_copy(out=pooled_sb[:, DRH:], in_=pool_b[:NSEQ, :])
            scr = meta.tile([NSEQ, PDIM], F32)
            y8 = meta.tile([NSEQ, 1], F32)
            nc.vector.scalar_tensor_tensor(
                out=scr[:],
                in0=pooled_sb[:, :PDIM],
                scalar=1.0,
                in1=w_sb[:],
                op0=mybir.AluOpType.mult,
                op1=mybir.AluOpType.mult,
                accum_out=y8[:],
            )
            o_sb = meta.tile([NSEQ, 1], F32)
            nc.scalar.activation(
                out=o_sb[:],
                in_=y8[:],
                func=mybir.ActivationFunctionType.Sigmoid,
                bias=b_sb[:],
                scale=1.0 / float(L),
            )
            nc.sync.dma_start(out=outp[0, :, None], in_=o_sb[:])

    nc.compile()
    if legalize:
        _legalize_sem_waits(nc, __import__("concourse.mybir", fromlist=["x"]))
    return nc


def _kernel_pe8(tokens, lengths, emb_table, W, b):
    from concourse.bass_utils import run_bass_kernel_spmd

    chunk = int(os.environ.get("BERT_CHUNK", "8"))
    gbufs = int(os.environ.get("BERT_GBUFS", "4"))
    gmode = os.environ.get("BERT_GMODE", "dmag")
    nq = int(os.environ.get("BERT_NQ", "4"))
    trace = os.environ.get("BERT_TRACE", "0") == "1"

    Tw, in_maps, assign = _marshal_pe8(tokens, lengths, emb_table, W, b)
    nc = _build_pe8(Tw, chunk, gbufs, gmode=gmode, nq=nq)
    res = run_bass_kernel_spmd(nc, in_maps, core_ids=list(range(NCORES)), trace=trace)
    LAST["results"] = res
    LAST["T"] = sum(Tw)
    LAST["Vmax"] = VOCAB
    out = np.zeros(B, dtype=np.float32)
    for c in range(NCORES):
        vals = res.results[c]["out"].reshape(-1)
        for j in range(NSEQ):
            out[assign[c, j]] = vals[j]
    return out


# ---------------------------------------------------------------------------
# Vocab-sharded fp8 all-PE variant (BERT_SHARD=vp8): global dedup across all
# 64 sequences, unique rows split into 8 equal contiguous vocab chunks (each
# span < 32768 so int16 indices need no windows). Each core gathers ~U/8 rows
# (~7.1k vs ~9.5k for the seq split -- the Pool engine's descriptor-gen ucode
# at ~8.5ns/row is the wall, so fewer rows is the lever), accumulates
# pooled[64,1000] on the PE, dots with W, and an AllGather (warmed up early)
# combines the per-core [64] partials.
def _marshal_vp8(tokens, lengths, emb_table, W, b):
    import ml_dtypes

    f8 = ml_dtypes.float8_e4m3
    tokens = np.asarray(tokens)
    lengths = np.asarray(lengths).astype(np.int64)

    mask = np.arange(L)[None, :] < lengths[:, None]
    flat_tok = tokens[mask].astype(np.int64)
    flat_b = np.broadcast_to(np.arange(B)[:, None], (B, L))[mask]
    uniq, inv = np.unique(flat_tok, return_inverse=True)
    U = len(uniq)
    cnt = np.zeros((U, B), dtype=np.float32)
    np.add.at(cnt, (inv, flat_b), 1.0)

    bounds = [U * c // NCORES for c in range(NCORES + 1)]
    T = max(-(-(bounds[c + 1] - bounds[c]) // 128) for c in range(NCORES))
    spans = []
    for c in range(NCORES):
        s, e = bounds[c], bounds[c + 1]
        lo = int(uniq[s]) if e > s else 0
        hi = int(uniq[e - 1]) + 1 if e > s else 1
        assert hi - lo < 32768, f"core {c} vocab span {hi-lo} exceeds int16"
        spans.append((s, e, lo, hi))
    Vmax = max(hi - lo for _, _, lo, hi in spans)

    emb8 = np.zeros((VOCAB, PDIMP), dtype=f8)
    emb8[:, :PDIM] = np.ascontiguousarray(emb_table, dtype=np.float32).astype(f8)
    w64 = np.ascontiguousarray(
        np.broadcast_to(np.asarray(W, dtype=np.float32).reshape(1, PDIM), (B, PDIM))
    )
    brep = np.full((1, 1), np.float32(np.asarray(b).reshape(-1)[0]), dtype=np.float32)

    in_maps = []
    for c in range(NCORES):
        s, e, lo, hi = spans[c]
        emb_c = np.zeros((Vmax, PDIMP), dtype=f8)
        emb_c[: hi - lo] = emb8[lo:hi]
        rows = np.zeros(T * 128, dtype=np.int16)
        rows[: e - s] = (uniq[s:e] - lo).astype(np.int16)
        selm = np.zeros((T * 128, B), dtype=np.float32)
        selm[: e - s] = cnt[s:e]
        wrapped = rows.reshape(T * 8, 16).T  # [16, T*8]
        in_maps.append(
            {
                "emb": emb_c,
                "idx16": np.tile(wrapped, (8, 1)).copy(),
                "sel": selm.astype(f8)
                .reshape(T, 128, B)
                .transpose(1, 0, 2)
                .reshape(128, T * B)
                .copy(),
                "wrep": w64,
                "brep": brep,
            }
        )
    return T, Vmax, in_maps


def _build_vp8(T, Vmax, chunk, gbufs, legalize=True, ccwarm=True):
    from concourse import bacc, mybir
    import concourse.tile as tile

    F32 = mybir.dt.float32
    F8 = mybir.dt.float8e4
    I16 = mybir.dt.int16

    nc = bacc.Bacc(None, num_devices=NCORES)
    emb = nc.declare_dram_parameter("emb", [Vmax, PDIMP], F8, isOutput=False)
    idx16 = nc.declare_dram_parameter("idx16", [128, T * 8], I16, isOutput=False)
    sel = nc.declare_dram_parameter("sel", [128, T * B], F8, isOutput=False)
    wrep = nc.declare_dram_parameter("wrep", [B, PDIM], F32, isOutput=False)
    brep = nc.declare_dram_parameter("brep", [1, 1], F32, isOutput=False)
    outp = nc.declare_dram_parameter("out", [1, B], F32, isOutput=True)

    HALF = PDIM // 2
    with tile.TileContext(nc) as tc:
        with (
            tc.tile_pool(name="meta", bufs=1) as meta,
            tc.tile_pool(name="g", bufs=gbufs) as gp,
            tc.tile_pool(name="ps", bufs=1, space="PSUM") as pp,
            tc.tile_pool(name="dram", bufs=1, space="DRAM") as dp,
        ):
            idx16_sb = meta.tile([128, T * 8], I16)
            nc.sync.dma_start(out=idx16_sb[:], in_=idx16[:])
            sel_sb = meta.tile([128, T * B], F8)
            nc.sync.dma_start(out=sel_sb[:], in_=sel[:])
            w_sb = meta.tile([B, PDIM], F32)
            nc.sync.dma_start(out=w_sb[:], in_=wrep[:])
            b_sb = meta.tile([1, 1], F32)
            nc.sync.dma_start(out=b_sb[:], in_=brep[:])

            if ccwarm:
                warm_sb = meta.tile([1, 4], F32)
                nc.vector.memset(warm_sb[:], 0.0)
                ccw_in = dp.tile([1, 4], F32)
                ccw_out = dp.tile([NCORES, 4], F32)
                nc.sync.dma_start(out=ccw_in[:], in_=warm_sb[:])
                nc.gpsimd.collective_compute(
                    "AllGather",
                    mybir.AluOpType.bypass,
                    replica_groups=[list(range(NCORES))],
                    ins=[ccw_in[:]],
                    outs=[ccw_out[:]],
                )

            pool_a = pp.tile([B, HALF], F32, tag="pa")
            pool_b = pp.tile([B, HALF], F32, tag="pb")

            t = 0
            first_chunk = True
            while t < T:
                c = min(4 if first_chunk else chunk, T - t)
                first_chunk = False
                g = gp.tile([128, c, PDIMP], F8, tag="g")
                nc.gpsimd.dma_gather(
                    out_ap=g[:],
                    in_ap=emb[:],
                    idxs_ap=idx16_sb[:, t * 8 : (t + c) * 8],
                    num_idxs=c * 128,
                    num_idxs_reg=c * 128,
                    elem_size=PDIMP,
                )
                gflat = g[:].rearrange("p c e -> p (c e)")
                for j in range(c):
                    tt = t + j
                    off = j * PDIMP
                    nc.tensor.matmul(
                        out=pool_a[:],
                        lhsT=sel_sb[:, tt * B : (tt + 1) * B],
                        rhs=gflat[:, off : off + HALF],
                        start=(tt == 0),
                        stop=(tt == T - 1),
                    )
                    nc.tensor.matmul(
                        out=pool_b[:],
                        lhsT=sel_sb[:, tt * B : (tt + 1) * B],
                        rhs=gflat[:, off + HALF : off + PDIM],
                        start=(tt == 0),
                        stop=(tt == T - 1),
                    )
                t += c

            pooled_sb = meta.tile([B, PDIM], F32)
            nc.vector.tensor_copy(out=pooled_sb[:, :HALF], in_=pool_a[:])
            nc.vector.tensor_copy(out=pooled_sb[:, HALF:], in_=pool_b[:])
            scr = meta.tile([B, PDIM], F32)
            y64 = meta.tile([B, 1], F32)
            nc.vector.scalar_tensor_tensor(
                out=scr[:],
                in0=pooled_sb[:],
                scalar=1.0,
                in1=w_sb[:],
                op0=mybir.AluOpType.mult,
                op1=mybir.AluOpType.mult,
                accum_out=y64[:],
            )
            cc_in = dp.tile([B, 1], F32)
            nc.sync.dma_start(out=cc_in[:], in_=y64[:])
            cc_out = dp.tile([NCORES * B, 1], F32)
            nc.gpsimd.collective_compute(
                "AllGather",
                mybir.AluOpType.bypass,
                replica_groups=[list(range(NCORES))],
                ins=[cc_in[:]],
                outs=[cc_out[:]],
            )
            allg_sb = meta.tile([NCORES, B], F32)
            nc.sync.dma_start(
                out=allg_sb[:],
                in_=cc_out[:].rearrange("a b -> (a b)").rearrange("(c n) -> c n", c=NCORES),
            )
            ones_sb = meta.tile([NCORES, 1], F32)
            nc.vector.memset(ones_sb[:], 1.0)
            sum_ps = pp.tile([1, B], F32, tag="sum")
            nc.tensor.matmul(
                out=sum_ps[:],
                lhsT=ones_sb[:],
                rhs=allg_sb[:],
                start=True,
                stop=True,
            )
            o_sb = meta.tile([1, B], F32)
            nc.scalar.activation(
                out=o_sb[:],
                in_=sum_ps[:],
                func=mybir.ActivationFunctionType.Sigmoid,
                bias=b_sb[:],
                scale=1.0 / float(L),
            )
            nc.sync.dma_start(out=outp[:], in_=o_sb[:])

    nc.compile()
    if legalize:
        _legalize_sem_waits(nc, __import__("concourse.mybir", fromlist=["x"]))
    return nc


def _kernel_vp8(tokens, lengths, emb_table, W, b):
    from concourse.bass_utils import run_bass_kernel_spmd

    chunk = int(os.environ.get("BERT_CHUNK", "8"))
    gbufs = int(os.environ.get("BERT_GBUFS", "4"))
    ccwarm = os.environ.get("BERT_CCWARM", "1") == "1"
    trace = os.environ.get("BERT_TRACE", "0") == "1"

    T, Vmax, in_maps = _marshal_vp8(tokens, lengths, emb_table, W, b)
    nc = _build_vp8(T, Vmax, chunk, gbufs, ccwarm=ccwarm)
    res = run_bass_kernel_spmd(nc, in_maps, core_ids=list(range(NCORES)), trace=trace)
    LAST["results"] = res
    LAST["T"] = T
    LAST["Vmax"] = Vmax
    return res.results[0]["out"].reshape(B).astype(np.float32)


def kernel(tokens, lengths, emb_table, W, b):
    shard = os.environ.get("BERT_SHARD", "vp8")
    if shard == "vp8":
        return _kernel_vp8(tokens, lengths, emb_table, W, b)
    if shard == "pe8":
        return _kernel_pe8(tokens, lengths, emb_table, W, b)
    if shard == "seq":
        return _kernel_seq(tokens, lengths, emb_table, W, b)
    return _kernel_vocab(tokens, lengths, emb_table, W, b)



# revision 20
# speedup vs baseline: 2.2445x; 1.0268x over previous
"""Trainium2 Bass kernel for nn_BerTII (masked-mean embedding bag -> 1-dim
linear -> sigmoid), distributed over 8 NeuronCores.

reference math:
  mask[b,l] = l < lengths[b]
  pooled[b,:] = sum_l mask[b,l] * emb[tokens[b,l],:] / L
  out[b] = sigmoid(pooled[b,:] @ W.T + bias)

The 1-output linear commutes with the masked mean:
  out[b] = sigmoid( (1/L) * sum_{l<len_b} (emb[tokens[b,l]] . W) + bias )
so the kernel never materializes the [B,L,P] gather. Host-side marshaling is
integer-only index work (the "all-to-all" of the sharding hint done at
input-staging time):
  - flatten all valid (b,l) tokens, dedupe globally (np.unique) and build a
    per-(unique-row, batch) multiplicity matrix;
  - split the unique rows into 8 equal-count contiguous chunks; core c
    receives ONLY the vocab slice spanning its chunk (rebased int16 indices),
    so the 800MB table is sharded across cores, not replicated; rows are
    padded 1000->1024 floats so each row is one 4KB 256B-aligned gather
    element;
  - each core bulk-gathers its ~U/8 rows with InstDMAGatherAnt, dots each row
    with W on the Vector engine (scalar_tensor_tensor accum), and accumulates
    per-batch partial dot products with tiny PE matmuls against the
    multiplicity matrix (y stationary [128,1], counts moving [128,64]);
  - an 8-core AllReduce(add) of the [64] partials, then sigmoid(x/L + b) on
    the Scalar engine. Every core emits the full [64] output; core 0's is
    returned.

DEFAULT (BERT_SHARD=seq): the sequence-ownership variant at the bottom of this
file instead — each core owns 8 length-balanced sequences end-to-end (table
replicated in bf16, int16 gathers windowed into 32768-row vocab slabs, no
collective), which removes cross-core straggler waits: ~129 us vs ~135-142 us
for the vocab-sharded path (BERT_SHARD=vocab).
"""
import os
import sys

sys.path.insert(0, "/opt/trn_rl_repo")

import numpy as np

VOCAB = 200000
PDIM = 1000
PDIMP = 1024  # row stride padded to 256B multiple for dma_gather
B = 64
L = 2048
NCORES = 8

LAST = {}  # debug: last BassKernelResults etc.


# ---------------------------------------------------------------------------
# walrus legalization: this toolchain allows at most ONE semaphore wait per
# instruction ("Too many sync wait commands"); split extras onto NoOps.
def _legalize_sem_waits(nc, mybir, max_waits=1):
    n = 0
    for f in nc.m.functions:
        for bb in f.blocks:
            new = []
            for inst in bb.instructions:
                si = inst.sync_info
                if si is not None and si.on_wait and len(si.on_wait) > max_waits:
                    waits = list(si.on_wait)
                    extra, keep = waits[:-max_waits], waits[-max_waits:]
                    k = 0
                    while extra:
                        chunk, extra = extra[:max_waits], extra[max_waits:]
                        new.append(
                            mybir.InstNoOp(
                                name=f"{inst.name}-ws{k}",
                                sync_info=mybir.SyncInfo(on_wait=chunk, on_update=[]),
                                bass_nofuse=True,
                                engine=inst.engine,
                            )
                        )
                        k += 1
                        n += 1
                    si.on_wait = keep
                new.append(inst)
            bb.instructions[:] = new
    return n


def _build(Vmax, T, chunk, gbufs, mode="dmag", legalize=True, ybufs=16, ramp_ind=0, cc="ag", dtype="f32", compute="pe"):
    from concourse import bass, bacc, mybir
    import concourse.tile as tile
    from concourse.tile import add_dep_helper

    F32 = mybir.dt.float32
    GDT = mybir.dt.bfloat16 if dtype == "bf16" else F32
    I16 = mybir.dt.int16
    I32 = mybir.dt.int32

    nc = bacc.Bacc(None, num_devices=NCORES)
    emb = nc.declare_dram_parameter("emb", [Vmax, PDIMP], GDT, isOutput=False)
    # idx16: gather index i of this core lives at [i % 16, i // 16], rows
    # replicated x8 down the partition dim (one copy per Q7 band).
    idx16 = nc.declare_dram_parameter("idx16", [128, T * 8], I16, isOutput=False)
    idx32 = nc.declare_dram_parameter("idx32", [128, T], I32, isOutput=False)
    SELDT = GDT if compute in ("pe", "split") else F32
    sel = nc.declare_dram_parameter("sel", [128, T * B], SELDT, isOutput=False)
    WDT = F32 if compute in ("pe", "split") else GDT
    wrep = nc.declare_dram_parameter("wrep", [128, PDIM], WDT, isOutput=False)
    brep = nc.declare_dram_parameter("brep", [1, 1], F32, isOutput=False)
    outp = nc.declare_dram_parameter("out", [1, B], F32, isOutput=True)

    with tile.TileContext(nc) as tc:
        with (
            tc.tile_pool(name="meta", bufs=1) as meta,
            tc.tile_pool(name="g", bufs=gbufs) as gp,
            tc.tile_pool(name="y", bufs=ybufs) as yp,
            tc.tile_pool(name="ps", bufs=1, space="PSUM") as pp,
            tc.tile_pool(name="dram", bufs=1, space="DRAM") as dp,
        ):
            idx16_sb = meta.tile([128, T * 8], I16)
            nc.sync.dma_start(out=idx16_sb[:], in_=idx16[:])
            idx32_sb = meta.tile([128, T], I32)
            nc.sync.dma_start(out=idx32_sb[:], in_=idx32[:])
            sel_sb = meta.tile([128, T * B], SELDT)
            nc.sync.dma_start(out=sel_sb[:], in_=sel[:])
            w_sb = meta.tile([128, PDIM], WDT)
            nc.sync.dma_start(out=w_sb[:], in_=wrep[:])
            b_sb = meta.tile([1, 1], F32)
            nc.sync.dma_start(out=b_sb[:], in_=brep[:])

            # warmup collective: absorb ncfw rendezvous/setup concurrently
            # with the gather pipeline so the real AllReduce at the end is
            # cheap.
            if os.environ.get("BERT_CCWARM", "1") == "1":
                warm_sb = meta.tile([1, 4], F32)
                nc.vector.memset(warm_sb[:], 0.0)
                ccw_in = dp.tile([1, 4], F32)
                ccw_out = dp.tile([NCORES if cc == "ag" else 1, 4], F32)
                nc.sync.dma_start(out=ccw_in[:], in_=warm_sb[:])
                nc.gpsimd.collective_compute(
                    "AllGather" if cc == "ag" else "AllReduce",
                    mybir.AluOpType.bypass if cc == "ag" else mybir.AluOpType.add,
                    replica_groups=[list(range(NCORES))],
                    ins=[ccw_in[:]],
                    outs=[ccw_out[:]],
                )

            dot_ps = pp.tile([1, B], F32)
            HALF = PDIM // 2
            pool_a = pp.tile([B, HALF], F32, tag="pa")
            pool_b = pp.tile([B, HALF], F32, tag="pb")
            if compute == "pe":
                pe_set = set(range(T))
            elif compute == "split":
                pe_set = set(range(1, T, 2))
            else:
                pe_set = set()
            stt_set = set(range(T)) - pe_set
            pe_lo, pe_hi = (min(pe_set), max(pe_set)) if pe_set else (0, 0)
            st_lo, st_hi = (min(stt_set), max(stt_set)) if stt_set else (0, 0)
            YDT = GDT if compute == "split" else F32
            if compute == "split":
                w16 = meta.tile([128, PDIM], GDT)
                nc.vector.tensor_copy(out=w16[:], in_=w_sb[:])
            else:
                w16 = w_sb

            def consume(gflat, off, t):
                """gflat: [128, >=off+PDIM] gathered rows tile; tile index t."""
                if t in pe_set:
                    # pooled[b,:] += sel_t[:,b]^T @ G ; W applied once at the end
                    nc.tensor.matmul(
                        out=pool_a[:],
                        lhsT=sel_sb[:, t * B : (t + 1) * B],
                        rhs=gflat[:, off : off + HALF],
                        start=(t == pe_lo),
                        stop=(t == pe_hi),
                    )
                    nc.tensor.matmul(
                        out=pool_b[:],
                        lhsT=sel_sb[:, t * B : (t + 1) * B],
                        rhs=gflat[:, off + HALF : off + PDIM],
                        start=(t == pe_lo),
                        stop=(t == pe_hi),
                    )
                    return
                y = yp.tile([128, 1], YDT)
                gs = gflat[:, off : off + PDIM]
                nc.vector.scalar_tensor_tensor(
                    out=gs,
                    in0=gs,
                    scalar=1.0,
                    in1=w16[:],
                    op0=mybir.AluOpType.mult,
                    op1=mybir.AluOpType.mult,
                    accum_out=y[:],
                )
                nc.tensor.matmul(
                    out=dot_ps[:],
                    lhsT=y[:],
                    rhs=sel_sb[:, t * B : (t + 1) * B],
                    start=(t == st_lo),
                    stop=(t == st_hi),
                )

            if mode == "dmag":
                # ramp-in: first tiles as single-row-set indirect gathers (low
                # latency), remainder as bulk dma_gather chunks (low overhead)
                nramp = min(ramp_ind, T)
                ramp_insts = []
                for t in range(nramp):
                    gi = gp.tile([128, PDIMP], GDT, tag="gi")
                    gi_inst = nc.gpsimd.indirect_dma_start(
                        out=gi[:],
                        out_offset=None,
                        in_=emb[:],
                        in_offset=bass.IndirectOffsetOnAxis(
                            ap=idx32_sb[:, t : t + 1], axis=0
                        ),
                    )
                    # keep the low-latency ramp singles in issue order
                    if ramp_insts:
                        add_dep_helper(gi_inst.ins, ramp_insts[-1].ins, reason="ramp order")
                    ramp_insts.append(gi_inst)
                    consume(gi[:], 0, t)
                sched = []
                rem = T - nramp
                while rem > 0:
                    c = min(chunk, rem)
                    sched.append(c)
                    rem -= c
                s = nramp
                first_dmag = True
                for c in sched:
                    g = gp.tile([128, c, PDIMP], GDT, tag="g")
                    dg_inst = nc.gpsimd.dma_gather(
                        out_ap=g[:],
                        in_ap=emb[:],
                        idxs_ap=idx16_sb[:, s * 8 : (s + c) * 8],
                        num_idxs=c * 128,
                        num_idxs_reg=c * 128,
                        elem_size=PDIMP,
                    )
                    if first_dmag and ramp_insts:
                        add_dep_helper(dg_inst.ins, ramp_insts[-1].ins, reason="ramp first")
                        first_dmag = False
                    gflat = g[:].rearrange("p c e -> p (c e)")
                    for j in range(c):
                        consume(gflat, j * PDIMP, s + j)
                    s += c
            else:  # indirect: one [128, PDIMP] row-gather per tile
                for t in range(T):
                    g = gp.tile([128, PDIMP], F32, tag="g")
                    nc.gpsimd.indirect_dma_start(
                        out=g[:],
                        out_offset=None,
                        in_=emb[:],
                        in_offset=bass.IndirectOffsetOnAxis(
                            ap=idx32_sb[:, t : t + 1], axis=0
                        ),
                    )
                    consume(g[:], 0, t)

            if compute in ("pe", "split"):
                pooled_sb = meta.tile([B, PDIM], F32)
                nc.vector.tensor_copy(out=pooled_sb[:, :HALF], in_=pool_a[:])
                nc.vector.tensor_copy(out=pooled_sb[:, HALF:], in_=pool_b[:])
                scr = meta.tile([B, PDIM], F32)
                y64 = meta.tile([B, 1], F32)
                nc.vector.scalar_tensor_tensor(
                    out=scr[:],
                    in0=pooled_sb[:],
                    scalar=1.0,
                    in1=w_sb[:B, :],
                    op0=mybir.AluOpType.mult,
                    op1=mybir.AluOpType.mult,
                    accum_out=y64[:],
                )
                if compute == "split":
                    # fold the stt-half partial [1,B] into partition-major form
                    part1_sb = meta.tile([1, B], F32)
                    nc.vector.tensor_copy(out=part1_sb[:], in_=dot_ps[:])
                    ident1 = meta.tile([1, 1], F32)
                    nc.vector.memset(ident1[:], 1.0)
                    dot_t = pp.tile([B, 1], F32, tag="dt")
                    nc.tensor.transpose(out=dot_t[:], in_=part1_sb[:], identity=ident1[:])
                    both = meta.tile([B, 1], F32)
                    nc.vector.tensor_tensor(
                        out=both[:], in0=y64[:], in1=dot_t[:], op=mybir.AluOpType.add
                    )
                    part_sb = both
                else:
                    part_sb = y64
                cc_in = dp.tile([B, 1], F32)
            else:
                part_sb = meta.tile([1, B], F32)
                nc.vector.tensor_copy(out=part_sb[:], in_=dot_ps[:])
                cc_in = dp.tile([1, B], F32)
            nc.sync.dma_start(out=cc_in[:], in_=part_sb[:])
            pmajor = compute in ("pe", "split")
            if cc == "ag":
                cc_out = dp.tile([NCORES * B, 1] if pmajor else [NCORES, B], F32)
                nc.gpsimd.collective_compute(
                    "AllGather",
                    mybir.AluOpType.bypass,
                    replica_groups=[list(range(NCORES))],
                    ins=[cc_in[:]],
                    outs=[cc_out[:]],
                )
                allg_sb = meta.tile([NCORES, B], F32)
                nc.sync.dma_start(out=allg_sb[:], in_=cc_out[:].rearrange("a b -> (a b)").rearrange("(c n) -> c n", c=NCORES) if pmajor else cc_out[:])
                ones_sb = meta.tile([NCORES, 1], F32)
                nc.vector.memset(ones_sb[:], 1.0)
                sum_ps = pp.tile([1, B], F32, tag="sum")
                nc.tensor.matmul(
                    out=sum_ps[:],
                    lhsT=ones_sb[:],
                    rhs=allg_sb[:],
                    start=True,
                    stop=True,
                )
                red_ap = sum_ps[:]
            else:
                cc_out = dp.tile([1, B], F32)
                nc.gpsimd.collective_compute(
                    "AllReduce",
                    mybir.AluOpType.add,
                    replica_groups=[list(range(NCORES))],
                    ins=[cc_in[:]],
                    outs=[cc_out[:]],
                )
                red_sb = meta.tile([1, B], F32)
                nc.sync.dma_start(out=red_sb[:], in_=cc_out[:])
                red_ap = red_sb[:]
            o_sb = meta.tile([1, B], F32)
            nc.scalar.activation(
                out=o_sb[:],
                in_=red_ap,
                func=mybir.ActivationFunctionType.Sigmoid,
                bias=b_sb[:],
                scale=1.0 / float(L),
            )
            nc.sync.dma_start(out=outp[:], in_=o_sb[:])

    nc.compile()
    if legalize:
        _legalize_sem_waits(nc, mybir)
    return nc


def _marshal(tokens, lengths, emb_table, W, b, dtype="f32"):
    if dtype == "bf16":
        import ml_dtypes

        sdt = ml_dtypes.bfloat16
    else:
        sdt = np.float32
    tokens = np.asarray(tokens)
    lengths = np.asarray(lengths).astype(np.int64)
    emb_table = np.ascontiguousarray(emb_table, dtype=np.float32)

    mask = np.arange(L)[None, :] < lengths[:, None]
    flat_tok = tokens[mask].astype(np.int64)
    flat_b = np.broadcast_to(np.arange(B)[:, None], (B, L))[mask]
    uniq, inv = np.unique(flat_tok, return_inverse=True)
    U = len(uniq)
    cnt = np.zeros((U, B), dtype=np.float32)
    np.add.at(cnt, (inv, flat_b), 1.0)

    bounds = [U * c // NCORES for c in range(NCORES + 1)]
    rows_max = max(bounds[c + 1] - bounds[c] for c in range(NCORES))
    T = -(-rows_max // 128)

    spans = []
    for c in range(NCORES):
        s, e = bounds[c], bounds[c + 1]
        lo = int(uniq[s]) if e > s else 0
        hi = int(uniq[e - 1]) + 1 if e > s else 1
        spans.append((s, e, lo, hi))
    Vmax = max(hi - lo for _, _, lo, hi in spans)

    wdt = np.float32 if os.environ.get("BERT_COMPUTE", "stt") in ("pe", "split") else sdt
    wrep = np.broadcast_to(
        np.asarray(W, dtype=np.float32).astype(wdt).reshape(1, PDIM), (128, PDIM)
    ).copy()
    brep = np.full((1, 1), np.float32(np.asarray(b).reshape(-1)[0]), dtype=np.float32)

    in_maps = []
    for c in range(NCORES):
        s, e, lo, hi = spans[c]
        span = hi - lo
        emb_c = np.zeros((Vmax, PDIMP), dtype=sdt)
        emb_c[:span, :PDIM] = emb_table[lo:hi].astype(sdt)
        rows = np.zeros(T * 128, dtype=np.int32)
        rows[: e - s] = (uniq[s:e] - lo).astype(np.int32)
        # int16 wrapped layout: index i -> [i % 16, i // 16], replicated x8
        wrapped = rows.astype(np.int16).reshape(T * 8, 16).T  # [16, T*8]
        idx16 = np.tile(wrapped, (8, 1)).copy()  # [128, T*8]
        seldt = sdt if os.environ.get("BERT_COMPUTE", "stt") in ("pe", "split") else np.float32
        selm = np.zeros((T * 128, B), dtype=seldt)
        selm[: e - s] = cnt[s:e].astype(seldt)
        in_maps.append(
            {
                "emb": emb_c,
                "idx16": idx16,
                "idx32": rows.reshape(T, 128).T.copy(),
                "sel": selm.reshape(T, 128, B).transpose(1, 0, 2).reshape(128, T * B).copy(),
                "wrep": wrep,
                "brep": brep,
            }
        )
    return T, Vmax, in_maps


def kernel(tokens, lengths, emb_table, W, b):
    from concourse.bass_utils import run_bass_kernel_spmd

    mode = os.environ.get("BERT_MODE", "dmag")
    chunk = int(os.environ.get("BERT_CHUNK", "8"))
    gbufs = int(os.environ.get("BERT_GBUFS", "4"))
    ybufs = int(os.environ.get("BERT_YBUFS", "16"))
    ramp_ind = int(os.environ.get("BERT_RAMPIND", "0"))
    cc = os.environ.get("BERT_CC", "ag")
    compute = os.environ.get("BERT_COMPUTE", "stt")
    trace = os.environ.get("BERT_TRACE", "0") == "1"

    dtype = os.environ.get("BERT_DTYPE", "bf16")
    T, Vmax, in_maps = _marshal(tokens, lengths, emb_table, W, b, dtype=dtype)
    nc = _build(Vmax, T, chunk, gbufs, mode=mode, ybufs=ybufs, ramp_ind=ramp_ind, cc=cc, dtype=dtype, compute=compute)
    res = run_bass_kernel_spmd(nc, in_maps, core_ids=list(range(NCORES)), trace=trace)
    LAST["results"] = res
    LAST["T"] = T
    LAST["Vmax"] = Vmax
    return res.results[0]["out"].reshape(B).astype(np.float32)

# ---------------------------------------------------------------------------
# Sequence-ownership variant: each core owns 8 sequences end-to-end (no
# collective, no cross-core skew sensitivity). Table replicated in bf16;
# gathers windowed into 32768-row vocab windows so rebased indices fit int16.
WIN = 32768
NW = -(-VOCAB // WIN)
NSEQ = B // NCORES
NSEL = 16  # sel columns per tile (NSEQ real + zero pad; DoubleRow wants %16)


def _marshal_seq(tokens, lengths, emb_table, W, b, dtype="bf16"):
    import ml_dtypes

    sdt = ml_dtypes.bfloat16 if dtype == "bf16" else np.float32
    tokens = np.asarray(tokens)
    lengths = np.asarray(lengths).astype(np.int64)

    # per-sequence unique-token histograms over vocab windows; greedy
    # vector-balancing assignment minimizes sum_w max_c rows (the padded
    # tile count is driven by per-window maxima, not total length)
    order = np.argsort(-lengths, kind="stable")
    hists = np.zeros((B, NW), dtype=np.int64)
    for bidx in range(B):
        u = np.unique(tokens[bidx, : lengths[bidx]].astype(np.int64))
        hists[bidx] = np.bincount(u // WIN, minlength=NW)
    Wc = np.zeros((NCORES, NW), dtype=np.int64)
    counts = np.zeros(NCORES, dtype=np.int64)
    assign = np.full((NCORES, NSEQ), -1, dtype=np.int64)
    for bidx in order:
        cands = np.where(counts < NSEQ)[0]
        best, bobj = None, None
        for c in cands:
            trial = Wc.copy()
            trial[c] += hists[bidx]
            obj = trial.max(axis=0).sum()
            if bobj is None or obj < bobj:
                best, bobj = c, obj
        assign[best, counts[best]] = bidx
        counts[best] += 1
        Wc[best] += hists[bidx]

    def _obj(Wm):
        return (-(-Wm.max(axis=0) // 128)).sum() * 1000000 + Wm.max(axis=0).sum()

    # swap refinement: directly minimize padded tile count sum_w ceil(max/128)
    for _ in range(40):
        improved = False
        cur = _obj(Wc)
        for c1 in range(NCORES):
            for j1 in range(NSEQ):
                for c2 in range(c1 + 1, NCORES):
                    for j2 in range(NSEQ):
                        b1, b2 = assign[c1, j1], assign[c2, j2]
                        trial = Wc.copy()
                        trial[c1] += hists[b2] - hists[b1]
                        trial[c2] += hists[b1] - hists[b2]
                        if _obj(trial) < cur:
                            assign[c1, j1], assign[c2, j2] = b2, b1
                            Wc = trial
                            cur = _obj(Wc)
                            improved = True
        if not improved:
            break

    per_core_rows = []  # (uniq, cnt8) per core
    for c in range(NCORES):
        toks = np.concatenate(
            [tokens[assign[c, j], : lengths[assign[c, j]]] for j in range(NSEQ)]
        ).astype(np.int64)
        locb = np.concatenate(
            [np.full(int(lengths[assign[c, j]]), j, dtype=np.int64) for j in range(NSEQ)]
        )
        uniq, inv = np.unique(toks, return_inverse=True)
        cnt8 = np.zeros((len(uniq), NSEQ), dtype=np.float32)
        np.add.at(cnt8, (inv, locb), 1.0)
        per_core_rows.append((uniq, cnt8))

    # per-window tile counts, common across cores (SPMD: same program)
    Tw = []
    bnds = []
    for w in range(NW):
        lo, hi = w * WIN, min((w + 1) * WIN, VOCAB)
        per_core_bnd = [
            (np.searchsorted(u, lo), np.searchsorted(u, hi)) for u, _ in per_core_rows
        ]
        bnds.append(per_core_bnd)
        Tw.append(max(-(-int(e - s) // 128) for s, e in per_core_bnd))
    T = sum(Tw)

    emb16 = np.zeros((VOCAB, PDIMP), dtype=sdt)
    emb16[:, :PDIM] = np.ascontiguousarray(emb_table, dtype=np.float32).astype(sdt)
    wdt = np.float32 if os.environ.get("BERT_SEQSPLIT", "1") == "1" else sdt
    wrep = np.broadcast_to(
        np.asarray(W, dtype=np.float32).astype(wdt).reshape(1, PDIM), (128, PDIM)
    ).copy()
    brep = np.full((NSEQ, 1), np.float32(np.asarray(b).reshape(-1)[0]), dtype=np.float32)

    in_maps = []
    for c in range(NCORES):
        uniq, cnt8 = per_core_rows[c]
        rows = np.zeros(T * 128, dtype=np.int16)
        selm = np.zeros((T * 128, NSEQ), dtype=np.float32)
        t0 = 0
        for w in range(NW):
            s0, e0 = bnds[w][c]
            n = int(e0 - s0)
            rows[t0 * 128 : t0 * 128 + n] = (uniq[s0:e0] - w * WIN).astype(np.int16)
            selm[t0 * 128 : t0 * 128 + n] = cnt8[s0:e0]
            t0 += Tw[w]
        if os.environ.get("BERT_SEQSPLIT", "1") == "1":
            selm = selm.astype(sdt)
        wrapped = rows.reshape(T * 8, 16).T  # [16, T*8]
        in_maps.append(
            {
                "emb": emb16,
                "idx16": np.tile(wrapped, (8, 1)).copy(),
                "sel": selm.reshape(T, 128, NSEQ)
                .transpose(1, 0, 2)
                .reshape(128, T * NSEQ)
                .copy(),
                "wrep": wrep,
                "brep": brep,
            }
        )
    return Tw, in_maps, assign


def _build_seq(Tw, chunk, gbufs, ybufs, dtype="bf16", legalize=True, split=True):
    from concourse import bacc, mybir
    import concourse.tile as tile

    F32 = mybir.dt.float32
    GDT = mybir.dt.bfloat16 if dtype == "bf16" else F32
    I16 = mybir.dt.int16
    T = sum(Tw)

    scratch = int(os.environ.get("BERT_DMASCRATCH", "131072"))
    nc = bacc.Bacc(None, num_devices=NCORES, dynamic_dma_scratch_size=scratch)
    emb = nc.declare_dram_parameter("emb", [VOCAB, PDIMP], GDT, isOutput=False)
    idx16 = nc.declare_dram_parameter("idx16", [128, T * 8], I16, isOutput=False)
    SELDT = GDT if split else F32
    sel = nc.declare_dram_parameter("sel", [128, T * NSEQ], SELDT, isOutput=False)
    WDT = F32 if split else GDT
    wrep = nc.declare_dram_parameter("wrep", [128, PDIM], WDT, isOutput=False)
    brep = nc.declare_dram_parameter("brep", [NSEQ, 1], F32, isOutput=False)
    outp = nc.declare_dram_parameter("out", [1, NSEQ], F32, isOutput=True)

    with tile.TileContext(nc) as tc:
        with (
            tc.tile_pool(name="meta", bufs=1) as meta,
            tc.tile_pool(name="g", bufs=gbufs) as gp,
            tc.tile_pool(name="y", bufs=ybufs) as yp,
            tc.tile_pool(name="ps", bufs=1, space="PSUM") as pp,
        ):
            idx16_sb = meta.tile([128, T * 8], I16)
            nc.sync.dma_start(out=idx16_sb[:], in_=idx16[:])
            sel_sb = meta.tile([128, T * NSEQ], SELDT)
            nc.sync.dma_start(out=sel_sb[:], in_=sel[:])
            w_sb = meta.tile([128, PDIM], WDT)
            nc.sync.dma_start(out=w_sb[:], in_=wrep[:])
            b_sb = meta.tile([NSEQ, 1], F32)
            nc.sync.dma_start(out=b_sb[:], in_=brep[:])

            dot_ps = pp.tile([1, NSEQ], F32)
            first_chunk = True
            HALF = PDIM // 2
            if split:
                # DVE handles even tiles (row.W dot), PE handles odd tiles
                # (pooled accumulation); W applied to the pooled half once.
                pe_set = set(range(1, T, 2))
                dot8 = pp.tile([NSEQ, 1], F32, tag="d8")
                pool_a = pp.tile([NSEQ, HALF], F32, tag="pa")
                pool_b = pp.tile([NSEQ, HALF], F32, tag="pb")
                w16 = meta.tile([128, PDIM], GDT)
                nc.vector.tensor_copy(out=w16[:], in_=w_sb[:])
            else:
                pe_set = set()
                w16 = w_sb
            stt_set = set(range(T)) - pe_set
            pe_lo, pe_hi = (min(pe_set), max(pe_set)) if pe_set else (0, 0)
            st_lo, st_hi = (min(stt_set), max(stt_set)) if stt_set else (0, 0)
            t = 0
            for w in range(NW):
                wlo = w * WIN
                whi = min(wlo + WIN, VOCAB)
                left = Tw[w]
                while left > 0:
                    # small first chunk: first gathered data lands sooner,
                    # cutting pipeline ramp-in before the consumers start
                    c = min(4 if first_chunk else chunk, left)
                    first_chunk = False
                    g = gp.tile([128, c, PDIMP], GDT, tag="g")
                    nc.gpsimd.dma_gather(
                        out_ap=g[:],
                        in_ap=emb[wlo:whi],
                        idxs_ap=idx16_sb[:, t * 8 : (t + c) * 8],
                        num_idxs=c * 128,
                        num_idxs_reg=c * 128,
                        elem_size=PDIMP,
                    )
                    gflat = g[:].rearrange("p c e -> p (c e)")
                    for j in range(c):
                        tt = t + j
                        off = j * PDIMP
                        if tt in pe_set:
                            nc.tensor.matmul(
                                out=pool_a[:],
                                lhsT=sel_sb[:, tt * NSEQ : (tt + 1) * NSEQ],
                                rhs=gflat[:, off : off + HALF],
                                start=(tt == pe_lo),
                                stop=(tt == pe_hi),
                            )
                            nc.tensor.matmul(
                                out=pool_b[:],
                                lhsT=sel_sb[:, tt * NSEQ : (tt + 1) * NSEQ],
                                rhs=gflat[:, off + HALF : off + PDIM],
                                start=(tt == pe_lo),
                                stop=(tt == pe_hi),
                            )
                            continue
                        y = yp.tile([128, 1], GDT if split else F32)
                        gs = gflat[:, off : off + PDIM]
                        nc.vector.scalar_tensor_tensor(
                            out=gs,
                            in0=gs,
                            scalar=1.0,
                            in1=w16[:],
                            op0=mybir.AluOpType.mult,
                            op1=mybir.AluOpType.mult,
                            accum_out=y[:],
                        )
                        if split:
                            nc.tensor.matmul(
                                out=dot8[:],
                                lhsT=sel_sb[:, tt * NSEQ : (tt + 1) * NSEQ],
                                rhs=y[:],
                                start=(tt == st_lo),
                                stop=(tt == st_hi),
                            )
                        else:
                            nc.tensor.matmul(
                                out=dot_ps[:],
                                lhsT=y[:],
                                rhs=sel_sb[:, tt * NSEQ : (tt + 1) * NSEQ],
                                start=(tt == st_lo),
                                stop=(tt == st_hi),
                            )
                    t += c
                    left -= c

            if split:
                pooled_sb = meta.tile([NSEQ, PDIM], F32)
                nc.vector.tensor_copy(out=pooled_sb[:, :HALF], in_=pool_a[:])
                nc.vector.tensor_copy(out=pooled_sb[:, HALF:], in_=pool_b[:])
                scr = meta.tile([NSEQ, PDIM], F32)
                y8 = meta.tile([NSEQ, 1], F32)
                nc.vector.scalar_tensor_tensor(
                    out=scr[:],
                    in0=pooled_sb[:],
                    scalar=1.0,
                    in1=w_sb[:NSEQ, :],
                    op0=mybir.AluOpType.mult,
                    op1=mybir.AluOpType.mult,
                    accum_out=y8[:],
                )
                part = meta.tile([NSEQ, 1], F32)
                nc.vector.tensor_tensor(
                    out=part[:], in0=dot8[:], in1=y8[:], op=mybir.AluOpType.add
                )
                o_sb = meta.tile([NSEQ, 1], F32)
                nc.scalar.activation(
                    out=o_sb[:],
                    in_=part[:],
                    func=mybir.ActivationFunctionType.Sigmoid,
                    bias=b_sb[:],
                    scale=1.0 / float(L),
                )
                nc.sync.dma_start(out=outp[0, :, None], in_=o_sb[:])
            else:
                o_sb = meta.tile([1, NSEQ], F32)
                nc.scalar.activation(
                    out=o_sb[:],
                    in_=dot_ps[:],
                    func=mybir.ActivationFunctionType.Sigmoid,
                    bias=b_sb[:1, :],
                    scale=1.0 / float(L),
                )
                nc.sync.dma_start(out=outp[:], in_=o_sb[:])

    nc.compile()
    if legalize:
        _legalize_sem_waits(nc, __import__("concourse.mybir", fromlist=["x"]))
    return nc


def _kernel_seq(tokens, lengths, emb_table, W, b):
    from concourse.bass_utils import run_bass_kernel_spmd

    dtype = os.environ.get("BERT_DTYPE", "bf16")
    chunk = int(os.environ.get("BERT_CHUNK", "8"))
    gbufs = int(os.environ.get("BERT_GBUFS", "4"))
    ybufs = int(os.environ.get("BERT_YBUFS", "16"))
    trace = os.environ.get("BERT_TRACE", "0") == "1"

    split = os.environ.get("BERT_SEQSPLIT", "1") == "1"
    Tw, in_maps, assign = _marshal_seq(tokens, lengths, emb_table, W, b, dtype=dtype)
    nc = _build_seq(Tw, chunk, gbufs, ybufs, dtype=dtype, split=split)
    res = run_bass_kernel_spmd(nc, in_maps, core_ids=list(range(NCORES)), trace=trace)
    LAST["results"] = res
    LAST["T"] = sum(Tw)
    LAST["Vmax"] = VOCAB
    out = np.zeros(B, dtype=np.float32)
    for c in range(NCORES):
        vals = res.results[c]["out"].reshape(-1)
        for j in range(NSEQ):
            out[assign[c, j]] = vals[j]
    return out


_kernel_vocab = kernel


# ---------------------------------------------------------------------------
# fp8 all-PE variant (BERT_SHARD=pe8, default): sequence-ownership sharding as
# above, but the table is cast to fp8e4 (halves gather DMA traffic; final
# sigmoid output error ~1e-4 << 2e-2 budget) and ALL per-tile compute runs on
# the PE: pooled[seq,:] += sel_t^T @ g_t accumulated in two PSUM banks across
# every tile. This removes the DVE<->PE zigzag (STT -> dot8 -> pool-MM) that
# paced the old pipeline at ~10.3us per 8-tile chunk with no engine saturated.
# The W dot + sigmoid happen once on [8,1000] at the end.
def _marshal_pe8(tokens, lengths, emb_table, W, b):
    import ml_dtypes

    f8 = ml_dtypes.float8_e4m3
    tokens = np.asarray(tokens)
    lengths = np.asarray(lengths).astype(np.int64)

    order = np.argsort(-lengths, kind="stable")
    hists = np.zeros((B, NW), dtype=np.int64)
    for bidx in range(B):
        u = np.unique(tokens[bidx, : lengths[bidx]].astype(np.int64))
        hists[bidx] = np.bincount(u // WIN, minlength=NW)
    Wc = np.zeros((NCORES, NW), dtype=np.int64)
    counts = np.zeros(NCORES, dtype=np.int64)
    assign = np.full((NCORES, NSEQ), -1, dtype=np.int64)
    for bidx in order:
        cands = np.where(counts < NSEQ)[0]
        best, bobj = None, None
        for c in cands:
            trial = Wc.copy()
            trial[c] += hists[bidx]
            obj = trial.max(axis=0).sum()
            if bobj is None or obj < bobj:
                best, bobj = c, obj
        assign[best, counts[best]] = bidx
        counts[best] += 1
        Wc[best] += hists[bidx]

    def _obj(Wm):
        return (-(-Wm.max(axis=0) // 128)).sum() * 1000000 + Wm.max(axis=0).sum()

    for _ in range(40):
        improved = False
        cur = _obj(Wc)
        for c1 in range(NCORES):
            for j1 in range(NSEQ):
                for c2 in range(c1 + 1, NCORES):
                    for j2 in range(NSEQ):
                        b1, b2 = assign[c1, j1], assign[c2, j2]
                        trial = Wc.copy()
                        trial[c1] += hists[b2] - hists[b1]
                        trial[c2] += hists[b1] - hists[b2]
                        if _obj(trial) < cur:
                            assign[c1, j1], assign[c2, j2] = b2, b1
                            Wc = trial
                            cur = _obj(Wc)
                            improved = True
        if not improved:
            break

    per_core_rows = []
    for c in range(NCORES):
        toks = np.concatenate(
            [tokens[assign[c, j], : lengths[assign[c, j]]] for j in range(NSEQ)]
        ).astype(np.int64)
        locb = np.concatenate(
            [np.full(int(lengths[assign[c, j]]), j, dtype=np.int64) for j in range(NSEQ)]
        )
        uniq, inv = np.unique(toks, return_inverse=True)
        cnt8 = np.zeros((len(uniq), NSEQ), dtype=np.float32)
        np.add.at(cnt8, (inv, locb), 1.0)
        per_core_rows.append((uniq, cnt8))

    Tw = []
    bnds = []
    for w in range(NW):
        lo, hi = w * WIN, min((w + 1) * WIN, VOCAB)
        per_core_bnd = [
            (np.searchsorted(u, lo), np.searchsorted(u, hi)) for u, _ in per_core_rows
        ]
        bnds.append(per_core_bnd)
        Tw.append(max(-(-int(e - s) // 128) for s, e in per_core_bnd))
    T = sum(Tw)

    emb8 = np.zeros((VOCAB, PDIMP), dtype=f8)
    emb8[:, :PDIM] = np.ascontiguousarray(emb_table, dtype=np.float32).astype(f8)
    w8 = np.ascontiguousarray(
        np.broadcast_to(np.asarray(W, dtype=np.float32).reshape(1, PDIM), (NSEQ, PDIM))
    )
    brep = np.full((NSEQ, 1), np.float32(np.asarray(b).reshape(-1)[0]), dtype=np.float32)

    gmode = os.environ.get("BERT_GMODE", "dmag")
    if gmode == "ind":
        # int32 full-vocab row indices -> no 32768-row windows, no window
        # padding; T is just the cross-core max tile count.
        T = max(-(-len(u) // 128) for u, _ in per_core_rows)
        in_maps = []
        for c in range(NCORES):
            uniq, cnt8 = per_core_rows[c]
            n = len(uniq)
            rows = np.zeros(T * 128, dtype=np.int32)
            rows[:n] = uniq.astype(np.int32)
            selm = np.zeros((T * 128, NSEL), dtype=np.float32)
            selm[:n, :NSEQ] = cnt8
            in_maps.append(
                {
                    "emb": emb8,
                    "idx32": rows.reshape(T, 128).T.copy(),
                    "sel": selm.astype(f8)
                    .reshape(T, 128, NSEL)
                    .transpose(1, 0, 2)
                    .reshape(128, T * NSEL)
                    .copy(),
                    "wrep": w8,
                    "brep": brep,
                }
            )
        return [T], in_maps, assign

    in_maps = []
    for c in range(NCORES):
        uniq, cnt8 = per_core_rows[c]
        rows = np.zeros(T * 128, dtype=np.int16)
        selm = np.zeros((T * 128, NSEQ), dtype=np.float32)
        t0 = 0
        for w in range(NW):
            s0, e0 = bnds[w][c]
            n = int(e0 - s0)
            rows[t0 * 128 : t0 * 128 + n] = (uniq[s0:e0] - w * WIN).astype(np.int16)
            selm[t0 * 128 : t0 * 128 + n] = cnt8[s0:e0]
            t0 += Tw[w]
        selp = np.zeros((T * 128, NSEL), dtype=np.float32)
        selp[:, :NSEQ] = selm
        wrapped = rows.reshape(T * 8, 16).T  # [16, T*8]
        in_maps.append(
            {
                "emb": emb8,
                "idx16": np.tile(wrapped, (8, 1)).copy(),
                "sel": selp.astype(f8)
                .reshape(T, 128, NSEL)
                .transpose(1, 0, 2)
                .reshape(128, T * NSEL)
                .copy(),
                "wrep": w8,
                "brep": brep,
            }
        )
    return Tw, in_maps, assign


def _build_pe8(Tw, chunk, gbufs, legalize=True, gmode="dmag", nq=1):
    from concourse import bass, bacc, mybir
    import concourse.tile as tile

    F32 = mybir.dt.float32
    F8 = mybir.dt.float8e4
    I16 = mybir.dt.int16
    I32 = mybir.dt.int32
    T = sum(Tw)

    nc = bacc.Bacc(None, num_devices=NCORES, num_swdge_queues=nq)
    emb = nc.declare_dram_parameter("emb", [VOCAB, PDIMP], F8, isOutput=False)
    if gmode == "ind":
        idx32 = nc.declare_dram_parameter("idx32", [128, T], I32, isOutput=False)
    else:
        idx16 = nc.declare_dram_parameter("idx16", [128, T * 8], I16, isOutput=False)
    sel = nc.declare_dram_parameter("sel", [128, T * NSEL], F8, isOutput=False)
    wrep = nc.declare_dram_parameter("wrep", [NSEQ, PDIM], F32, isOutput=False)
    brep = nc.declare_dram_parameter("brep", [NSEQ, 1], F32, isOutput=False)
    outp = nc.declare_dram_parameter("out", [1, NSEQ], F32, isOutput=True)

    HALF = PDIM // 2
    with tile.TileContext(nc) as tc:
        with (
            tc.tile_pool(name="meta", bufs=1) as meta,
            tc.tile_pool(name="g", bufs=gbufs) as gp,
            tc.tile_pool(name="ps", bufs=1, space="PSUM") as pp,
        ):
            if gmode == "ind":
                idx32_sb = meta.tile([128, T], I32)
                nc.sync.dma_start(out=idx32_sb[:], in_=idx32[:])
            else:
                idx16_sb = meta.tile([128, T * 8], I16)
                nc.sync.dma_start(out=idx16_sb[:], in_=idx16[:])
            if os.environ.get("BERT_GWARM", "1") == "1":
                # throwaway gather (all-zero on-chip indices, no DMA dep):
                # absorbs the one-time first-DMAGatherAnt stall while the real
                # index DMA is still landing.
                idxw = meta.tile([128, 8], I16)
                nc.vector.memset(idxw[:], 0)
                gw = meta.tile([128, 1, PDIMP], F8)
                nc.gpsimd.dma_gather(
                    out_ap=gw[:],
                    in_ap=emb[0:1],
                    idxs_ap=idxw[:],
                    num_idxs=128,
                    num_idxs_reg=128,
                    elem_size=PDIMP,
                )
            sel_sb = meta.tile([128, T * NSEL], F8)
            nc.sync.dma_start(out=sel_sb[:], in_=sel[:])
            w_sb = meta.tile([NSEQ, PDIM], F32)
            nc.sync.dma_start(out=w_sb[:], in_=wrep[:])
            b_sb = meta.tile([NSEQ, 1], F32)
            nc.sync.dma_start(out=b_sb[:], in_=brep[:])

            DRH = PDIMP // 2  # 512-col halves: DoubleRow needs inner %16==0
            pool_a = pp.tile([NSEL, DRH], F32, tag="pa")
            pool_b = pp.tile([NSEL, DRH], F32, tag="pb")
            use_dr = os.environ.get("BERT_DR", "1") == "1"

            def consume(gflat, off, tt):
                nc.tensor.matmul(
                    out=pool_a[:],
                    lhsT=sel_sb[:, tt * NSEL : tt * NSEL + NSEL],
                    rhs=gflat[:, off : off + DRH],
                    start=(tt == 0),
                    stop=(tt == T - 1),
                )
                nc.tensor.matmul(
                    out=pool_b[:],
                    lhsT=sel_sb[:, tt * NSEL : tt * NSEL + NSEL],
                    rhs=gflat[:, off + DRH : off + PDIMP],
                    start=(tt == 0),
                    stop=(tt == T - 1),
                )

            def consume_pair(g3, j, tt):
                """DoubleRow: tiles tt, tt+1 (chunk slots j, j+1) in one pass.

                g3 is the [128, c, PDIMP] chunk tile; rhs [128, 2, 512] pairs
                the two tiles' half-columns as the two k-tiles (512 not 500:
                DoubleRow needs inner size % 16 == 0; cols 1000-1023 are zero
                pad in the table so they add nothing), lhsT [128, 2, NSEQ] is
                their adjacent sel slices.
                """
                sel2 = sel_sb[:, tt * NSEL : (tt + 2) * NSEL].rearrange(
                    "p (two f) -> p two f", two=2
                )
                nc.tensor.matmul(
                    out=pool_a[:],
                    lhsT=sel2,
                    rhs=g3[:, j : j + 2, 0:DRH],
                    start=(tt == 0),
                    stop=(tt + 1 == T - 1),
                    perf_mode=mybir.MatmulPerfMode.DoubleRow,
                )
                nc.tensor.matmul(
                    out=pool_b[:],
                    lhsT=sel2,
                    rhs=g3[:, j : j + 2, DRH:PDIMP],
                    start=(tt == 0),
                    stop=(tt + 1 == T - 1),
                    perf_mode=mybir.MatmulPerfMode.DoubleRow,
                )

            if gmode == "ind":
                for t in range(T):
                    g = gp.tile([128, PDIMP], F8, tag="g")
                    nc.gpsimd.indirect_dma_start(
                        out=g[:],
                        out_offset=None,
                        in_=emb[:],
                        in_offset=bass.IndirectOffsetOnAxis(
                            ap=idx32_sb[:, t : t + 1], axis=0
                        ),
                    )
                    consume(g[:], 0, t)
            else:
                t = 0
                gi = 0
                first_chunk = True
                for w in range(NW):
                    wlo = w * WIN
                    whi = min(wlo + WIN, VOCAB)
                    left = Tw[w]
                    while left > 0:
                        c = min(
                            int(os.environ.get("BERT_FC", "4")) if first_chunk else chunk,
                            left,
                        )
                        first_chunk = False
                        g = gp.tile([128, c, PDIMP], F8, tag="g")
                        nc.gpsimd.dma_gather(
                            out_ap=g[:],
                            in_ap=emb[wlo:whi],
                            idxs_ap=idx16_sb[:, t * 8 : (t + c) * 8],
                            num_idxs=c * 128,
                            num_idxs_reg=c * 128,
                            elem_size=PDIMP,
                            queue_num=gi % nq,
                        )
                        gi += 1
                        gflat = g[:].rearrange("p c e -> p (c e)")
                        j = 0
                        while j < c:
                            if use_dr and j + 1 < c:
                                consume_pair(g[:], j, t + j)
                                j += 2
                            else:
                                consume(gflat, j * PDIMP, t + j)
                                j += 1
                        t += c
                        left -= c

            pooled_sb = meta.tile([NSEQ, PDIMP], F32)
            nc.vector.tensor_copy(out=pooled_sb[:, :DRH], in_=pool_a[:NSEQ, :])
            nc.vector.tensor_copy(out=pooled_sb[:, DRH:], in_=pool_b[:NSEQ, :])
            scr = meta.tile([NSEQ, PDIM], F32)
            y8 = meta.tile([NSEQ, 1], F32)
            nc.vector.scalar_tensor_tensor(
                out=scr[:],
                in0=pooled_sb[:, :PDIM],
                scalar=1.0,
                in1=w_sb[:],
                op0=mybir.AluOpType.mult,
                op1=mybir.AluOpType.mult,
                accum_out=y8[:],
            )
            o_sb = meta.tile([NSEQ, 1], F32)
            nc.scalar.activation(
                out=o_sb[:],
                in_=y8[:],
                func=mybir.ActivationFunctionType.Sigmoid,
                bias=b_sb[:],
                scale=1.0 / float(L),
            )
            nc.sync.dma_start(out=outp[0, :, None], in_=o_sb[:])

    nc.compile()
    if legalize:
        _legalize_sem_waits(nc, __import__("concourse.mybir", fromlist=["x"]))
    return nc


def _kernel_pe8(tokens, lengths, emb_table, W, b):
    from concourse.bass_utils import run_bass_kernel_spmd

    chunk = int(os.environ.get("BERT_CHUNK", "8"))
    gbufs = int(os.environ.get("BERT_GBUFS", "4"))
    gmode = os.environ.get("BERT_GMODE", "dmag")
    nq = int(os.environ.get("BERT_NQ", "4"))
    trace = os.environ.get("BERT_TRACE", "0") == "1"

    Tw, in_maps, assign = _marshal_pe8(tokens, lengths, emb_table, W, b)
    nc = _build_pe8(Tw, chunk, gbufs, gmode=gmode, nq=nq)
    res = run_bass_kernel_spmd(nc, in_maps, core_ids=list(range(NCORES)), trace=trace)
    LAST["results"] = res
    LAST["T"] = sum(Tw)
    LAST["Vmax"] = VOCAB
    out = np.zeros(B, dtype=np.float32)
    for c in range(NCORES):
        vals = res.results[c]["out"].reshape(-1)
        for j in range(NSEQ):
            out[assign[c, j]] = vals[j]
    return out


# ---------------------------------------------------------------------------
# Vocab-sharded fp8 all-PE variant (BERT_SHARD=vp8): global dedup across all
# 64 sequences, unique rows split into 8 equal contiguous vocab chunks (each
# span < 32768 so int16 indices need no windows). Each core gathers ~U/8 rows
# (~7.1k vs ~9.5k for the seq split -- the Pool engine's descriptor-gen ucode
# at ~8.5ns/row is the wall, so fewer rows is the lever), accumulates
# pooled[64,1000] on the PE, dots with W, and an AllGather (warmed up early)
# combines the per-core [64] partials.
def _marshal_vp8(tokens, lengths, emb_table, W, b):
    import ml_dtypes

    f8 = ml_dtypes.float8_e4m3
    tokens = np.asarray(tokens)
    lengths = np.asarray(lengths).astype(np.int64)

    mask = np.arange(L)[None, :] < lengths[:, None]
    flat_tok = tokens[mask].astype(np.int64)
    flat_b = np.broadcast_to(np.arange(B)[:, None], (B, L))[mask]
    uniq, inv = np.unique(flat_tok, return_inverse=True)
    U = len(uniq)
    cnt = np.zeros((U, B), dtype=np.float32)
    np.add.at(cnt, (inv, flat_b), 1.0)

    bounds = [U * c // NCORES for c in range(NCORES + 1)]
    T = max(-(-(bounds[c + 1] - bounds[c]) // 128) for c in range(NCORES))
    spans = []
    for c in range(NCORES):
        s, e = bounds[c], bounds[c + 1]
        lo = int(uniq[s]) if e > s else 0
        hi = int(uniq[e - 1]) + 1 if e > s else 1
        assert hi - lo < 32768, f"core {c} vocab span {hi-lo} exceeds int16"
        spans.append((s, e, lo, hi))
    Vmax = max(hi - lo for _, _, lo, hi in spans)

    emb8 = np.zeros((VOCAB, PDIMP), dtype=f8)
    emb8[:, :PDIM] = np.ascontiguousarray(emb_table, dtype=np.float32).astype(f8)
    w64 = np.ascontiguousarray(
        np.broadcast_to(np.asarray(W, dtype=np.float32).reshape(1, PDIM), (B, PDIM))
    )
    brep = np.full((1, 1), np.float32(np.asarray(b).reshape(-1)[0]), dtype=np.float32)

    in_maps = []
    for c in range(NCORES):
        s, e, lo, hi = spans[c]
        emb_c = np.zeros((Vmax, PDIMP), dtype=f8)
        emb_c[: hi - lo] = emb8[lo:hi]
        rows = np.zeros(T * 128, dtype=np.int16)
        rows[: e - s] = (uniq[s:e] - lo).astype(np.int16)
        selm = np.zeros((T * 128, B), dtype=np.float32)
        selm[: e - s] = cnt[s:e]
        wrapped = rows.reshape(T * 8, 16).T  # [16, T*8]
        in_maps.append(
            {
                "emb": emb_c,
                "idx16": np.tile(wrapped, (8, 1)).copy(),
                "sel": selm.astype(f8)
                .reshape(T, 128, B)
                .transpose(1, 0, 2)
                .reshape(128, T * B)
                .copy(),
                "wrep": w64,
                "brep": brep,
            }
        )
    return T, Vmax, in_maps


def _build_vp8(T, Vmax, chunk, gbufs, legalize=True, ccwarm=True):
    from concourse import bacc, mybir
    import concourse.tile as tile

    F32 = mybir.dt.float32
    F8 = mybir.dt.float8e4
    I16 = mybir.dt.int16

    nc = bacc.Bacc(None, num_devices=NCORES)
    emb = nc.declare_dram_parameter("emb", [Vmax, PDIMP], F8, isOutput=False)
    idx16 = nc.declare_dram_parameter("idx16", [128, T * 8], I16, isOutput=False)
    sel = nc.declare_dram_parameter("sel", [128, T * B], F8, isOutput=False)
    wrep = nc.declare_dram_parameter("wrep", [B, PDIM], F32, isOutput=False)
    brep = nc.declare_dram_parameter("brep", [1, 1], F32, isOutput=False)
    outp = nc.declare_dram_parameter("out", [1, B], F32, isOutput=True)

    HALF = PDIM // 2
    with tile.TileContext(nc) as tc:
        with (
            tc.tile_pool(name="meta", bufs=1) as meta,
            tc.tile_pool(name="g", bufs=gbufs) as gp,
            tc.tile_pool(name="ps", bufs=1, space="PSUM") as pp,
            tc.tile_pool(name="dram", bufs=1, space="DRAM") as dp,
        ):
            idx16_sb = meta.tile([128, T * 8], I16)
            nc.sync.dma_start(out=idx16_sb[:], in_=idx16[:])
            sel_sb = meta.tile([128, T * B], F8)
            nc.sync.dma_start(out=sel_sb[:], in_=sel[:])
            w_sb = meta.tile([B, PDIM], F32)
            nc.sync.dma_start(out=w_sb[:], in_=wrep[:])
            b_sb = meta.tile([1, 1], F32)
            nc.sync.dma_start(out=b_sb[:], in_=brep[:])

            if ccwarm:
                warm_sb = meta.tile([1, 4], F32)
                nc.vector.memset(warm_sb[:], 0.0)
                ccw_in = dp.tile([1, 4], F32)
                ccw_out = dp.tile([NCORES, 4], F32)
                nc.sync.dma_start(out=ccw_in[:], in_=warm_sb[:])
                nc.gpsimd.collective_compute(
                    "AllGather",
                    mybir.AluOpType.bypass,
                    replica_groups=[list(range(NCORES))],
                    ins=[ccw_in[:]],
                    outs=[ccw_out[:]],
                )

            pool_a = pp.tile([B, HALF], F32, tag="pa")
            pool_b = pp.tile([B, HALF], F32, tag="pb")

            t = 0
            first_chunk = True
            while t < T:
                c = min(4 if first_chunk else chunk, T - t)
                first_chunk = False
                g = gp.tile([128, c, PDIMP], F8, tag="g")
                nc.gpsimd.dma_gather(
                    out_ap=g[:],
                    in_ap=emb[:],
                    idxs_ap=idx16_sb[:, t * 8 : (t + c) * 8],
                    num_idxs=c * 128,
                    num_idxs_reg=c * 128,
                    elem_size=PDIMP,
                )
                gflat = g[:].rearrange("p c e -> p (c e)")
                for j in range(c):
                    tt = t + j
                    off = j * PDIMP
                    nc.tensor.matmul(
                        out=pool_a[:],
                        lhsT=sel_sb[:, tt * B : (tt + 1) * B],
                        rhs=gflat[:, off : off + HALF],
                        start=(tt == 0),
                        stop=(tt == T - 1),
                    )
                    nc.tensor.matmul(
                        out=pool_b[:],
                        lhsT=sel_sb[:, tt * B : (tt + 1) * B],
                        rhs=gflat[:, off + HALF : off + PDIM],
                        start=(tt == 0),
                        stop=(tt == T - 1),
                    )
                t += c

            pooled_sb = meta.tile([B, PDIM], F32)
            nc.vector.tensor_copy(out=pooled_sb[:, :HALF], in_=pool_a[:])
            nc.vector.tensor_copy(out=pooled_sb[:, HALF:], in_=pool_b[:])
            scr = meta.tile([B, PDIM], F32)
            y64 = meta.tile([B, 1], F32)
            nc.vector.scalar_tensor_tensor(
                out=scr[:],
                in0=pooled_sb[:],
                scalar=1.0,
                in1=w_sb[:],
                op0=mybir.AluOpType.mult,
                op1=mybir.AluOpType.mult,
                accum_out=y64[:],
            )
            cc_in = dp.tile([B, 1], F32)
            nc.sync.dma_start(out=cc_in[:], in_=y64[:])
            cc_out = dp.tile([NCORES * B, 1], F32)
            nc.gpsimd.collective_compute(
                "AllGather",
                mybir.AluOpType.bypass,
                replica_groups=[list(range(NCORES))],
                ins=[cc_in[:]],
                outs=[cc_out[:]],
            )
            allg_sb = meta.tile([NCORES, B], F32)
            nc.sync.dma_start(
                out=allg_sb[:],
                in_=cc_out[:].rearrange("a b -> (a b)").rearrange("(c n) -> c n", c=NCORES),
            )
            ones_sb = meta.tile([NCORES, 1], F32)
            nc.vector.memset(ones_sb[:], 1.0)
            sum_ps = pp.tile([1, B], F32, tag="sum")
            nc.tensor.matmul(
                out=sum_ps[:],
                lhsT=ones_sb[:],
                rhs=allg_sb[:],
                start=True,
                stop=True,
            )
            o_sb = meta.tile([1, B], F32)
            nc.scalar.activation(
                out=o_sb[:],
                in_=sum_ps[:],
                func=mybir.ActivationFunctionType.Sigmoid,
                bias=b_sb[:],
                scale=1.0 / float(L),
            )
            nc.sync.dma_start(out=outp[:], in_=o_sb[:])

    nc.compile()
    if legalize:
        _legalize_sem_waits(nc, __import__("concourse.mybir", fromlist=["x"]))
    return nc


def _kernel_vp8(tokens, lengths, emb_table, W, b):
    from concourse.bass_utils import run_bass_kernel_spmd

    chunk = int(os.environ.get("BERT_CHUNK", "8"))
    gbufs = int(os.environ.get("BERT_GBUFS", "4"))
    ccwarm = os.environ.get("BERT_CCWARM", "1") == "1"
    trace = os.environ.get("BERT_TRACE", "0") == "1"

    T, Vmax, in_maps = _marshal_vp8(tokens, lengths, emb_table, W, b)
    nc = _build_vp8(T, Vmax, chunk, gbufs, ccwarm=ccwarm)
    res = run_bass_kernel_spmd(nc, in_maps, core_ids=list(range(NCORES)), trace=trace)
    LAST["results"] = res
    LAST["T"] = T
    LAST["Vmax"] = Vmax
    return res.results[0]["out"].reshape(B).astype(np.float32)


def kernel(tokens, lengths, emb_table, W, b):
    shard = os.environ.get("BERT_SHARD", "vp8")
    if shard == "vp8":
        return _kernel_vp8(tokens, lengths, emb_table, W, b)
    if shard == "pe8":
        return _kernel_pe8(tokens, lengths, emb_table, W, b)
    if shard == "seq":
        return _kernel_seq(tokens, lengths, emb_table, W, b)
    return _kernel_vocab(tokens, lengths, emb_table, W, b)

